# revision 1
# baseline (speedup 1.0000x reference)
"""Trainium2 Bass kernel for dual-attention block (CAM + SAM + bottleneck).

Contract: kernel(**inputs) takes FULL unsharded inputs
  x     [8, 64, 64, 64] f32
  w_cam [64, 64], w_q [32, 64], w_k [32, 64], w_v [64, 64], w_bn [64, 128]
and returns the full [8, 64, 64, 64] f32 output.

Sharding: data-parallel over batch across 8 NeuronCores (1 image each);
weights replicated. Per-core math (c=64 channels, n=m=4096 spatial):

  CAM: xcT = x.T @ w_cam.T ; Ec = xcT.T @ xcT;
       attn_c = softmax_rows(Ec); bn1 = (wbn1 @ attn_c) @ x   (folded M1)
  SAM: q4/k4 = (w stacked 4x) @ x  -> q,k replicated on 4 partition groups
       S[m,n] = sum_c k[c,m] q[c,n]  (row-tiled K=32 matmuls, 4-concurrent)
       E = exp(S - ln64) in fp8-e4m3  (max|S|=9.05 -> E'max 133 < 240;
           the 1/64 cancels between numerator and denominator)
       acc[c,n] = sum_m W[m,c] E[m,n]  with W = [v.T | ones] in fp8,
                  one DoubleRow matmul per m-tile PAIR (K=256 contraction)
                  -> rows 0..63 unnormalized out_s, row 64 = Z
  out = x + bn1 + (wbn2 @ acc[0:64]) * (1/Z)
        (per-n 1/Z broadcast to 64 partitions via a K=1 PE matmul)

Design (v4): ScalarE (ACT) is the bound -- 16.8M exp at 1/lane/cycle
~= 110us/core (143us with per-instr overhead).  The HAM clock gate
keeps a <100%-duty PE at 1.2 GHz, so the PE workload is cut (fp8
DoubleRow acc; bf16 wvc/ec) and S matmuls are emitted in GROUP PAIRS
so 4 K=32 matmuls run concurrently on disjoint row quadrants before
each full-array DR matmul -- cold-PE ~940ns/group < 1114ns exp pace.
Preamble: x DMA split across both HWDGE queues (sync+scalar), chunked
x_bf cast, q4/k4 PSUM evacuation alternating DVE/ACT -> first exp
~11us.  The 3.3us DVE reciprocal and the deferred epilogues are
slotted mid-block so they never head-of-line-block the PE/DVE FIFOs.
PSUM: spoolA(2) + spoolB(2) + vacc/EC(2) + ppool(2) = 8 banks.
"""

import sys
from contextlib import ExitStack

import numpy as np

if "/opt/trn_rl_repo" not in sys.path:
    sys.path.insert(0, "/opt/trn_rl_repo")

import concourse.bass as bass
import concourse.tile as tile
from concourse import bacc, mybir
from concourse.bass_utils import run_bass_kernel_spmd

F32 = mybir.dt.float32
BF16 = mybir.dt.bfloat16
FP8 = mybir.dt.float8e4

C = 64          # channels
HW = 4096       # 64*64 spatial
NB = 8          # number of 512-wide n blocks
BLK = 512
MT = 32         # m tiles of 128
NG = 16         # groups of 2 m-tiles per n-block
WP = 80         # wt8 per-m-tile stride (65 used; 80 for DoubleRow step%16==0)
NLOG64 = -4.1588830833596715

Exp = mybir.ActivationFunctionType.Exp
DR = mybir.MatmulPerfMode.DoubleRow


def _build_kernel(ctx: ExitStack, tc: tile.TileContext, io: dict):
    nc = tc.nc
    x_d = io["x"]
    out_d = io["out"]

    consts = ctx.enter_context(tc.tile_pool(name="consts", bufs=1))
    bigs = ctx.enter_context(tc.tile_pool(name="bigs", bufs=1))
    epool = ctx.enter_context(tc.tile_pool(name="epool", bufs=3))
    campool = ctx.enter_context(tc.tile_pool(name="campool", bufs=1))
    sampool = ctx.enter_context(tc.tile_pool(name="sampool", bufs=2))
    spoolA = ctx.enter_context(
        tc.tile_pool(name="spoolA", bufs=1, space=bass.MemorySpace.PSUM)
    )
    spoolB = ctx.enter_context(
        tc.tile_pool(name="spoolB", bufs=1, space=bass.MemorySpace.PSUM)
    )
    vpool = ctx.enter_context(
        tc.tile_pool(name="vpool", bufs=2, space=bass.MemorySpace.PSUM)
    )
    ppool = ctx.enter_context(
        tc.tile_pool(name="ppool", bufs=2, space=bass.MemorySpace.PSUM)
    )

    # ---- x DMA first, split across both HWDGE queues ----
    x_sb = bigs.tile([C, HW], F32)
    nc.sync.dma_start(x_sb[:, 0 : HW // 2], x_d[:, 0 : HW // 2])
    nc.scalar.dma_start(x_sb[:, HW // 2 :], x_d[:, HW // 2 :])

    # ---- constants ----
    wq4T = consts.tile([C, 128], BF16)    # (w_q stacked 4x).T
    wk4T = consts.tile([C, 128], BF16)
    wvc = consts.tile([C, 128], BF16)     # [v.T | w_cam.T]
    wbn1T = consts.tile([C, C], F32)
    wbn2T = consts.tile([C, C], BF16)
    ones_r = consts.tile([128, C], BF16)  # row 64 holds ones[1, 64]
    zb = consts.tile([128, 1], F32)
    nlog64 = consts.tile([128, 1], F32)   # exp bias: E'=E/64 fits fp8e4 max 240
    dummy = consts.tile([128, 1], F32)

    nc.vector.memset(zb[:], 0.0)
    # Trigger the exp ACT-table load right behind the x-DMA issue (overlaps
    # the transfer) instead of in front of the first real exp.
    nc.scalar.activation(dummy[:], zb[:], Exp, bias=zb[:])
    nc.sync.dma_start(wk4T[:], io["wk4T"][:])
    nc.sync.dma_start(wq4T[:], io["wq4T"][:])
    nc.scalar.dma_start(wvc[:], io["wvc"][:])
    nc.scalar.dma_start(wbn1T[:], io["wbn1T"][:])
    nc.scalar.dma_start(wbn2T[:], io["wbn2T"][:])
    nc.scalar.dma_start(ones_r[C : C + 1, :], io["ones64"][:])
    nc.vector.memset(nlog64[:], NLOG64)

    q4 = bigs.tile([128, HW], BF16)
    k4 = bigs.tile([128, HW], BF16)
    wt8 = bigs.tile([128, MT * WP], FP8)   # per m-tile [vT | ones | pad]
    xct = bigs.tile([128, MT * C], BF16)   # xcT, m-tile-major
    x_bf = bigs.tile([C, HW], BF16)

    # ones column of wt8 (wvc copies below only write cols 0..63)
    nc.vector.memset(
        wt8[:].rearrange("p (t c) -> p t c", c=WP)[:, :, 64:65], 1.0
    )

    # x in bf16 feeds the q4/k4/wvc/bn1 matmuls at full PE rate (2 chunks so
    # the first q/k matmuls start as soon as the first x half lands).
    nc.vector.tensor_copy(x_bf[:, 0 : HW // 2], x_sb[:, 0 : HW // 2])
    nc.vector.tensor_copy(x_bf[:, HW // 2 :], x_sb[:, HW // 2 :])

    # ---- q4 / k4: replicated q,k via stacked-weight 1x1 convs ----
    # k4 first (block 0 needs all of k4 but only q4's first chunk);
    # PSUM evacuation alternates DVE / ACT (ACT is idle in the preamble).
    def qk_group(wT, dst, chunks, pool, on_scalar):
        ps = pool.tile([128, 2 * BLK], F32, tag="s", name="qkps")
        for i, cch in enumerate(chunks):
            nc.tensor.matmul(
                ps[:, i * BLK : (i + 1) * BLK],
                wT[:],
                x_bf[:, cch * BLK : (cch + 1) * BLK],
                start=True,
                stop=True,
            )
        w = len(chunks) * BLK
        lo = chunks[0] * BLK
        if on_scalar:
            nc.scalar.copy(dst[:, lo : lo + w], ps[:, :w])
        else:
            nc.vector.tensor_copy(dst[:, lo : lo + w], ps[:, :w])

    qk_group(wk4T, k4, [0, 1], spoolA, False)
    qk_group(wq4T, q4, [0, 1], spoolB, True)
    qk_group(wk4T, k4, [2, 3], spoolA, False)
    qk_group(wk4T, k4, [4, 5], spoolB, True)
    qk_group(wk4T, k4, [6, 7], spoolA, False)
    qk_group(wq4T, q4, [2, 3], spoolB, True)
    qk_group(wq4T, q4, [4, 5], spoolA, False)
    qk_group(wq4T, q4, [6, 7], spoolB, True)

    state = {}  # EC tile, allocated at block 1 start (vpool slot timing)

    def wvc_group(base, size):
        """xcT and WT (=[vT|ones]) production for one m-tile group."""
        ps_w = ppool.tile([128, BLK], F32, tag="p", name="wvcps")
        for j in range(size):
            m = base + j
            nc.tensor.matmul(
                ps_w[:, j * 128 : (j + 1) * 128],
                x_bf[:, m * 128 : (m + 1) * 128],
                wvc[:],
                start=True,
                stop=True,
            )
        src = ps_w[:, : size * 128].rearrange("p (j c) -> p j c", c=128)
        wt_dst = wt8[:, base * WP : (base + size) * WP].rearrange(
            "p (j c) -> p j c", c=WP
        )
        with nc.allow_low_precision(reason="v in fp8 for DoubleRow acc"):
            nc.vector.tensor_copy(wt_dst[:, :, 0:C], src[:, :, 0:C])
        xct_dst = xct[:, base * C : (base + size) * C].rearrange(
            "p (j c) -> p j c", c=C
        )
        with nc.allow_low_precision(reason="xcT in bf16 for cheap ec matmuls"):
            nc.vector.tensor_copy(xct_dst, src[:, :, C : 2 * C])

    def ec_group(base, size):
        EC = state["EC"]
        for j in range(size):
            m = base + j
            nc.tensor.matmul(
                EC[0:C, 0:C],
                xct[:, m * C : (m + 1) * C],
                xct[:, m * C : (m + 1) * C],
                start=(m == 0),
                stop=(m == MT - 1),
            )

    # ---- per-block state for split epilogues ----
    vaccs = [None] * NB
    sam = [None] * NB   # sam65 [65, BLK] f32: rows 0..63 unnorm out_s, 64 = Z
    rzs = [None] * NB   # rz [65, BLK] bf16: row 64 = 1/Z
    M1T_sb = campool.tile([C, C], BF16)

    def epilogue_a(nb):
        """At block end: evacuate vacc (recip is emitted separately)."""
        aux = sampool.tile([C + 1, BLK], F32, tag="aux", name="aux")
        nc.vector.tensor_copy(aux[:], vaccs[nb][0 : C + 1, :])
        sam[nb] = aux

    def emit_recip(nb):
        """1/Z for block nb; emitted at a quiet mid-block DVE slot."""
        rzb = sampool.tile([C + 1, BLK], BF16, tag="rz", name="rzb")
        with nc.allow_low_precision(reason="1/Z in bf16: 0.4% on the SAM term"):
            nc.vector.reciprocal(rzb[C : C + 1, :], sam[nb][C : C + 1, :])
        rzs[nb] = rzb

    def epilogue_b1(nb):
        """Broadcast 1/Z to 64 partitions and scale the SAM accumulator."""
        bc = ppool.tile([128, BLK], F32, tag="p", name="bc")
        nc.tensor.matmul(
            bc[0:C, :],
            ones_r[C : C + 1, 0:C],
            rzs[nb][C : C + 1, :],
            start=True,
            stop=True,
            tile_position=(C, 0),
        )
        sam_sc = sampool.tile([C, BLK], BF16, tag="sc", name="sam_sc")
        nc.vector.tensor_mul(sam_sc[:], sam[nb][0:C, :], bc[0:C, :])
        return sam_sc

    def epilogue_b2(nb, sam_sc):
        """Bottleneck conv, residual add, DMA out."""
        ncol = slice(nb * BLK, (nb + 1) * BLK)
        bn = ppool.tile([128, BLK], F32, tag="p", name="bn")
        nc.tensor.matmul(
            bn[0:C, :], M1T_sb[:], x_bf[:, ncol], start=True, stop=False
        )
        nc.tensor.matmul(
            bn[0:C, :], wbn2T[:], sam_sc[:], start=False, stop=True
        )
        o_t = sampool.tile([C, BLK], F32, tag="ot", name="o_t")
        nc.vector.tensor_add(o_t[:], x_sb[:, ncol], bn[0:C, :])
        nc.sync.dma_start(out_d[:, ncol], o_t[:])

    def cam_chain():
        """CAM softmax -> attn_c -> M1T = (wbn1 @ attn_c).T"""
        EC = state["EC"]
        negmax = campool.tile([C, 1], F32)
        nc.vector.reduce_max(
            negmax[:], EC[0:C, 0:C], axis=mybir.AxisListType.X, negate=True
        )
        exp_c = campool.tile([C, C], F32)
        nc.scalar.activation(exp_c[:], EC[0:C, 0:C], Exp, bias=negmax[:])
        sum_c = campool.tile([C, 1], F32)
        nc.vector.reduce_sum(sum_c[:], exp_c[:], axis=mybir.AxisListType.X)
        rec_c = campool.tile([C, 1], F32)
        nc.vector.reciprocal(rec_c[:], sum_c[:])
        attn_c = campool.tile([C, C], F32)
        nc.vector.tensor_scalar_mul(attn_c[:], exp_c[:], rec_c[:])
        m1ps = ppool.tile([128, BLK], F32, tag="p", name="m1ps")
        nc.tensor.matmul(
            m1ps[0:C, 0:C], attn_c[:], wbn1T[:], start=True, stop=True
        )
        nc.vector.tensor_copy(M1T_sb[:], m1ps[0:C, 0:C])

    # ---- main SAM loop over 8 n-blocks, groups emitted in PAIRS ----
    sc_pend = {}  # nb -> sam_sc awaiting epilogue_b2
    for nb in range(NB):
        ncol = slice(nb * BLK, (nb + 1) * BLK)
        if nb == 1:
            # EC takes a vpool rotation slot; its last readers (CAM softmax,
            # start of block 2) finish before vacc(2) re-claims the slot.
            state["EC"] = vpool.tile([128, BLK], F32, tag="v", name="EC")
        if nb == 2:
            # CAM chain first so vacc(2), which aliases EC's bank, only
            # waits on the (fast) softmax reads of EC.
            cam_chain()
        vacc = vpool.tile([128, BLK], F32, tag="v", name="vacc")
        vaccs[nb] = vacc
        for p in range(NG // 2):
            g0, g1 = 2 * p, 2 * p + 1
            s_ts = []
            for g in (g0, g1):
                pool = spoolA if g % 2 == 0 else spoolB
                s_t = pool.tile([128, 2 * BLK], F32, tag="s", name="s_t")
                s_ts.append(s_t)
                for j in range(2):
                    m = 2 * g + j
                    r = 2 * (g % 2) + j  # row quadrants 0,1 / 2,3
                    nc.tensor.matmul(
                        s_t[:, j * BLK : (j + 1) * BLK],
                        k4[32 * r : 32 * r + 32, m * 128 : (m + 1) * 128],
                        q4[32 * r : 32 * r + 32, ncol],
                        start=True,
                        stop=True,
                        tile_position=(32 * r, 0),
                    )
            if nb == 0:
                wvc_group(2 * g0, 2)
                wvc_group(2 * g1, 2)
            if nb == 1:
                ec_group(2 * g0, 2)
                ec_group(2 * g1, 2)
            e_ts = []
            for g, s_t in zip((g0, g1), s_ts):
                e_t = epool.tile([128, 2 * BLK], FP8, tag="e", name="e_t")
                e_ts.append(e_t)
                with nc.allow_low_precision(reason="E in fp8: ~1e-4 on out"):
                    nc.scalar.activation(e_t[:], s_t[:], Exp, bias=nlog64[:])
            for g, e_t in zip((g0, g1), e_ts):
                lhsT = wt8[:, 2 * g * WP : (2 * g + 2) * WP].rearrange(
                    "p (two f) -> p two f", two=2
                )[:, :, 0:65]
                rhs = e_t[:].rearrange("p (two f) -> p two f", two=2)
                nc.tensor.matmul(
                    vacc[0 : C + 1, :],
                    lhsT,
                    rhs,
                    start=(g == 0),
                    stop=(g == NG - 1),
                    perf_mode=DR,
                )
            # deferred work, slotted into quiet spots mid-block:
            if nb >= 1 and p == 0:
                # recip(nb-1) FIRST on this block's DVE FIFO: it has no
                # unmet deps, so the 3.3us op never head-of-line-blocks
                # the epilogue chain behind a PE round-trip.
                emit_recip(nb - 1)
            if nb >= 2 and p == 1:
                sc_pend[nb - 2] = epilogue_b1(nb - 2)
            if nb >= 2 and p == 3:
                epilogue_b2(nb - 2, sc_pend.pop(nb - 2))

        epilogue_a(nb)
    # ---- tail: last two blocks' epilogues ----
    sc6 = epilogue_b1(NB - 2)
    epilogue_b2(NB - 2, sc6)
    emit_recip(NB - 1)
    sc7 = epilogue_b1(NB - 1)
    epilogue_b2(NB - 1, sc7)


def build_nc():
    nc = bacc.Bacc(
        "TRN2",
        target_bir_lowering=False,
        debug=False,
        enable_asserts=False,
        num_devices=8,
    )
    io = {}
    io["x"] = nc.dram_tensor("x", [C, HW], F32, kind="ExternalInput").ap()
    io["wq4T"] = nc.dram_tensor("wq4T", [C, 128], BF16, kind="ExternalInput").ap()
    io["wk4T"] = nc.dram_tensor("wk4T", [C, 128], BF16, kind="ExternalInput").ap()
    io["wvc"] = nc.dram_tensor("wvc", [C, 128], BF16, kind="ExternalInput").ap()
    io["wbn1T"] = nc.dram_tensor("wbn1T", [C, C], F32, kind="ExternalInput").ap()
    io["wbn2T"] = nc.dram_tensor("wbn2T", [C, C], BF16, kind="ExternalInput").ap()
    io["ones64"] = nc.dram_tensor("ones64", [1, C], BF16, kind="ExternalInput").ap()
    io["out"] = nc.dram_tensor("out", [C, HW], F32, kind="ExternalOutput").ap()

    with tile.TileContext(nc) as tc:
        with ExitStack() as ctx:
            _build_kernel(ctx, tc, io)
    nc.compile()
    return nc


def make_in_maps(x, w_cam, w_q, w_k, w_v, w_bn):
    import ml_dtypes

    f = lambda a: np.ascontiguousarray(np.asarray(a, dtype=np.float32))
    fb = lambda a: np.ascontiguousarray(
        np.asarray(a, dtype=np.float32).astype(ml_dtypes.bfloat16)
    )
    base = {
        "wq4T": fb(np.concatenate([np.asarray(w_q).T] * 4, axis=1)),
        "wk4T": fb(np.concatenate([np.asarray(w_k).T] * 4, axis=1)),
        "wvc": fb(np.concatenate([np.asarray(w_v).T, np.asarray(w_cam).T], axis=1)),
        "wbn1T": f(np.asarray(w_bn)[:, :C].T),
        "wbn2T": fb(np.asarray(w_bn)[:, C:].T),
        "ones64": fb(np.ones((1, C))),
    }
    x = np.asarray(x)
    return [dict(base, x=f(x[b].reshape(C, HW))) for b in range(8)]


_NC_CACHE = None


def kernel(x, w_cam, w_q, w_k, w_v, w_bn):
    global _NC_CACHE
    if _NC_CACHE is None:
        _NC_CACHE = build_nc()
    nc = _NC_CACHE
    in_maps = make_in_maps(x, w_cam, w_q, w_k, w_v, w_bn)
    res = run_bass_kernel_spmd(nc, in_maps, list(range(8)))
    out = np.stack([res.results[b]["out"].reshape(C, 64, 64) for b in range(8)])
    return out.astype(np.float32)



# revision 6
# speedup vs baseline: 1.0692x; 1.0692x over previous
"""Trainium2 Bass kernel for dual-attention block (CAM + SAM + bottleneck).

Contract: kernel(**inputs) takes FULL unsharded inputs
  x     [8, 64, 64, 64] f32
  w_cam [64, 64], w_q [32, 64], w_k [32, 64], w_v [64, 64], w_bn [64, 128]
and returns the full [8, 64, 64, 64] f32 output.

Sharding: data-parallel over batch across 8 NeuronCores (1 image each);
weights replicated. Per-core math (c=64 channels, n=m=4096 spatial):

  CAM: xcT = x.T @ w_cam.T ; Ec = xcT.T @ xcT;
       attn_c = softmax_rows(Ec); bn1 = (wbn1 @ attn_c) @ x   (folded M1)
  SAM: q4/k4 = (w stacked 4x) @ x  -> q,k replicated on 4 partition groups
       S[m,n] = sum_c k[c,m] q[c,n]  (row-tiled K=32 matmuls, 4-concurrent)
       E = exp(S - ln64) in fp8-e4m3  (max|S|=9.05 -> E'max 133 < 240;
           the 1/64 cancels between numerator and denominator)
       acc[c,n] = sum_m W[m,c] E[m,n]  with W = [v.T | ones] in fp8,
                  one DoubleRow matmul per m-tile PAIR (K=256 contraction)
                  -> rows 0..63 unnormalized out_s, row 64 = Z
  out = x + bn1 + (wbn2 @ acc[0:64]) * (1/Z)
        (per-n 1/Z broadcast to 64 partitions via a K=1 PE matmul)

v5 changes over v4 (which measured 221us):
  - 1/Z via reciprocal_approx_fast (custom DVE op, ~5x faster than the
    8-cycle iterative divide) + a tiny bf16 cast.  The old 3.0us serial
    reciprocal sat in the in-order DVE queue and head-of-line blocked the
    bc matmul -> PE -> next block's exps for a constant 3.45us at EVERY
    block boundary.
  - Preamble restructured: x DMA + bf16 cast in 4 column chunks across
    both HWDGE queues; only q/k chunks 0-1 are produced up front, so the
    first SAM exp fires at ~5us instead of 25us.  The remaining 12 q/k
    chunks are emitted as single-matmul fill-in groups spread over blocks
    0-2 (each chunk 2 pairs ahead of its consumption deadline).
  - Only the q01 PSUM evacuation runs on ScalarE; everything else
    evacuates on DVE so ACT time is exp + one copy.
  - Dense back-to-back preamble matmuls may flip the PE HAM clock gate
    to 2.4 GHz before the steady loop starts.
PSUM: spoolA(2) + spoolB(2) + vacc/EC(2) + ppool(2) = 8 banks.
"""

import sys
from contextlib import ExitStack

import numpy as np

if "/opt/trn_rl_repo" not in sys.path:
    sys.path.insert(0, "/opt/trn_rl_repo")

import concourse.bass as bass
import concourse.tile as tile
from concourse import bacc, mybir
from concourse.bass_utils import run_bass_kernel_spmd

F32 = mybir.dt.float32
BF16 = mybir.dt.bfloat16
FP8 = mybir.dt.float8e4

C = 64          # channels
HW = 4096       # 64*64 spatial
NB = 8          # number of 512-wide n blocks
BLK = 512
MT = 32         # m tiles of 128
NG = 16         # groups of 2 m-tiles per n-block
WP = 80         # wt8 per-m-tile stride (65 used; 80 for DoubleRow step%16==0)
NLOG64 = -4.1588830833596715

Exp = mybir.ActivationFunctionType.Exp
DR = mybir.MatmulPerfMode.DoubleRow


def _build_kernel(ctx: ExitStack, tc: tile.TileContext, io: dict):
    nc = tc.nc
    x_d = io["x"]
    out_d = io["out"]

    consts = ctx.enter_context(tc.tile_pool(name="consts", bufs=1))
    bigs = ctx.enter_context(tc.tile_pool(name="bigs", bufs=1))
    epool = ctx.enter_context(tc.tile_pool(name="epool", bufs=3))
    campool = ctx.enter_context(tc.tile_pool(name="campool", bufs=1))
    sampool = ctx.enter_context(tc.tile_pool(name="sampool", bufs=2))
    spoolA = ctx.enter_context(
        tc.tile_pool(name="spoolA", bufs=1, space=bass.MemorySpace.PSUM)
    )
    spoolB = ctx.enter_context(
        tc.tile_pool(name="spoolB", bufs=1, space=bass.MemorySpace.PSUM)
    )
    vpool = ctx.enter_context(
        tc.tile_pool(name="vpool", bufs=2, space=bass.MemorySpace.PSUM)
    )
    ppool = ctx.enter_context(
        tc.tile_pool(name="ppool", bufs=2, space=bass.MemorySpace.PSUM)
    )

    # ---- x DMA first: 4 column chunks split across both HWDGE queues so
    # the first 1024 columns (all that q01/k01 need) land in ~0.8us ----
    x_sb = bigs.tile([C, HW], F32)
    XQ = HW // 4
    nc.sync.dma_start(x_sb[:, 0:XQ], x_d[:, 0:XQ])
    nc.scalar.dma_start(x_sb[:, XQ : 2 * XQ], x_d[:, XQ : 2 * XQ])
    nc.sync.dma_start(x_sb[:, 2 * XQ : 3 * XQ], x_d[:, 2 * XQ : 3 * XQ])
    nc.scalar.dma_start(x_sb[:, 3 * XQ :], x_d[:, 3 * XQ :])

    # ---- constants ----
    wq4T = consts.tile([C, 128], BF16)    # (w_q stacked 4x).T
    wk4T = consts.tile([C, 128], BF16)
    wvc = consts.tile([C, 128], BF16)     # [v.T | w_cam.T]
    wbn1T = consts.tile([C, C], F32)
    wbn2T = consts.tile([C, C], BF16)
    ones_r = consts.tile([128, C], BF16)  # row 64 holds ones[1, 64]
    zb = consts.tile([128, 1], F32)
    nlog64 = consts.tile([128, 1], F32)   # exp bias: E'=E/64 fits fp8e4 max 240
    dummy = consts.tile([128, 1], F32)

    nc.vector.memset(zb[:], 0.0)
    # Trigger the exp ACT-table load right behind the x-DMA issue (overlaps
    # the transfer) instead of in front of the first real exp.
    nc.scalar.activation(dummy[:], zb[:], Exp, bias=zb[:])
    nc.sync.dma_start(wk4T[:], io["wk4T"][:])
    nc.sync.dma_start(wq4T[:], io["wq4T"][:])
    nc.scalar.dma_start(wvc[:], io["wvc"][:])
    nc.scalar.dma_start(wbn1T[:], io["wbn1T"][:])
    nc.scalar.dma_start(wbn2T[:], io["wbn2T"][:])
    nc.scalar.dma_start(ones_r[0:1, :], io["ones64"][:])
    nc.vector.memset(nlog64[:], NLOG64)

    q4 = bigs.tile([128, HW], BF16)
    k4 = bigs.tile([128, HW], BF16)
    wt8 = bigs.tile([128, MT * WP], FP8)   # per m-tile [vT | ones | pad]
    xct = bigs.tile([128, MT * C], BF16)   # xcT, m-tile-major
    x_bf = bigs.tile([C, HW], BF16)

    # ones column of wt8 (wvc copies below only write cols 0..63)
    nc.vector.memset(
        wt8[:].rearrange("p (t c) -> p t c", c=WP)[:, :, 64:65], 1.0
    )

    # x in bf16 feeds the q4/k4/wvc/bn1 matmuls at full PE rate; 4 chunks so
    # the first q/k matmuls start as soon as the first x quarter lands.
    for xc_ in range(4):
        nc.vector.tensor_copy(
            x_bf[:, xc_ * XQ : (xc_ + 1) * XQ], x_sb[:, xc_ * XQ : (xc_ + 1) * XQ]
        )

    # ---- q4 / k4: replicated q,k via stacked-weight 1x1 convs ----
    # Only chunks 0-1 of k and q are computed up front (all that block 0
    # pair 0 needs); the rest are emitted as fill-in singles inside the
    # block loop, each 2+ pairs ahead of its consumption deadline.
    def qk_group(wT, dst, chunks, pool, on_scalar):
        ps = pool.tile([128, 2 * BLK], F32, tag="s", name="qkps")
        for i, cch in enumerate(chunks):
            nc.tensor.matmul(
                ps[:, i * BLK : (i + 1) * BLK],
                wT[:],
                x_bf[:, cch * BLK : (cch + 1) * BLK],
                start=True,
                stop=True,
            )
        w = len(chunks) * BLK
        lo = chunks[0] * BLK
        if on_scalar:
            nc.scalar.copy(dst[:, lo : lo + w], ps[:, :w])
        else:
            nc.vector.tensor_copy(dst[:, lo : lo + w], ps[:, :w])

    qk_group(wk4T, k4, [0, 1], spoolA, False)
    qk_group(wq4T, q4, [0, 1], spoolB, True)

    def qk_single(which, cch):
        """One q/k chunk: 1 matmul into a ppool bank + DVE evacuation."""
        wT, dst = (wk4T, k4) if which == "k" else (wq4T, q4)
        ps = ppool.tile([128, BLK], F32, tag="p", name="qks")
        nc.tensor.matmul(
            ps[:], wT[:], x_bf[:, cch * BLK : (cch + 1) * BLK],
            start=True, stop=True,
        )
        nc.vector.tensor_copy(dst[:, cch * BLK : (cch + 1) * BLK], ps[:])

    # (block, pair) -> (which, chunk); deadlines: k chunk c consumed at
    # block-0 pair c; q chunk c consumed at block c pair 0.
    qk_fill = {
        (0, 0): ("k", 2), (0, 1): ("k", 3), (0, 2): ("k", 4),
        (0, 3): ("k", 5), (0, 4): ("k", 6), (0, 5): ("k", 7),
        (1, 0): ("q", 2), (1, 2): ("q", 3), (1, 4): ("q", 4),
        (1, 6): ("q", 5), (2, 0): ("q", 6), (2, 2): ("q", 7),
    }

    state = {}  # EC tile, allocated at block 1 start (vpool slot timing)

    def wvc_group(base, size):
        """xcT and WT (=[vT|ones]) production for one m-tile group."""
        ps_w = ppool.tile([128, BLK], F32, tag="p", name="wvcps")
        for j in range(size):
            m = base + j
            nc.tensor.matmul(
                ps_w[:, j * 128 : (j + 1) * 128],
                x_bf[:, m * 128 : (m + 1) * 128],
                wvc[:],
                start=True,
                stop=True,
            )
        src = ps_w[:, : size * 128].rearrange("p (j c) -> p j c", c=128)
        wt_dst = wt8[:, base * WP : (base + size) * WP].rearrange(
            "p (j c) -> p j c", c=WP
        )
        with nc.allow_low_precision(reason="v in fp8 for DoubleRow acc"):
            nc.vector.tensor_copy(wt_dst[:, :, 0:C], src[:, :, 0:C])
        xct_dst = xct[:, base * C : (base + size) * C].rearrange(
            "p (j c) -> p j c", c=C
        )
        with nc.allow_low_precision(reason="xcT in bf16 for cheap ec matmuls"):
            nc.vector.tensor_copy(xct_dst, src[:, :, C : 2 * C])

    def ec_group(base, size):
        EC = state["EC"]
        for j in range(size):
            m = base + j
            nc.tensor.matmul(
                EC[0:C, 0:C],
                xct[:, m * C : (m + 1) * C],
                xct[:, m * C : (m + 1) * C],
                start=(m == 0),
                stop=(m == MT - 1),
            )

    # ---- per-block state for split epilogues ----
    vaccs = [None] * NB
    sam = [None] * NB   # sam65 [65, BLK] f32: rows 0..63 unnorm out_s, 64 = Z
    rzs = [None] * NB   # rz [65, BLK] bf16: row 64 = 1/Z
    M1T_sb = campool.tile([C, C], BF16)

    def epilogue_a(nb):
        """At block end: evacuate vacc (recip is emitted separately)."""
        aux = sampool.tile([C + 1, BLK], F32, tag="aux", name="aux")
        nc.vector.tensor_copy(aux[:], vaccs[nb][0 : C + 1, :])
        sam[nb] = aux

    def emit_recip(nb):
        """1/Z for block nb via the fast approx recip + bf16 cast.

        The custom DVE op only works at base partition 0 (and DVE lanes
        cannot move data across partitions), so the Z row is first moved
        from partition 64 to partition 0 by a tiny SBUF->SBUF DMA on the
        otherwise-idle sync queue.
        """
        z0 = sampool.tile([1, BLK], F32, tag="z0", name="z0")
        nc.sync.dma_start(z0[:], sam[nb][C : C + 1, :])
        rz32 = sampool.tile([1, BLK], F32, tag="rz32", name="rz32")
        nc.vector.reciprocal_approx_fast(rz32[:], z0[:])
        rzb = sampool.tile([1, BLK], BF16, tag="rz", name="rzb")
        with nc.allow_low_precision(reason="1/Z in bf16: 0.4% on the SAM term"):
            nc.vector.tensor_copy(rzb[:], rz32[:])
        rzs[nb] = rzb

    def epilogue_b1(nb):
        """Broadcast 1/Z to 64 partitions and scale the SAM accumulator."""
        bc = ppool.tile([128, BLK], F32, tag="p", name="bc")
        nc.tensor.matmul(
            bc[0:C, :],
            ones_r[0:1, 0:C],
            rzs[nb][:],
            start=True,
            stop=True,
            tile_position=(0, 0),
        )
        sam_sc = sampool.tile([C, BLK], BF16, tag="sc", name="sam_sc")
        nc.vector.tensor_mul(sam_sc[:], sam[nb][0:C, :], bc[0:C, :])
        return sam_sc

    def epilogue_b2(nb, sam_sc):
        """Bottleneck conv, residual add, DMA out."""
        ncol = slice(nb * BLK, (nb + 1) * BLK)
        bn = ppool.tile([128, BLK], F32, tag="p", name="bn")
        nc.tensor.matmul(
            bn[0:C, :], M1T_sb[:], x_bf[:, ncol], start=True, stop=False
        )
        nc.tensor.matmul(
            bn[0:C, :], wbn2T[:], sam_sc[:], start=False, stop=True
        )
        o_t = sampool.tile([C, BLK], F32, tag="ot", name="o_t")
        nc.vector.tensor_add(o_t[:], x_sb[:, ncol], bn[0:C, :])
        nc.sync.dma_start(out_d[:, ncol], o_t[:])

    def cam_chain():
        """CAM softmax -> attn_c -> M1T = (wbn1 @ attn_c).T"""
        EC = state["EC"]
        negmax = campool.tile([C, 1], F32)
        nc.vector.reduce_max(
            negmax[:], EC[0:C, 0:C], axis=mybir.AxisListType.X, negate=True
        )
        exp_c = campool.tile([C, C], F32)
        nc.scalar.activation(exp_c[:], EC[0:C, 0:C], Exp, bias=negmax[:])
        sum_c = campool.tile([C, 1], F32)
        nc.vector.reduce_sum(sum_c[:], exp_c[:], axis=mybir.AxisListType.X)
        rec_c = campool.tile([C, 1], F32)
        nc.vector.reciprocal(rec_c[:], sum_c[:])
        attn_c = campool.tile([C, C], F32)
        nc.vector.tensor_scalar_mul(attn_c[:], exp_c[:], rec_c[:])
        m1ps = ppool.tile([128, BLK], F32, tag="p", name="m1ps")
        nc.tensor.matmul(
            m1ps[0:C, 0:C], attn_c[:], wbn1T[:], start=True, stop=True
        )
        nc.vector.tensor_copy(M1T_sb[:], m1ps[0:C, 0:C])

    # ---- main SAM loop over 8 n-blocks, groups emitted in PAIRS ----
    sc_pend = {}  # nb -> sam_sc awaiting epilogue_b2
    for nb in range(NB):
        ncol = slice(nb * BLK, (nb + 1) * BLK)
        if nb == 1:
            # EC takes a vpool rotation slot; its last readers (CAM softmax,
            # start of block 2) finish before vacc(2) re-claims the slot.
            state["EC"] = vpool.tile([128, BLK], F32, tag="v", name="EC")
        if nb == 2:
            # CAM chain first so vacc(2), which aliases EC's bank, only
            # waits on the (fast) softmax reads of EC.
            cam_chain()
        vacc = vpool.tile([128, BLK], F32, tag="v", name="vacc")
        vaccs[nb] = vacc
        for p in range(NG // 2):
            g0, g1 = 2 * p, 2 * p + 1
            s_ts = []
            for g in (g0, g1):
                pool = spoolA if g % 2 == 0 else spoolB
                s_t = pool.tile([128, 2 * BLK], F32, tag="s", name="s_t")
                s_ts.append(s_t)
                for j in range(2):
                    m = 2 * g + j
                    r = 2 * (g % 2) + j  # row quadrants 0,1 / 2,3
                    nc.tensor.matmul(
                        s_t[:, j * BLK : (j + 1) * BLK],
                        k4[32 * r : 32 * r + 32, m * 128 : (m + 1) * 128],
                        q4[32 * r : 32 * r + 32, ncol],
                        start=True,
                        stop=True,
                        tile_position=(32 * r, 0),
                    )
            if nb == 0:
                wvc_group(2 * g0, 2)
                wvc_group(2 * g1, 2)
            if nb == 1:
                ec_group(2 * g0, 2)
                ec_group(2 * g1, 2)
            if (nb, p) in qk_fill:
                qk_single(*qk_fill[(nb, p)])
            e_ts = []
            for g, s_t in zip((g0, g1), s_ts):
                e_t = epool.tile([128, 2 * BLK], FP8, tag="e", name="e_t")
                e_ts.append(e_t)
                with nc.allow_low_precision(reason="E in fp8: ~1e-4 on out"):
                    nc.scalar.activation(e_t[:], s_t[:], Exp, bias=nlog64[:])
            for g, e_t in zip((g0, g1), e_ts):
                lhsT = wt8[:, 2 * g * WP : (2 * g + 2) * WP].rearrange(
                    "p (two f) -> p two f", two=2
                )[:, :, 0:65]
                rhs = e_t[:].rearrange("p (two f) -> p two f", two=2)
                nc.tensor.matmul(
                    vacc[0 : C + 1, :],
                    lhsT,
                    rhs,
                    start=(g == 0),
                    stop=(g == NG - 1),
                    perf_mode=DR,
                )
            # deferred work, slotted into quiet spots mid-block:
            if nb >= 1 and p == 0:
                # recip(nb-1) FIRST on this block's DVE FIFO; with the
                # approx-fast recip this is ~1us of DVE, not 3.
                emit_recip(nb - 1)
            if nb >= 2 and p == 1:
                sc_pend[nb - 2] = epilogue_b1(nb - 2)
            if nb >= 2 and p == 3:
                epilogue_b2(nb - 2, sc_pend.pop(nb - 2))

        epilogue_a(nb)
    # ---- tail: last two blocks' epilogues ----
    sc6 = epilogue_b1(NB - 2)
    epilogue_b2(NB - 2, sc6)
    emit_recip(NB - 1)
    sc7 = epilogue_b1(NB - 1)
    epilogue_b2(NB - 1, sc7)


def build_nc():
    nc = bacc.Bacc(
        "TRN2",
        target_bir_lowering=False,
        debug=False,
        enable_asserts=False,
        num_devices=8,
    )
    io = {}
    io["x"] = nc.dram_tensor("x", [C, HW], F32, kind="ExternalInput").ap()
    io["wq4T"] = nc.dram_tensor("wq4T", [C, 128], BF16, kind="ExternalInput").ap()
    io["wk4T"] = nc.dram_tensor("wk4T", [C, 128], BF16, kind="ExternalInput").ap()
    io["wvc"] = nc.dram_tensor("wvc", [C, 128], BF16, kind="ExternalInput").ap()
    io["wbn1T"] = nc.dram_tensor("wbn1T", [C, C], F32, kind="ExternalInput").ap()
    io["wbn2T"] = nc.dram_tensor("wbn2T", [C, C], BF16, kind="ExternalInput").ap()
    io["ones64"] = nc.dram_tensor("ones64", [1, C], BF16, kind="ExternalInput").ap()
    io["out"] = nc.dram_tensor("out", [C, HW], F32, kind="ExternalOutput").ap()

    with tile.TileContext(nc) as tc:
        with ExitStack() as ctx:
            _build_kernel(ctx, tc, io)
    nc.compile()
    return nc


def make_in_maps(x, w_cam, w_q, w_k, w_v, w_bn):
    import ml_dtypes

    f = lambda a: np.ascontiguousarray(np.asarray(a, dtype=np.float32))
    fb = lambda a: np.ascontiguousarray(
        np.asarray(a, dtype=np.float32).astype(ml_dtypes.bfloat16)
    )
    base = {
        "wq4T": fb(np.concatenate([np.asarray(w_q).T] * 4, axis=1)),
        "wk4T": fb(np.concatenate([np.asarray(w_k).T] * 4, axis=1)),
        "wvc": fb(np.concatenate([np.asarray(w_v).T, np.asarray(w_cam).T], axis=1)),
        "wbn1T": f(np.asarray(w_bn)[:, :C].T),
        "wbn2T": fb(np.asarray(w_bn)[:, C:].T),
        "ones64": fb(np.ones((1, C))),
    }
    x = np.asarray(x)
    return [dict(base, x=f(x[b].reshape(C, HW))) for b in range(8)]


_NC_CACHE = None


def kernel(x, w_cam, w_q, w_k, w_v, w_bn):
    global _NC_CACHE
    if _NC_CACHE is None:
        _NC_CACHE = build_nc()
    nc = _NC_CACHE
    in_maps = make_in_maps(x, w_cam, w_q, w_k, w_v, w_bn)
    res = run_bass_kernel_spmd(nc, in_maps, list(range(8)))
    out = np.stack([res.results[b]["out"].reshape(C, 64, 64) for b in range(8)])
    return out.astype(np.float32)


# revision 8
# speedup vs baseline: 1.0721x; 1.0027x over previous
"""Trainium2 Bass kernel for dual-attention block (CAM + SAM + bottleneck).

Contract: kernel(**inputs) takes FULL unsharded inputs
  x     [8, 64, 64, 64] f32
  w_cam [64, 64], w_q [32, 64], w_k [32, 64], w_v [64, 64], w_bn [64, 128]
and returns the full [8, 64, 64, 64] f32 output.

Sharding: data-parallel over batch across 8 NeuronCores (1 image each);
weights replicated. Per-core math (c=64 channels, n=m=4096 spatial):

  CAM: xcT = x.T @ w_cam.T ; Ec = xcT.T @ xcT;
       attn_c = softmax_rows(Ec); bn1 = (wbn1 @ attn_c) @ x   (folded M1)
  SAM: q4/k4 = (w stacked 4x) @ x  -> q,k replicated on 4 partition groups
       S[m,n] = sum_c k[c,m] q[c,n]  (row-tiled K=32 matmuls, 4-concurrent)
       E = exp(S - ln64) in fp8-e4m3  (max|S|=9.05 -> E'max 133 < 240;
           the 1/64 cancels between numerator and denominator)
       acc[c,n] = sum_m W[m,c] E[m,n]  with W = [v.T | ones] in fp8,
                  one DoubleRow matmul per m-tile PAIR (K=256 contraction)
                  -> rows 0..63 unnormalized out_s, row 64 = Z
  out = x + bn1 + (wbn2 @ acc[0:64]) * (1/Z)
        (per-n 1/Z broadcast to 64 partitions via a K=1 PE matmul)

v5 changes over v4 (which measured 221us):
  - 1/Z via reciprocal_approx_fast (custom DVE op, ~5x faster than the
    8-cycle iterative divide) + a tiny bf16 cast.  The old 3.0us serial
    reciprocal sat in the in-order DVE queue and head-of-line blocked the
    bc matmul -> PE -> next block's exps for a constant 3.45us at EVERY
    block boundary.
  - Preamble restructured: x DMA + bf16 cast in 4 column chunks across
    both HWDGE queues; only q/k chunks 0-1 are produced up front, so the
    first SAM exp fires at ~5us instead of 25us.  The remaining 12 q/k
    chunks are emitted as single-matmul fill-in groups spread over blocks
    0-2 (each chunk 2 pairs ahead of its consumption deadline).
  - Only the q01 PSUM evacuation runs on ScalarE; everything else
    evacuates on DVE so ACT time is exp + one copy.
  - Dense back-to-back preamble matmuls may flip the PE HAM clock gate
    to 2.4 GHz before the steady loop starts.
PSUM: spoolA(2) + spoolB(2) + vacc/EC(2) + ppool(2) = 8 banks.
"""

import sys
from contextlib import ExitStack

import numpy as np

if "/opt/trn_rl_repo" not in sys.path:
    sys.path.insert(0, "/opt/trn_rl_repo")

import concourse.bass as bass
import concourse.tile as tile
from concourse import bacc, mybir
from concourse.bass_utils import run_bass_kernel_spmd

F32 = mybir.dt.float32
BF16 = mybir.dt.bfloat16
FP8 = mybir.dt.float8e4

C = 64          # channels
HW = 4096       # 64*64 spatial
NB = 8          # number of 512-wide n blocks
BLK = 512
MT = 32         # m tiles of 128
NG = 16         # groups of 2 m-tiles per n-block
WP = 80         # wt8 per-m-tile stride (65 used; 80 for DoubleRow step%16==0)
NLOG64 = -4.1588830833596715

Exp = mybir.ActivationFunctionType.Exp
DR = mybir.MatmulPerfMode.DoubleRow


def _build_kernel(ctx: ExitStack, tc: tile.TileContext, io: dict):
    nc = tc.nc
    x_d = io["x"]
    out_d = io["out"]

    consts = ctx.enter_context(tc.tile_pool(name="consts", bufs=1))
    bigs = ctx.enter_context(tc.tile_pool(name="bigs", bufs=1))
    epool = ctx.enter_context(tc.tile_pool(name="epool", bufs=3))
    campool = ctx.enter_context(tc.tile_pool(name="campool", bufs=1))
    sampool = ctx.enter_context(tc.tile_pool(name="sampool", bufs=2))
    spoolA = ctx.enter_context(
        tc.tile_pool(name="spoolA", bufs=1, space=bass.MemorySpace.PSUM)
    )
    spoolB = ctx.enter_context(
        tc.tile_pool(name="spoolB", bufs=1, space=bass.MemorySpace.PSUM)
    )
    vpool = ctx.enter_context(
        tc.tile_pool(name="vpool", bufs=2, space=bass.MemorySpace.PSUM)
    )
    ppool = ctx.enter_context(
        tc.tile_pool(name="ppool", bufs=2, space=bass.MemorySpace.PSUM)
    )

    # ---- weight DMAs first, on the otherwise-idle GpSimd queue (tiny; if
    # they queued behind the 1MB x transfer the first matmul waits ~15us) --
    wq4T = consts.tile([C, 128], BF16)    # (w_q stacked 4x).T
    wk4T = consts.tile([C, 128], BF16)
    wvc = consts.tile([C, 128], BF16)     # [v.T | w_cam.T]
    wbn1T = consts.tile([C, C], F32)
    wbn2T = consts.tile([C, C], BF16)
    ones_r = consts.tile([128, C], BF16)  # row 0 holds ones[1, 64]
    zb = consts.tile([128, 1], F32)
    nlog64 = consts.tile([128, 1], F32)   # exp bias: E'=E/64 fits fp8e4 max 240
    dummy = consts.tile([128, 1], F32)
    warm_w = consts.tile([128, BLK], BF16)

    nc.gpsimd.dma_start(wk4T[:], io["wk4T"][:])
    nc.gpsimd.dma_start(wq4T[:], io["wq4T"][:])
    nc.gpsimd.dma_start(wvc[:], io["wvc"][:])
    nc.gpsimd.dma_start(wbn1T[:], io["wbn1T"][:])
    nc.gpsimd.dma_start(wbn2T[:], io["wbn2T"][:])
    nc.gpsimd.dma_start(ones_r[0:1, :], io["ones64"][:])

    # ---- x DMA: 8 column chunks round-robin over 4 HWDGE queues (each
    # queue sustains only ~100 GB/s; 4 queues cut the 1MB load to ~2.7us,
    # and the first 1024 columns -- all that k01/q01 need -- land first) --
    x_sb = bigs.tile([C, HW], F32)
    x_qs = [nc.sync, nc.scalar, nc.gpsimd]
    for xc_ in range(8):
        x_qs[xc_ % 3].dma_start(
            x_sb[:, xc_ * BLK : (xc_ + 1) * BLK],
            x_d[:, xc_ * BLK : (xc_ + 1) * BLK],
        )

    nc.vector.memset(zb[:], 0.0)
    # Trigger the exp ACT-table load right behind the x-DMA issue (overlaps
    # the transfer) instead of in front of the first real exp.
    nc.scalar.activation(dummy[:], zb[:], Exp, bias=zb[:])
    nc.vector.memset(nlog64[:], NLOG64)

    q4 = bigs.tile([128, HW], BF16)
    k4 = bigs.tile([128, HW], BF16)
    wt8 = bigs.tile([128, MT * WP], FP8)   # per m-tile [vT | ones | pad]
    xct = bigs.tile([128, MT * C], BF16)   # xcT, m-tile-major
    x_bf = bigs.tile([C, HW], BF16)

    # ---- PE warm-up: ~4.3us of dense back-to-back matmuls on junk data
    # while the x DMA is in flight.  The HAM clock gate needs ~3.4us of
    # sustained PE busy to lift the PE from 1.2 to 2.4 GHz; without this
    # the whole kernel runs matmuls at half clock. ----
    nc.vector.memset(warm_w[:], 1.0)
    warm_ps = ppool.tile([128, BLK], F32, tag="p", name="warm_ps")
    for _ in range(7):
        nc.tensor.matmul(
            warm_ps[:], warm_w[:, 0:128], warm_w[:], start=True, stop=True
        )

    # ones column of wt8 (wvc copies below only write cols 0..63)
    nc.vector.memset(
        wt8[:].rearrange("p (t c) -> p t c", c=WP)[:, :, 64:65], 1.0
    )

    # x in bf16 feeds the q4/k4/wvc/bn1 matmuls at full PE rate; 8 chunks so
    # the first q/k matmuls start as soon as the first x columns land.
    for xc_ in range(8):
        nc.vector.tensor_copy(
            x_bf[:, xc_ * BLK : (xc_ + 1) * BLK], x_sb[:, xc_ * BLK : (xc_ + 1) * BLK]
        )

    # ---- q4 / k4: replicated q,k via stacked-weight 1x1 convs ----
    # Only chunks 0-1 of k and q are computed up front (all that block 0
    # pair 0 needs); the rest are emitted as fill-in singles inside the
    # block loop, each 2+ pairs ahead of its consumption deadline.
    def qk_group(wT, dst, chunks, pool, on_scalar):
        ps = pool.tile([128, 2 * BLK], F32, tag="s", name="qkps")
        for i, cch in enumerate(chunks):
            nc.tensor.matmul(
                ps[:, i * BLK : (i + 1) * BLK],
                wT[:],
                x_bf[:, cch * BLK : (cch + 1) * BLK],
                start=True,
                stop=True,
            )
        w = len(chunks) * BLK
        lo = chunks[0] * BLK
        if on_scalar:
            nc.scalar.copy(dst[:, lo : lo + w], ps[:, :w])
        else:
            nc.vector.tensor_copy(dst[:, lo : lo + w], ps[:, :w])

    qk_group(wk4T, k4, [0, 1], spoolA, False)
    qk_group(wq4T, q4, [0, 1], spoolB, True)

    def qk_single(which, cch):
        """One q/k chunk: 1 matmul into a ppool bank + DVE evacuation."""
        wT, dst = (wk4T, k4) if which == "k" else (wq4T, q4)
        ps = ppool.tile([128, BLK], F32, tag="p", name="qks")
        nc.tensor.matmul(
            ps[:], wT[:], x_bf[:, cch * BLK : (cch + 1) * BLK],
            start=True, stop=True,
        )
        nc.vector.tensor_copy(dst[:, cch * BLK : (cch + 1) * BLK], ps[:])

    # (block, pair) -> (which, chunk); deadlines: k chunk c consumed at
    # block-0 pair c; q chunk c consumed at block c pair 0.
    qk_fill = {
        (0, 0): ("k", 2), (0, 1): ("k", 3), (0, 2): ("k", 4),
        (0, 3): ("k", 5), (0, 4): ("k", 6), (0, 5): ("k", 7),
        (1, 0): ("q", 2), (1, 2): ("q", 3), (1, 4): ("q", 4),
        (1, 6): ("q", 5), (2, 0): ("q", 6), (2, 2): ("q", 7),
    }

    state = {}  # EC tile, allocated at block 1 start (vpool slot timing)

    def wvc_group(base, size):
        """xcT and WT (=[vT|ones]) production for one m-tile group."""
        ps_w = ppool.tile([128, BLK], F32, tag="p", name="wvcps")
        for j in range(size):
            m = base + j
            nc.tensor.matmul(
                ps_w[:, j * 128 : (j + 1) * 128],
                x_bf[:, m * 128 : (m + 1) * 128],
                wvc[:],
                start=True,
                stop=True,
            )
        src = ps_w[:, : size * 128].rearrange("p (j c) -> p j c", c=128)
        wt_dst = wt8[:, base * WP : (base + size) * WP].rearrange(
            "p (j c) -> p j c", c=WP
        )
        with nc.allow_low_precision(reason="v in fp8 for DoubleRow acc"):
            nc.vector.tensor_copy(wt_dst[:, :, 0:C], src[:, :, 0:C])
        xct_dst = xct[:, base * C : (base + size) * C].rearrange(
            "p (j c) -> p j c", c=C
        )
        with nc.allow_low_precision(reason="xcT in bf16 for cheap ec matmuls"):
            nc.vector.tensor_copy(xct_dst, src[:, :, C : 2 * C])

    def ec_group(base, size):
        EC = state["EC"]
        for j in range(size):
            m = base + j
            nc.tensor.matmul(
                EC[0:C, 0:C],
                xct[:, m * C : (m + 1) * C],
                xct[:, m * C : (m + 1) * C],
                start=(m == 0),
                stop=(m == MT - 1),
            )

    # ---- per-block state for split epilogues ----
    vaccs = [None] * NB
    sam = [None] * NB   # sam65 [65, BLK] f32: rows 0..63 unnorm out_s, 64 = Z
    rzs = [None] * NB   # rz [65, BLK] bf16: row 64 = 1/Z
    M1T_sb = campool.tile([C, C], BF16)

    def epilogue_a(nb):
        """At block end: evacuate vacc (recip is emitted separately)."""
        aux = sampool.tile([C + 1, BLK], F32, tag="aux", name="aux")
        nc.vector.tensor_copy(aux[:], vaccs[nb][0 : C + 1, :])
        sam[nb] = aux

    def emit_recip(nb):
        """1/Z for block nb via the fast approx recip + bf16 cast.

        The custom DVE op only works at base partition 0 (and DVE lanes
        cannot move data across partitions), so the Z row is first moved
        from partition 64 to partition 0 by a tiny SBUF->SBUF DMA on the
        otherwise-idle sync queue.
        """
        z0 = sampool.tile([1, BLK], F32, tag="z0", name="z0")
        nc.sync.dma_start(z0[:], sam[nb][C : C + 1, :])
        rz32 = sampool.tile([1, BLK], F32, tag="rz32", name="rz32")
        nc.vector.reciprocal_approx_fast(rz32[:], z0[:])
        rzb = sampool.tile([1, BLK], BF16, tag="rz", name="rzb")
        with nc.allow_low_precision(reason="1/Z in bf16: 0.4% on the SAM term"):
            nc.vector.tensor_copy(rzb[:], rz32[:])
        rzs[nb] = rzb

    def epilogue_b1(nb):
        """Broadcast 1/Z to 64 partitions and scale the SAM accumulator."""
        bc = ppool.tile([128, BLK], F32, tag="p", name="bc")
        nc.tensor.matmul(
            bc[0:C, :],
            ones_r[0:1, 0:C],
            rzs[nb][:],
            start=True,
            stop=True,
            tile_position=(0, 0),
        )
        sam_sc = sampool.tile([C, BLK], BF16, tag="sc", name="sam_sc")
        nc.vector.tensor_mul(sam_sc[:], sam[nb][0:C, :], bc[0:C, :])
        return sam_sc

    def epilogue_b2(nb, sam_sc):
        """Bottleneck conv, residual add, DMA out."""
        ncol = slice(nb * BLK, (nb + 1) * BLK)
        bn = ppool.tile([128, BLK], F32, tag="p", name="bn")
        nc.tensor.matmul(
            bn[0:C, :], M1T_sb[:], x_bf[:, ncol], start=True, stop=False
        )
        nc.tensor.matmul(
            bn[0:C, :], wbn2T[:], sam_sc[:], start=False, stop=True
        )
        o_t = sampool.tile([C, BLK], F32, tag="ot", name="o_t")
        nc.vector.tensor_add(o_t[:], x_sb[:, ncol], bn[0:C, :])
        nc.sync.dma_start(out_d[:, ncol], o_t[:])

    def cam_chain():
        """CAM softmax -> attn_c -> M1T = (wbn1 @ attn_c).T"""
        EC = state["EC"]
        negmax = campool.tile([C, 1], F32)
        nc.vector.reduce_max(
            negmax[:], EC[0:C, 0:C], axis=mybir.AxisListType.X, negate=True
        )
        exp_c = campool.tile([C, C], F32)
        nc.scalar.activation(exp_c[:], EC[0:C, 0:C], Exp, bias=negmax[:])
        sum_c = campool.tile([C, 1], F32)
        nc.vector.reduce_sum(sum_c[:], exp_c[:], axis=mybir.AxisListType.X)
        rec_c = campool.tile([C, 1], F32)
        nc.vector.reciprocal(rec_c[:], sum_c[:])
        attn_c = campool.tile([C, C], F32)
        nc.vector.tensor_scalar_mul(attn_c[:], exp_c[:], rec_c[:])
        m1ps = ppool.tile([128, BLK], F32, tag="p", name="m1ps")
        nc.tensor.matmul(
            m1ps[0:C, 0:C], attn_c[:], wbn1T[:], start=True, stop=True
        )
        nc.vector.tensor_copy(M1T_sb[:], m1ps[0:C, 0:C])

    # ---- main SAM loop over 8 n-blocks, groups emitted in PAIRS ----
    sc_pend = {}  # nb -> sam_sc awaiting epilogue_b2
    for nb in range(NB):
        ncol = slice(nb * BLK, (nb + 1) * BLK)
        if nb == 1:
            # EC takes a vpool rotation slot; its last readers (CAM softmax,
            # start of block 2) finish before vacc(2) re-claims the slot.
            state["EC"] = vpool.tile([128, BLK], F32, tag="v", name="EC")
        if nb == 2:
            # CAM chain first so vacc(2), which aliases EC's bank, only
            # waits on the (fast) softmax reads of EC.
            cam_chain()
        vacc = vpool.tile([128, BLK], F32, tag="v", name="vacc")
        vaccs[nb] = vacc
        for p in range(NG // 2):
            g0, g1 = 2 * p, 2 * p + 1
            s_ts = []
            for g in (g0, g1):
                pool = spoolA if g % 2 == 0 else spoolB
                s_t = pool.tile([128, 2 * BLK], F32, tag="s", name="s_t")
                s_ts.append(s_t)
                for j in range(2):
                    m = 2 * g + j
                    r = 2 * (g % 2) + j  # row quadrants 0,1 / 2,3
                    nc.tensor.matmul(
                        s_t[:, j * BLK : (j + 1) * BLK],
                        k4[32 * r : 32 * r + 32, m * 128 : (m + 1) * 128],
                        q4[32 * r : 32 * r + 32, ncol],
                        start=True,
                        stop=True,
                        tile_position=(32 * r, 0),
                    )
            if nb == 0:
                wvc_group(2 * g0, 2)
                wvc_group(2 * g1, 2)
            if nb == 1:
                ec_group(2 * g0, 2)
                ec_group(2 * g1, 2)
            if (nb, p) in qk_fill:
                qk_single(*qk_fill[(nb, p)])
            e_ts = []
            for g, s_t in zip((g0, g1), s_ts):
                e_t = epool.tile([128, 2 * BLK], FP8, tag="e", name="e_t")
                e_ts.append(e_t)
                with nc.allow_low_precision(reason="E in fp8: ~1e-4 on out"):
                    nc.scalar.activation(e_t[:], s_t[:], Exp, bias=nlog64[:])
            for g, e_t in zip((g0, g1), e_ts):
                lhsT = wt8[:, 2 * g * WP : (2 * g + 2) * WP].rearrange(
                    "p (two f) -> p two f", two=2
                )[:, :, 0:65]
                rhs = e_t[:].rearrange("p (two f) -> p two f", two=2)
                nc.tensor.matmul(
                    vacc[0 : C + 1, :],
                    lhsT,
                    rhs,
                    start=(g == 0),
                    stop=(g == NG - 1),
                    perf_mode=DR,
                )
            # deferred work, slotted into quiet spots mid-block:
            if nb >= 1 and p == 0:
                # recip(nb-1) FIRST on this block's DVE FIFO; with the
                # approx-fast recip this is ~1us of DVE, not 3.
                emit_recip(nb - 1)
            if nb >= 2 and p == 1:
                sc_pend[nb - 2] = epilogue_b1(nb - 2)
            if nb >= 2 and p == 3:
                epilogue_b2(nb - 2, sc_pend.pop(nb - 2))

        epilogue_a(nb)
    # ---- tail: last two blocks' epilogues ----
    sc6 = epilogue_b1(NB - 2)
    epilogue_b2(NB - 2, sc6)
    emit_recip(NB - 1)
    sc7 = epilogue_b1(NB - 1)
    epilogue_b2(NB - 1, sc7)


def build_nc():
    nc = bacc.Bacc(
        "TRN2",
        target_bir_lowering=False,
        debug=False,
        enable_asserts=False,
        num_devices=8,
    )
    io = {}
    io["x"] = nc.dram_tensor("x", [C, HW], F32, kind="ExternalInput").ap()
    io["wq4T"] = nc.dram_tensor("wq4T", [C, 128], BF16, kind="ExternalInput").ap()
    io["wk4T"] = nc.dram_tensor("wk4T", [C, 128], BF16, kind="ExternalInput").ap()
    io["wvc"] = nc.dram_tensor("wvc", [C, 128], BF16, kind="ExternalInput").ap()
    io["wbn1T"] = nc.dram_tensor("wbn1T", [C, C], F32, kind="ExternalInput").ap()
    io["wbn2T"] = nc.dram_tensor("wbn2T", [C, C], BF16, kind="ExternalInput").ap()
    io["ones64"] = nc.dram_tensor("ones64", [1, C], BF16, kind="ExternalInput").ap()
    io["out"] = nc.dram_tensor("out", [C, HW], F32, kind="ExternalOutput").ap()

    with tile.TileContext(nc) as tc:
        with ExitStack() as ctx:
            _build_kernel(ctx, tc, io)
    nc.compile()
    return nc


def make_in_maps(x, w_cam, w_q, w_k, w_v, w_bn):
    import ml_dtypes

    f = lambda a: np.ascontiguousarray(np.asarray(a, dtype=np.float32))
    fb = lambda a: np.ascontiguousarray(
        np.asarray(a, dtype=np.float32).astype(ml_dtypes.bfloat16)
    )
    base = {
        "wq4T": fb(np.concatenate([np.asarray(w_q).T] * 4, axis=1)),
        "wk4T": fb(np.concatenate([np.asarray(w_k).T] * 4, axis=1)),
        "wvc": fb(np.concatenate([np.asarray(w_v).T, np.asarray(w_cam).T], axis=1)),
        "wbn1T": f(np.asarray(w_bn)[:, :C].T),
        "wbn2T": fb(np.asarray(w_bn)[:, C:].T),
        "ones64": fb(np.ones((1, C))),
    }
    x = np.asarray(x)
    return [dict(base, x=f(x[b].reshape(C, HW))) for b in range(8)]


_NC_CACHE = None


def kernel(x, w_cam, w_q, w_k, w_v, w_bn):
    global _NC_CACHE
    if _NC_CACHE is None:
        _NC_CACHE = build_nc()
    nc = _NC_CACHE
    in_maps = make_in_maps(x, w_cam, w_q, w_k, w_v, w_bn)
    res = run_bass_kernel_spmd(nc, in_maps, list(range(8)))
    out = np.stack([res.results[b]["out"].reshape(C, 64, 64) for b in range(8)])
    return out.astype(np.float32)


# revision 10
# speedup vs baseline: 1.0745x; 1.0022x over previous
"""Trainium2 Bass kernel for dual-attention block (CAM + SAM + bottleneck).

Contract: kernel(**inputs) takes FULL unsharded inputs
  x     [8, 64, 64, 64] f32
  w_cam [64, 64], w_q [32, 64], w_k [32, 64], w_v [64, 64], w_bn [64, 128]
and returns the full [8, 64, 64, 64] f32 output.

Sharding: data-parallel over batch across 8 NeuronCores (1 image each);
weights replicated. Per-core math (c=64 channels, n=m=4096 spatial):

  CAM: xcT = x.T @ w_cam.T ; Ec = xcT.T @ xcT;
       attn_c = softmax_rows(Ec); bn1 = (wbn1 @ attn_c) @ x   (folded M1)
  SAM: q4/k4 = (w stacked 4x) @ x  -> q,k replicated on 4 partition groups
       S[m,n] = sum_c k[c,m] q[c,n]  (row-tiled K=32 matmuls, 4-concurrent)
       E = exp(S - ln64) in fp8-e4m3  (max|S|=9.05 -> E'max 133 < 240;
           the 1/64 cancels between numerator and denominator)
       acc[c,n] = sum_m W[m,c] E[m,n]  with W = [v.T | ones] in fp8,
                  one DoubleRow matmul per m-tile PAIR (K=256 contraction)
                  -> rows 0..63 unnormalized out_s, row 64 = Z
  out = x + bn1 + (wbn2 @ acc[0:64]) * (1/Z)
        (per-n 1/Z broadcast to 64 partitions via a K=1 PE matmul)

v5 changes over v4 (which measured 221us):
  - 1/Z via reciprocal_approx_fast (custom DVE op, ~5x faster than the
    8-cycle iterative divide) + a tiny bf16 cast.  The old 3.0us serial
    reciprocal sat in the in-order DVE queue and head-of-line blocked the
    bc matmul -> PE -> next block's exps for a constant 3.45us at EVERY
    block boundary.
  - Preamble restructured: x DMA + bf16 cast in 4 column chunks across
    both HWDGE queues; only q/k chunks 0-1 are produced up front, so the
    first SAM exp fires at ~5us instead of 25us.  The remaining 12 q/k
    chunks are emitted as single-matmul fill-in groups spread over blocks
    0-2 (each chunk 2 pairs ahead of its consumption deadline).
  - Only the q01 PSUM evacuation runs on ScalarE; everything else
    evacuates on DVE so ACT time is exp + one copy.
  - Dense back-to-back preamble matmuls may flip the PE HAM clock gate
    to 2.4 GHz before the steady loop starts.
PSUM: spoolA(2) + spoolB(2) + vacc/EC(2) + ppool(2) = 8 banks.
"""

import sys
from contextlib import ExitStack

import numpy as np

if "/opt/trn_rl_repo" not in sys.path:
    sys.path.insert(0, "/opt/trn_rl_repo")

import concourse.bass as bass
import concourse.tile as tile
from concourse import bacc, mybir
from concourse.bass_utils import run_bass_kernel_spmd

F32 = mybir.dt.float32
BF16 = mybir.dt.bfloat16
FP8 = mybir.dt.float8e4

C = 64          # channels
HW = 4096       # 64*64 spatial
NB = 8          # number of 512-wide n blocks
BLK = 512
MT = 32         # m tiles of 128
NG = 16         # groups of 2 m-tiles per n-block
WP = 80         # wt8 per-m-tile stride (65 used; 80 for DoubleRow step%16==0)
NLOG64 = -4.1588830833596715

Exp = mybir.ActivationFunctionType.Exp
DR = mybir.MatmulPerfMode.DoubleRow


def _build_kernel(ctx: ExitStack, tc: tile.TileContext, io: dict):
    nc = tc.nc
    x_d = io["x"]
    out_d = io["out"]

    consts = ctx.enter_context(tc.tile_pool(name="consts", bufs=1))
    bigs = ctx.enter_context(tc.tile_pool(name="bigs", bufs=1))
    epool = ctx.enter_context(tc.tile_pool(name="epool", bufs=3))
    campool = ctx.enter_context(tc.tile_pool(name="campool", bufs=1))
    sampool = ctx.enter_context(tc.tile_pool(name="sampool", bufs=2))
    spoolA = ctx.enter_context(
        tc.tile_pool(name="spoolA", bufs=1, space=bass.MemorySpace.PSUM)
    )
    spoolB = ctx.enter_context(
        tc.tile_pool(name="spoolB", bufs=1, space=bass.MemorySpace.PSUM)
    )
    vpool = ctx.enter_context(
        tc.tile_pool(name="vpool", bufs=2, space=bass.MemorySpace.PSUM)
    )
    ppool = ctx.enter_context(
        tc.tile_pool(name="ppool", bufs=2, space=bass.MemorySpace.PSUM)
    )

    # ---- weight DMAs first, on the otherwise-idle GpSimd queue (tiny; if
    # they queued behind the 1MB x transfer the first matmul waits ~15us) --
    wq4T = consts.tile([C, 128], BF16)    # (w_q stacked 4x).T
    wk4T = consts.tile([C, 128], BF16)
    wvc = consts.tile([C, 128], BF16)     # [v.T | w_cam.T]
    wbn1T = consts.tile([C, C], F32)
    wbn2T = consts.tile([C, C], BF16)
    ones_r = consts.tile([128, C], BF16)  # row 0 holds ones[1, 64]
    zb = consts.tile([128, 1], F32)
    nlog64 = consts.tile([128, 1], F32)   # exp bias: E'=E/64 fits fp8e4 max 240
    dummy = consts.tile([128, 1], F32)
    warm_w = consts.tile([128, BLK], BF16)

    nc.gpsimd.dma_start(wk4T[:], io["wk4T"][:])
    nc.gpsimd.dma_start(wq4T[:], io["wq4T"][:])
    nc.gpsimd.dma_start(wvc[:], io["wvc"][:])
    nc.gpsimd.dma_start(wbn1T[:], io["wbn1T"][:])
    nc.gpsimd.dma_start(wbn2T[:], io["wbn2T"][:])
    nc.gpsimd.dma_start(ones_r[0:1, :], io["ones64"][:])

    # ---- x DMA: 8 column chunks round-robin over 4 HWDGE queues (each
    # queue sustains only ~100 GB/s; 4 queues cut the 1MB load to ~2.7us,
    # and the first 1024 columns -- all that k01/q01 need -- land first) --
    x_sb = bigs.tile([C, HW], F32)
    x_qs = [nc.sync, nc.scalar, nc.gpsimd]
    for xc_ in range(8):
        x_qs[xc_ % 3].dma_start(
            x_sb[:, xc_ * BLK : (xc_ + 1) * BLK],
            x_d[:, xc_ * BLK : (xc_ + 1) * BLK],
        )

    # warm_w memset is the FIRST DVE op so the PE warm-up burst below can
    # start as early as possible.
    nc.vector.memset(warm_w[:], 1.0)
    nc.vector.memset(zb[:], 0.0)
    # Trigger the exp ACT-table load right behind the x-DMA issue (overlaps
    # the transfer) instead of in front of the first real exp.
    nc.scalar.activation(dummy[:], zb[:], Exp, bias=zb[:])
    nc.vector.memset(nlog64[:], NLOG64)

    q4 = bigs.tile([128, HW], BF16)
    k4 = bigs.tile([128, HW], BF16)
    wt8 = bigs.tile([128, MT * WP], FP8)   # per m-tile [vT | ones | pad]
    xct = bigs.tile([128, MT * C], BF16)   # xcT, m-tile-major
    x_bf = bigs.tile([C, HW], BF16)

    # ---- PE warm-up: ~6.8us of dense back-to-back matmuls on junk data
    # while the x DMA is in flight.  The HAM clock gate needs a full
    # free-running ~3.4us window of sustained PE busy to lift the PE from
    # 1.2 to 2.4 GHz; without this the whole kernel runs matmuls at half
    # clock.  16 x FD=512 at ~427ns cold spacing guarantees one full
    # window regardless of phase. ----
    warm_ps = ppool.tile([128, BLK], F32, tag="p", name="warm_ps")
    for _ in range(16):
        nc.tensor.matmul(
            warm_ps[:], warm_w[:, 0:128], warm_w[:], start=True, stop=True
        )

    # ones column of wt8 (wvc copies below only write cols 0..63)
    nc.vector.memset(
        wt8[:].rearrange("p (t c) -> p t c", c=WP)[:, :, 64:65], 1.0
    )

    # x in bf16 feeds the q4/k4/wvc/bn1 matmuls at full PE rate; 8 chunks so
    # the first q/k matmuls start as soon as the first x columns land.
    for xc_ in range(8):
        nc.vector.tensor_copy(
            x_bf[:, xc_ * BLK : (xc_ + 1) * BLK], x_sb[:, xc_ * BLK : (xc_ + 1) * BLK]
        )

    # ---- q4 / k4: replicated q,k via stacked-weight 1x1 convs ----
    # Only chunks 0-1 of k and q are computed up front (all that block 0
    # pair 0 needs); the rest are emitted as fill-in singles inside the
    # block loop, each 2+ pairs ahead of its consumption deadline.
    def qk_group(wT, dst, chunks, pool, on_scalar):
        ps = pool.tile([128, 2 * BLK], F32, tag="s", name="qkps")
        for i, cch in enumerate(chunks):
            nc.tensor.matmul(
                ps[:, i * BLK : (i + 1) * BLK],
                wT[:],
                x_bf[:, cch * BLK : (cch + 1) * BLK],
                start=True,
                stop=True,
            )
        w = len(chunks) * BLK
        lo = chunks[0] * BLK
        if on_scalar:
            nc.scalar.copy(dst[:, lo : lo + w], ps[:, :w])
        else:
            nc.vector.tensor_copy(dst[:, lo : lo + w], ps[:, :w])

    qk_group(wk4T, k4, [0, 1], spoolA, False)
    qk_group(wq4T, q4, [0, 1], spoolB, True)

    def qk_single(which, cch):
        """One q/k chunk: 1 matmul into a ppool bank + DVE evacuation."""
        wT, dst = (wk4T, k4) if which == "k" else (wq4T, q4)
        ps = ppool.tile([128, BLK], F32, tag="p", name="qks")
        nc.tensor.matmul(
            ps[:], wT[:], x_bf[:, cch * BLK : (cch + 1) * BLK],
            start=True, stop=True,
        )
        nc.vector.tensor_copy(dst[:, cch * BLK : (cch + 1) * BLK], ps[:])

    # (block, pair) -> (which, chunk); deadlines: k chunk c consumed at
    # block-0 pair c; q chunk c consumed at block c pair 0.
    qk_fill = {
        (0, 0): ("k", 2), (0, 1): ("k", 3), (0, 2): ("k", 4),
        (0, 3): ("k", 5), (0, 4): ("k", 6), (0, 5): ("k", 7),
        (1, 0): ("q", 2), (1, 2): ("q", 3), (1, 4): ("q", 4),
        (1, 6): ("q", 5), (2, 0): ("q", 6), (2, 2): ("q", 7),
    }

    state = {}  # EC tile, allocated at block 1 start (vpool slot timing)

    def wvc_group(base, size):
        """xcT and WT (=[vT|ones]) production for one m-tile group."""
        ps_w = ppool.tile([128, BLK], F32, tag="p", name="wvcps")
        for j in range(size):
            m = base + j
            nc.tensor.matmul(
                ps_w[:, j * 128 : (j + 1) * 128],
                x_bf[:, m * 128 : (m + 1) * 128],
                wvc[:],
                start=True,
                stop=True,
            )
        src = ps_w[:, : size * 128].rearrange("p (j c) -> p j c", c=128)
        wt_dst = wt8[:, base * WP : (base + size) * WP].rearrange(
            "p (j c) -> p j c", c=WP
        )
        with nc.allow_low_precision(reason="v in fp8 for DoubleRow acc"):
            nc.vector.tensor_copy(wt_dst[:, :, 0:C], src[:, :, 0:C])
        xct_dst = xct[:, base * C : (base + size) * C].rearrange(
            "p (j c) -> p j c", c=C
        )
        with nc.allow_low_precision(reason="xcT in bf16 for cheap ec matmuls"):
            nc.vector.tensor_copy(xct_dst, src[:, :, C : 2 * C])

    def ec_group(base, size):
        EC = state["EC"]
        for j in range(size):
            m = base + j
            nc.tensor.matmul(
                EC[0:C, 0:C],
                xct[:, m * C : (m + 1) * C],
                xct[:, m * C : (m + 1) * C],
                start=(m == 0),
                stop=(m == MT - 1),
            )

    # ---- per-block state for split epilogues ----
    vaccs = [None] * NB
    sam = [None] * NB   # sam65 [65, BLK] f32: rows 0..63 unnorm out_s, 64 = Z
    rzs = [None] * NB   # rz [65, BLK] bf16: row 64 = 1/Z
    M1T_sb = campool.tile([C, C], BF16)

    def epilogue_a(nb):
        """At block end: evacuate vacc (recip is emitted separately)."""
        aux = sampool.tile([C + 1, BLK], F32, tag="aux", name="aux")
        nc.vector.tensor_copy(aux[:], vaccs[nb][0 : C + 1, :])
        sam[nb] = aux

    def emit_recip(nb):
        """1/Z for block nb via the fast approx recip + bf16 cast.

        The custom DVE op only works at base partition 0 (and DVE lanes
        cannot move data across partitions), so the Z row is first moved
        from partition 64 to partition 0 by a tiny SBUF->SBUF DMA on the
        otherwise-idle sync queue.
        """
        z0 = sampool.tile([1, BLK], F32, tag="z0", name="z0")
        nc.sync.dma_start(z0[:], sam[nb][C : C + 1, :])
        rz32 = sampool.tile([1, BLK], F32, tag="rz32", name="rz32")
        nc.vector.reciprocal_approx_fast(rz32[:], z0[:])
        rzb = sampool.tile([1, BLK], BF16, tag="rz", name="rzb")
        with nc.allow_low_precision(reason="1/Z in bf16: 0.4% on the SAM term"):
            nc.vector.tensor_copy(rzb[:], rz32[:])
        rzs[nb] = rzb

    def epilogue_b1(nb):
        """Broadcast 1/Z to 64 partitions and scale the SAM accumulator."""
        bc = ppool.tile([128, BLK], F32, tag="p", name="bc")
        nc.tensor.matmul(
            bc[0:C, :],
            ones_r[0:1, 0:C],
            rzs[nb][:],
            start=True,
            stop=True,
            tile_position=(0, 0),
        )
        sam_sc = sampool.tile([C, BLK], BF16, tag="sc", name="sam_sc")
        nc.vector.tensor_mul(sam_sc[:], sam[nb][0:C, :], bc[0:C, :])
        return sam_sc

    def epilogue_b2(nb, sam_sc):
        """Bottleneck conv, residual add, DMA out."""
        ncol = slice(nb * BLK, (nb + 1) * BLK)
        bn = ppool.tile([128, BLK], F32, tag="p", name="bn")
        nc.tensor.matmul(
            bn[0:C, :], M1T_sb[:], x_bf[:, ncol], start=True, stop=False
        )
        nc.tensor.matmul(
            bn[0:C, :], wbn2T[:], sam_sc[:], start=False, stop=True
        )
        o_t = sampool.tile([C, BLK], F32, tag="ot", name="o_t")
        nc.vector.tensor_add(o_t[:], x_sb[:, ncol], bn[0:C, :])
        nc.sync.dma_start(out_d[:, ncol], o_t[:])

    def cam_chain():
        """CAM softmax -> attn_c -> M1T = (wbn1 @ attn_c).T"""
        EC = state["EC"]
        negmax = campool.tile([C, 1], F32)
        nc.vector.reduce_max(
            negmax[:], EC[0:C, 0:C], axis=mybir.AxisListType.X, negate=True
        )
        exp_c = campool.tile([C, C], F32)
        nc.scalar.activation(exp_c[:], EC[0:C, 0:C], Exp, bias=negmax[:])
        sum_c = campool.tile([C, 1], F32)
        nc.vector.reduce_sum(sum_c[:], exp_c[:], axis=mybir.AxisListType.X)
        rec_c = campool.tile([C, 1], F32)
        nc.vector.reciprocal(rec_c[:], sum_c[:])
        attn_c = campool.tile([C, C], F32)
        nc.vector.tensor_scalar_mul(attn_c[:], exp_c[:], rec_c[:])
        m1ps = ppool.tile([128, BLK], F32, tag="p", name="m1ps")
        nc.tensor.matmul(
            m1ps[0:C, 0:C], attn_c[:], wbn1T[:], start=True, stop=True
        )
        nc.vector.tensor_copy(M1T_sb[:], m1ps[0:C, 0:C])

    # ---- main SAM loop over 8 n-blocks, groups emitted in PAIRS ----
    sc_pend = {}  # nb -> sam_sc awaiting epilogue_b2
    for nb in range(NB):
        ncol = slice(nb * BLK, (nb + 1) * BLK)
        if nb == 1:
            # EC takes a vpool rotation slot; its last readers (CAM softmax,
            # start of block 2) finish before vacc(2) re-claims the slot.
            state["EC"] = vpool.tile([128, BLK], F32, tag="v", name="EC")
        if nb == 2:
            # CAM chain first so vacc(2), which aliases EC's bank, only
            # waits on the (fast) softmax reads of EC.
            cam_chain()
        vacc = vpool.tile([128, BLK], F32, tag="v", name="vacc")
        vaccs[nb] = vacc
        for p in range(NG // 2):
            g0, g1 = 2 * p, 2 * p + 1
            s_ts = []
            for g in (g0, g1):
                pool = spoolA if g % 2 == 0 else spoolB
                s_t = pool.tile([128, 2 * BLK], F32, tag="s", name="s_t")
                s_ts.append(s_t)
                for j in range(2):
                    m = 2 * g + j
                    r = 2 * (g % 2) + j  # row quadrants 0,1 / 2,3
                    nc.tensor.matmul(
                        s_t[:, j * BLK : (j + 1) * BLK],
                        k4[32 * r : 32 * r + 32, m * 128 : (m + 1) * 128],
                        q4[32 * r : 32 * r + 32, ncol],
                        start=True,
                        stop=True,
                        tile_position=(32 * r, 0),
                    )
            if nb == 0:
                wvc_group(2 * g0, 2)
                wvc_group(2 * g1, 2)
            if nb == 1:
                ec_group(2 * g0, 2)
                ec_group(2 * g1, 2)
            if (nb, p) in qk_fill:
                qk_single(*qk_fill[(nb, p)])
            e_ts = []
            for g, s_t in zip((g0, g1), s_ts):
                e_t = epool.tile([128, 2 * BLK], FP8, tag="e", name="e_t")
                e_ts.append(e_t)
                with nc.allow_low_precision(reason="E in fp8: ~1e-4 on out"):
                    nc.scalar.activation(e_t[:], s_t[:], Exp, bias=nlog64[:])
            for g, e_t in zip((g0, g1), e_ts):
                lhsT = wt8[:, 2 * g * WP : (2 * g + 2) * WP].rearrange(
                    "p (two f) -> p two f", two=2
                )[:, :, 0:65]
                rhs = e_t[:].rearrange("p (two f) -> p two f", two=2)
                nc.tensor.matmul(
                    vacc[0 : C + 1, :],
                    lhsT,
                    rhs,
                    start=(g == 0),
                    stop=(g == NG - 1),
                    perf_mode=DR,
                )
            # deferred work, slotted into quiet spots mid-block:
            if nb >= 1 and p == 0:
                # recip(nb-1) FIRST on this block's DVE FIFO; with the
                # approx-fast recip this is ~1us of DVE, not 3.
                emit_recip(nb - 1)
            if nb >= 2 and p == 1:
                sc_pend[nb - 2] = epilogue_b1(nb - 2)
            if nb >= 2 and p == 3:
                epilogue_b2(nb - 2, sc_pend.pop(nb - 2))

        epilogue_a(nb)
    # ---- tail: last two blocks' epilogues ----
    sc6 = epilogue_b1(NB - 2)
    epilogue_b2(NB - 2, sc6)
    emit_recip(NB - 1)
    sc7 = epilogue_b1(NB - 1)
    epilogue_b2(NB - 1, sc7)


def build_nc():
    nc = bacc.Bacc(
        "TRN2",
        target_bir_lowering=False,
        debug=False,
        enable_asserts=False,
        num_devices=8,
    )
    io = {}
    io["x"] = nc.dram_tensor("x", [C, HW], F32, kind="ExternalInput").ap()
    io["wq4T"] = nc.dram_tensor("wq4T", [C, 128], BF16, kind="ExternalInput").ap()
    io["wk4T"] = nc.dram_tensor("wk4T", [C, 128], BF16, kind="ExternalInput").ap()
    io["wvc"] = nc.dram_tensor("wvc", [C, 128], BF16, kind="ExternalInput").ap()
    io["wbn1T"] = nc.dram_tensor("wbn1T", [C, C], F32, kind="ExternalInput").ap()
    io["wbn2T"] = nc.dram_tensor("wbn2T", [C, C], BF16, kind="ExternalInput").ap()
    io["ones64"] = nc.dram_tensor("ones64", [1, C], BF16, kind="ExternalInput").ap()
    io["out"] = nc.dram_tensor("out", [C, HW], F32, kind="ExternalOutput").ap()

    with tile.TileContext(nc) as tc:
        with ExitStack() as ctx:
            _build_kernel(ctx, tc, io)
    nc.compile()
    return nc


def make_in_maps(x, w_cam, w_q, w_k, w_v, w_bn):
    import ml_dtypes

    f = lambda a: np.ascontiguousarray(np.asarray(a, dtype=np.float32))
    fb = lambda a: np.ascontiguousarray(
        np.asarray(a, dtype=np.float32).astype(ml_dtypes.bfloat16)
    )
    base = {
        "wq4T": fb(np.concatenate([np.asarray(w_q).T] * 4, axis=1)),
        "wk4T": fb(np.concatenate([np.asarray(w_k).T] * 4, axis=1)),
        "wvc": fb(np.concatenate([np.asarray(w_v).T, np.asarray(w_cam).T], axis=1)),
        "wbn1T": f(np.asarray(w_bn)[:, :C].T),
        "wbn2T": fb(np.asarray(w_bn)[:, C:].T),
        "ones64": fb(np.ones((1, C))),
    }
    x = np.asarray(x)
    return [dict(base, x=f(x[b].reshape(C, HW))) for b in range(8)]


_NC_CACHE = None


def kernel(x, w_cam, w_q, w_k, w_v, w_bn):
    global _NC_CACHE
    if _NC_CACHE is None:
        _NC_CACHE = build_nc()
    nc = _NC_CACHE
    in_maps = make_in_maps(x, w_cam, w_q, w_k, w_v, w_bn)
    res = run_bass_kernel_spmd(nc, in_maps, list(range(8)))
    out = np.stack([res.results[b]["out"].reshape(C, 64, 64) for b in range(8)])
    return out.astype(np.float32)


# revision 12
# speedup vs baseline: 1.1314x; 1.0529x over previous
"""Trainium2 Bass kernel for dual-attention block (CAM + SAM + bottleneck).

Contract: kernel(**inputs) takes FULL unsharded inputs
  x     [8, 64, 64, 64] f32
  w_cam [64, 64], w_q [32, 64], w_k [32, 64], w_v [64, 64], w_bn [64, 128]
and returns the full [8, 64, 64, 64] f32 output.

Sharding: data-parallel over batch across 8 NeuronCores (1 image each);
weights replicated. Per-core math (c=64 channels, n=m=4096 spatial):

  CAM: xcT = x.T @ w_cam.T ; Ec = xcT.T @ xcT;
       attn_c = softmax_rows(Ec); bn = ((wbn1 @ attn_c) + I) @ x
       (the +I folds the residual x into the CAM bottleneck matmul)
  SAM: q4/k4 = (w stacked 4x) @ x  -> q,k replicated on 4 partition groups
       S[m,n] = sum_c k[c,m] q[c,n]  (row-tiled K=32 matmuls, 4-concurrent)
       E = exp(S - ln64) in fp8-e4m3  (max|S|=9.05 -> E'max 133 < 240;
           the 1/64 cancels between numerator and denominator)
       acc[c,n] = sum_m W[m,c] E[m,n]  with W = [v'.T | ones] in fp8 and
                  v' = (wbn2 @ w_v) x  (bottleneck conv folded into the
                  value weights on the host), one DoubleRow matmul per
                  m-tile PAIR (K=256) -> rows 0..63 = wbn2-projected SAM
                  contribution (unnormalized), row 64 = Z
  out = bn + acc[0:64] * (1/Z)
        (1/Z via custom-DVE fast reciprocal at partition 0, broadcast to
        64 partitions by GpSimd partition_broadcast)

v8 structure (v4 measured 221us, v7 206us):
  - spool: ONE 3-slot rotation (3 x 2 PSUM banks) for the S tiles.  With
    A/B ping-pong a pair's second S group had to wait for the previous
    exp, serializing the quadrant matmuls 2+2; with 3 slots a group's
    bank is free 3 exp-periods ahead, so all 4 K=32 quadrant matmuls of
    a pair issue back-to-back and run concurrently on disjoint row
    quadrants.  All other PSUM scratch (warm-up, q/k chunk production,
    wvc, bn, m1ps) rides the same rotation; vacc/EC keep 2 banks.
  - wbn2 folded into the DR weights host-side; the per-block bottleneck
    matmul on the SAM path is gone.  The residual +x is folded into the
    CAM bottleneck matmul as (M1+I) via a device identity add.
  - 1/Z: fast approx reciprocal (partition 0, via a 2KB DMA hop) and
    GpSimd partition_broadcast instead of a K=1 PE matmul.
  - Preamble: weight DMAs first on the GpSimd queue; x in 8 chunks over
    3 DMA queues; ~6us dense PE warm-up (HAM -> 2.4GHz) overlapping the
    x DMA; q/k chunks 0-1 as single FD=1024 matmuls right behind it.
PSUM: spool 3x2 + vacc/EC 2 = 8 banks.
"""

import sys
from contextlib import ExitStack

import numpy as np

if "/opt/trn_rl_repo" not in sys.path:
    sys.path.insert(0, "/opt/trn_rl_repo")

import concourse.bass as bass
import concourse.tile as tile
from concourse import bacc, mybir
from concourse.bass_utils import run_bass_kernel_spmd

F32 = mybir.dt.float32
BF16 = mybir.dt.bfloat16
FP8 = mybir.dt.float8e4

C = 64          # channels
HW = 4096       # 64*64 spatial
NB = 8          # number of 512-wide n blocks
BLK = 512
MT = 32         # m tiles of 128
NG = 16         # groups of 2 m-tiles per n-block
WP = 80         # wt8 per-m-tile stride (65 used; 80 for DoubleRow step%16==0)
NLOG64 = -4.1588830833596715

Exp = mybir.ActivationFunctionType.Exp
DR = mybir.MatmulPerfMode.DoubleRow


def _build_kernel(ctx: ExitStack, tc: tile.TileContext, io: dict):
    nc = tc.nc
    x_d = io["x"]
    out_d = io["out"]

    consts = ctx.enter_context(tc.tile_pool(name="consts", bufs=1))
    bigs = ctx.enter_context(tc.tile_pool(name="bigs", bufs=1))
    epool = ctx.enter_context(tc.tile_pool(name="epool", bufs=3))
    campool = ctx.enter_context(tc.tile_pool(name="campool", bufs=1))
    sampool = ctx.enter_context(tc.tile_pool(name="sampool", bufs=2))
    spool = ctx.enter_context(
        tc.tile_pool(name="spool", bufs=3, space=bass.MemorySpace.PSUM)
    )
    vpool = ctx.enter_context(
        tc.tile_pool(name="vpool", bufs=2, space=bass.MemorySpace.PSUM)
    )

    # ---- weight DMAs first, on the otherwise-idle GpSimd queue (tiny; if
    # they queued behind the 1MB x transfer the first matmul waits ~15us) --
    wq4T = consts.tile([C, 128], BF16)    # (w_q stacked 4x).T
    wk4T = consts.tile([C, 128], BF16)
    wvc = consts.tile([C, 128], BF16)     # [(wbn2 w_v).T | w_cam.T]
    wbn1T = consts.tile([C, C], F32)
    id64 = consts.tile([C, C], BF16)
    zb = consts.tile([128, 1], F32)
    nlog64 = consts.tile([128, 1], F32)   # exp bias: E'=E/64 fits fp8e4 max 240
    dummy = consts.tile([128, 1], F32)
    warm_w = consts.tile([128, BLK], BF16)

    nc.gpsimd.dma_start(wk4T[:], io["wk4T"][:])
    nc.gpsimd.dma_start(wq4T[:], io["wq4T"][:])
    nc.gpsimd.dma_start(wvc[:], io["wvc"][:])
    nc.gpsimd.dma_start(wbn1T[:], io["wbn1T"][:])
    nc.gpsimd.dma_start(id64[:], io["id64"][:])

    # ---- x DMA: 8 column chunks round-robin over 3 HWDGE queues (each
    # queue sustains only ~100 GB/s; the first 1024 columns -- all that
    # k01/q01 need -- land first) ----
    x_sb = bigs.tile([C, HW], F32)
    x_qs = [nc.sync, nc.scalar, nc.gpsimd]
    for xc_ in range(8):
        x_qs[xc_ % 3].dma_start(
            x_sb[:, xc_ * BLK : (xc_ + 1) * BLK],
            x_d[:, xc_ * BLK : (xc_ + 1) * BLK],
        )

    # warm_w memset is the FIRST DVE op so the PE warm-up burst below can
    # start as early as possible.
    nc.vector.memset(warm_w[:], 1.0)
    nc.vector.memset(zb[:], 0.0)
    # Trigger the exp ACT-table load right behind the x-DMA issue (overlaps
    # the transfer) instead of in front of the first real exp.
    nc.scalar.activation(dummy[:], zb[:], Exp, bias=zb[:])
    nc.vector.memset(nlog64[:], NLOG64)

    q4 = bigs.tile([128, HW], BF16)
    k4 = bigs.tile([128, HW], BF16)
    wt8 = bigs.tile([128, MT * WP], FP8)   # per m-tile [v'T | ones | pad]
    xct = bigs.tile([128, MT * C], BF16)   # xcT, m-tile-major
    x_bf = bigs.tile([C, HW], BF16)

    # ---- PE warm-up: ~6us of dense back-to-back matmuls on junk data
    # while the x DMA is in flight.  The HAM clock gate needs a full
    # free-running ~3.4us window of sustained PE busy to lift the PE from
    # 1.2 to 2.4 GHz; without this every preamble matmul runs at half
    # clock. ----
    for wp_ in range(7):
        warm_ps = spool.tile([128, BLK], F32, tag="s", name="warm_ps")
        nc.tensor.matmul(
            warm_ps[:], warm_w[:, 0:128], warm_w[:], start=True, stop=True
        )

    # ones column of wt8 (wvc copies below only write cols 0..63)
    nc.vector.memset(
        wt8[:].rearrange("p (t c) -> p t c", c=WP)[:, :, 64:65], 1.0
    )

    # x in bf16 feeds the q4/k4/wvc/bn matmuls at full PE rate; 8 chunks so
    # the first q/k matmuls start as soon as the first x columns land.
    for xc_ in range(8):
        nc.vector.tensor_copy(
            x_bf[:, xc_ * BLK : (xc_ + 1) * BLK], x_sb[:, xc_ * BLK : (xc_ + 1) * BLK]
        )

    # ---- q4 / k4: replicated q,k via stacked-weight 1x1 convs.  Each
    # 2-chunk group is ONE FD=1024 matmul.  Chunks 0-1 of k and q are
    # produced up front; the rest are fill-in groups inside the block
    # loop, each 2+ pairs ahead of its consumption deadline. ----
    def qk_group(which, cch, on_scalar=False):
        wT, dst = (wk4T, k4) if which == "k" else (wq4T, q4)
        ps = spool.tile([128, 2 * BLK], F32, tag="s", name="qkps")
        for i in range(2):
            nc.tensor.matmul(
                ps[:, i * BLK : (i + 1) * BLK],
                wT[:],
                x_bf[:, (cch + i) * BLK : (cch + i + 1) * BLK],
                start=True,
                stop=True,
            )
        lo = cch * BLK
        if on_scalar:
            nc.scalar.copy(dst[:, lo : lo + 2 * BLK], ps[:])
        else:
            nc.vector.tensor_copy(dst[:, lo : lo + 2 * BLK], ps[:])

    qk_group("k", 0)
    qk_group("q", 0, on_scalar=True)

    # (block, pair) -> (which, first-chunk); deadlines: k chunks (c,c+1)
    # consumed at block-0 pairs c..c+1; q chunks (c,c+1) at blocks c..c+1.
    qk_fill = {
        (0, 0): ("k", 2), (0, 2): ("k", 4), (0, 4): ("k", 6),
        (0, 6): ("q", 2), (1, 1): ("q", 4), (1, 3): ("q", 6),
    }

    state = {}  # EC tile, allocated at block 1 start (vpool slot timing)

    def wvc_group(base, size):
        """xcT and WT (=[v'T|ones]) production for one m-tile group."""
        ps_w = spool.tile([128, BLK], F32, tag="s", name="wvcps")
        for j in range(size):
            m = base + j
            nc.tensor.matmul(
                ps_w[:, j * 128 : (j + 1) * 128],
                x_bf[:, m * 128 : (m + 1) * 128],
                wvc[:],
                start=True,
                stop=True,
            )
        src = ps_w[:, : size * 128].rearrange("p (j c) -> p j c", c=128)
        wt_dst = wt8[:, base * WP : (base + size) * WP].rearrange(
            "p (j c) -> p j c", c=WP
        )
        with nc.allow_low_precision(reason="v' in fp8 for DoubleRow acc"):
            nc.vector.tensor_copy(wt_dst[:, :, 0:C], src[:, :, 0:C])
        xct_dst = xct[:, base * C : (base + size) * C].rearrange(
            "p (j c) -> p j c", c=C
        )
        with nc.allow_low_precision(reason="xcT in bf16 for cheap ec matmuls"):
            nc.vector.tensor_copy(xct_dst, src[:, :, C : 2 * C])

    def ec_group(base, size):
        EC = state["EC"]
        for j in range(size):
            m = base + j
            nc.tensor.matmul(
                EC[0:C, 0:C],
                xct[:, m * C : (m + 1) * C],
                xct[:, m * C : (m + 1) * C],
                start=(m == 0),
                stop=(m == MT - 1),
            )

    # ---- per-block state for split epilogues ----
    vaccs = [None] * NB
    sam = [None] * NB   # sam65 [65, BLK] f32: rows 0..63 unnorm SAM out, 64 = Z
    rzs = [None] * NB   # rz [1, BLK] bf16 at partition 0
    M1T_sb = campool.tile([C, C], BF16)

    def epilogue_a(nb):
        """At block end: evacuate vacc (recip is emitted separately)."""
        aux = sampool.tile([C + 1, BLK], F32, tag="aux", name="aux")
        nc.vector.tensor_copy(aux[:], vaccs[nb][0 : C + 1, :])
        sam[nb] = aux

    def emit_recip(nb):
        """1/Z for block nb via the fast approx recip + bf16 cast.

        The custom DVE op only works at base partition 0 (and DVE lanes
        cannot move data across partitions), so the Z row is first moved
        from partition 64 to partition 0 by a tiny SBUF->SBUF DMA on the
        otherwise-idle sync queue.
        """
        z0 = sampool.tile([1, BLK], F32, tag="z0", name="z0")
        nc.sync.dma_start(z0[:], sam[nb][C : C + 1, :])
        rz32 = sampool.tile([1, BLK], F32, tag="rz32", name="rz32")
        nc.vector.reciprocal_approx_fast(rz32[:], z0[:])
        rzb = sampool.tile([1, BLK], BF16, tag="rz", name="rzb")
        with nc.allow_low_precision(reason="1/Z in bf16: 0.4% on the SAM term"):
            nc.vector.tensor_copy(rzb[:], rz32[:])
        rzs[nb] = rzb

    def epilogue_b1(nb):
        """Broadcast 1/Z to 64 partitions (GpSimd) and scale the SAM rows."""
        bcast = sampool.tile([C, BLK], BF16, tag="bc", name="bcast")
        nc.gpsimd.partition_broadcast(bcast[:], rzs[nb][:])
        sam_sc = sampool.tile([C, BLK], F32, tag="sc", name="sam_sc")
        nc.vector.tensor_mul(sam_sc[:], sam[nb][0:C, :], bcast[:])
        return sam_sc

    def epilogue_b2(nb, sam_sc):
        """CAM bottleneck (+residual via I) matmul, add SAM term, DMA out."""
        ncol = slice(nb * BLK, (nb + 1) * BLK)
        bn = spool.tile([128, BLK], F32, tag="s", name="bn")
        nc.tensor.matmul(
            bn[0:C, :], M1T_sb[:], x_bf[:, ncol], start=True, stop=True
        )
        o_t = sampool.tile([C, BLK], F32, tag="ot", name="o_t")
        nc.vector.tensor_add(o_t[:], bn[0:C, :], sam_sc[:])
        nc.sync.dma_start(out_d[:, ncol], o_t[:])

    def cam_chain():
        """CAM softmax -> attn_c -> M1T = (wbn1 @ attn_c).T + I"""
        EC = state["EC"]
        negmax = campool.tile([C, 1], F32)
        nc.vector.reduce_max(
            negmax[:], EC[0:C, 0:C], axis=mybir.AxisListType.X, negate=True
        )
        exp_c = campool.tile([C, C], F32)
        nc.scalar.activation(exp_c[:], EC[0:C, 0:C], Exp, bias=negmax[:])
        sum_c = campool.tile([C, 1], F32)
        nc.vector.reduce_sum(sum_c[:], exp_c[:], axis=mybir.AxisListType.X)
        rec_c = campool.tile([C, 1], F32)
        nc.vector.reciprocal(rec_c[:], sum_c[:])
        attn_c = campool.tile([C, C], F32)
        nc.vector.tensor_scalar_mul(attn_c[:], exp_c[:], rec_c[:])
        m1ps = spool.tile([128, BLK], F32, tag="s", name="m1ps")
        nc.tensor.matmul(
            m1ps[0:C, 0:C], attn_c[:], wbn1T[:], start=True, stop=True
        )
        with nc.allow_low_precision(reason="M1T in bf16 feeds a bf16 matmul"):
            nc.vector.tensor_add(M1T_sb[:], m1ps[0:C, 0:C], id64[:])

    # ---- main SAM loop over 8 n-blocks, groups emitted in PAIRS ----
    sc_pend = {}  # nb -> sam_sc awaiting epilogue_b2
    for nb in range(NB):
        if nb == 1:
            # EC takes a vpool rotation slot; its last readers (CAM softmax,
            # start of block 2) finish before vacc(2) re-claims the slot.
            state["EC"] = vpool.tile([128, BLK], F32, tag="v", name="EC")
        if nb == 2:
            # CAM chain first so vacc(2), which aliases EC's bank, only
            # waits on the (fast) softmax reads of EC.
            cam_chain()
        vacc = vpool.tile([128, BLK], F32, tag="v", name="vacc")
        vaccs[nb] = vacc
        ncol = slice(nb * BLK, (nb + 1) * BLK)
        for p in range(NG // 2):
            g0, g1 = 2 * p, 2 * p + 1
            s_ts = []
            for g in (g0, g1):
                s_t = spool.tile([128, 2 * BLK], F32, tag="s", name="s_t")
                s_ts.append(s_t)
                for j in range(2):
                    m = 2 * g + j
                    r = 2 * (g % 2) + j  # row quadrants 0,1 / 2,3
                    nc.tensor.matmul(
                        s_t[:, j * BLK : (j + 1) * BLK],
                        k4[32 * r : 32 * r + 32, m * 128 : (m + 1) * 128],
                        q4[32 * r : 32 * r + 32, ncol],
                        start=True,
                        stop=True,
                        tile_position=(32 * r, 0),
                    )
            if nb == 0:
                wvc_group(2 * g0, 2)
                wvc_group(2 * g1, 2)
            if nb == 1:
                ec_group(2 * g0, 2)
                ec_group(2 * g1, 2)
            if (nb, p) in qk_fill:
                qk_group(*qk_fill[(nb, p)])
            e_ts = []
            for g, s_t in zip((g0, g1), s_ts):
                e_t = epool.tile([128, 2 * BLK], FP8, tag="e", name="e_t")
                e_ts.append(e_t)
                with nc.allow_low_precision(reason="E in fp8: ~1e-4 on out"):
                    nc.scalar.activation(e_t[:], s_t[:], Exp, bias=nlog64[:])
            for g, e_t in zip((g0, g1), e_ts):
                lhsT = wt8[:, 2 * g * WP : (2 * g + 2) * WP].rearrange(
                    "p (two f) -> p two f", two=2
                )[:, :, 0:65]
                rhs = e_t[:].rearrange("p (two f) -> p two f", two=2)
                nc.tensor.matmul(
                    vacc[0 : C + 1, :],
                    lhsT,
                    rhs,
                    start=(g == 0),
                    stop=(g == NG - 1),
                    perf_mode=DR,
                )
            # deferred work, slotted into quiet spots mid-block:
            if nb >= 1 and p == 0:
                emit_recip(nb - 1)
            if nb >= 2 and p == 1:
                sc_pend[nb - 2] = epilogue_b1(nb - 2)
            if nb >= 2 and p == 3:
                epilogue_b2(nb - 2, sc_pend.pop(nb - 2))

        epilogue_a(nb)
    # ---- tail: last two blocks' epilogues ----
    sc6 = epilogue_b1(NB - 2)
    epilogue_b2(NB - 2, sc6)
    emit_recip(NB - 1)
    sc7 = epilogue_b1(NB - 1)
    epilogue_b2(NB - 1, sc7)


def build_nc():
    nc = bacc.Bacc(
        "TRN2",
        target_bir_lowering=False,
        debug=False,
        enable_asserts=False,
        num_devices=8,
    )
    io = {}
    io["x"] = nc.dram_tensor("x", [C, HW], F32, kind="ExternalInput").ap()
    io["wq4T"] = nc.dram_tensor("wq4T", [C, 128], BF16, kind="ExternalInput").ap()
    io["wk4T"] = nc.dram_tensor("wk4T", [C, 128], BF16, kind="ExternalInput").ap()
    io["wvc"] = nc.dram_tensor("wvc", [C, 128], BF16, kind="ExternalInput").ap()
    io["wbn1T"] = nc.dram_tensor("wbn1T", [C, C], F32, kind="ExternalInput").ap()
    io["id64"] = nc.dram_tensor("id64", [C, C], BF16, kind="ExternalInput").ap()
    io["out"] = nc.dram_tensor("out", [C, HW], F32, kind="ExternalOutput").ap()

    with tile.TileContext(nc) as tc:
        with ExitStack() as ctx:
            _build_kernel(ctx, tc, io)
    nc.compile()
    return nc


def make_in_maps(x, w_cam, w_q, w_k, w_v, w_bn):
    import ml_dtypes

    f = lambda a: np.ascontiguousarray(np.asarray(a, dtype=np.float32))
    fb = lambda a: np.ascontiguousarray(
        np.asarray(a, dtype=np.float32).astype(ml_dtypes.bfloat16)
    )
    w_bn = np.asarray(w_bn, dtype=np.float64)
    w_vp = w_bn[:, C:] @ np.asarray(w_v, dtype=np.float64)  # wbn2 folded into v
    base = {
        "wq4T": fb(np.concatenate([np.asarray(w_q).T] * 4, axis=1)),
        "wk4T": fb(np.concatenate([np.asarray(w_k).T] * 4, axis=1)),
        "wvc": fb(np.concatenate([w_vp.T, np.asarray(w_cam).T], axis=1)),
        "wbn1T": f(w_bn[:, :C].T),
        "id64": fb(np.eye(C)),
    }
    x = np.asarray(x)
    return [dict(base, x=f(x[b].reshape(C, HW))) for b in range(8)]


_NC_CACHE = None


def kernel(x, w_cam, w_q, w_k, w_v, w_bn):
    global _NC_CACHE
    if _NC_CACHE is None:
        _NC_CACHE = build_nc()
    nc = _NC_CACHE
    in_maps = make_in_maps(x, w_cam, w_q, w_k, w_v, w_bn)
    res = run_bass_kernel_spmd(nc, in_maps, list(range(8)))
    out = np.stack([res.results[b]["out"].reshape(C, 64, 64) for b in range(8)])
    return out.astype(np.float32)


# revision 14
# speedup vs baseline: 1.1656x; 1.0303x over previous
"""Trainium2 Bass kernel for dual-attention block (CAM + SAM + bottleneck).

Contract: kernel(**inputs) takes FULL unsharded inputs
  x     [8, 64, 64, 64] f32
  w_cam [64, 64], w_q [32, 64], w_k [32, 64], w_v [64, 64], w_bn [64, 128]
and returns the full [8, 64, 64, 64] f32 output.

Sharding: data-parallel over batch across 8 NeuronCores (1 image each);
weights replicated. Per-core math (c=64 channels, n=m=4096 spatial):

  CAM: xcT = x.T @ w_cam.T ; Ec = xcT.T @ xcT;
       attn_c = softmax_rows(Ec); bn = ((wbn1 @ attn_c) + I) @ x
       (the +I folds the residual x into the CAM bottleneck matmul)
  SAM: q4/k4 = (w stacked 4x) @ x  -> q,k replicated on 4 partition groups
       S[m,n] = sum_c k[c,m] q[c,n]  (row-tiled K=32 matmuls, 4-concurrent)
       E = exp(S - ln64) in fp8-e4m3  (max|S|=9.05 -> E'max 133 < 240;
           the 1/64 cancels between numerator and denominator)
       acc[c,n] = sum_m W[m,c] E[m,n]  with W = [v'.T | ones] in fp8 and
                  v' = (wbn2 @ w_v) x  (bottleneck conv folded into the
                  value weights on the host), one DoubleRow matmul per
                  m-tile PAIR (K=256) -> rows 0..63 = wbn2-projected SAM
                  contribution (unnormalized), row 64 = Z
  out = bn + acc[0:64] * (1/Z)
        (1/Z via custom-DVE fast reciprocal at partition 0, broadcast to
        64 partitions by GpSimd partition_broadcast)

v8 structure (v4 measured 221us, v7 206us):
  - spool: ONE 3-slot rotation (3 x 2 PSUM banks) for the S tiles.  With
    A/B ping-pong a pair's second S group had to wait for the previous
    exp, serializing the quadrant matmuls 2+2; with 3 slots a group's
    bank is free 3 exp-periods ahead, so all 4 K=32 quadrant matmuls of
    a pair issue back-to-back and run concurrently on disjoint row
    quadrants.  All other PSUM scratch (warm-up, q/k chunk production,
    wvc, bn, m1ps) rides the same rotation; vacc/EC keep 2 banks.
  - wbn2 folded into the DR weights host-side; the per-block bottleneck
    matmul on the SAM path is gone.  The residual +x is folded into the
    CAM bottleneck matmul as (M1+I) via a device identity add.
  - 1/Z: fast approx reciprocal (partition 0, via a 2KB DMA hop) and
    GpSimd partition_broadcast instead of a K=1 PE matmul.
  - Preamble: weight DMAs first on the GpSimd queue; x in 8 chunks over
    3 DMA queues; ~6us dense PE warm-up (HAM -> 2.4GHz) overlapping the
    x DMA; q/k chunks 0-1 as single FD=1024 matmuls right behind it.
PSUM: spool 3x2 + vacc/EC 2 = 8 banks.
"""

import sys
from contextlib import ExitStack

import numpy as np

if "/opt/trn_rl_repo" not in sys.path:
    sys.path.insert(0, "/opt/trn_rl_repo")

import concourse.bass as bass
import concourse.tile as tile
from concourse import bacc, mybir
from concourse.bass_utils import run_bass_kernel_spmd

F32 = mybir.dt.float32
BF16 = mybir.dt.bfloat16
FP8 = mybir.dt.float8e4

C = 64          # channels
HW = 4096       # 64*64 spatial
NB = 8          # number of 512-wide n blocks
BLK = 512
MT = 32         # m tiles of 128
NG = 16         # groups of 2 m-tiles per n-block
WP = 80         # wt8 per-m-tile stride (65 used; 80 for DoubleRow step%16==0)
NLOG64 = -4.1588830833596715

Exp = mybir.ActivationFunctionType.Exp
DR = mybir.MatmulPerfMode.DoubleRow


def _build_kernel(ctx: ExitStack, tc: tile.TileContext, io: dict):
    nc = tc.nc
    x_d = io["x"]
    out_d = io["out"]

    consts = ctx.enter_context(tc.tile_pool(name="consts", bufs=1))
    bigs = ctx.enter_context(tc.tile_pool(name="bigs", bufs=1))
    epool = ctx.enter_context(tc.tile_pool(name="epool", bufs=3))
    campool = ctx.enter_context(tc.tile_pool(name="campool", bufs=1))
    sampool = ctx.enter_context(tc.tile_pool(name="sampool", bufs=2))
    spool = ctx.enter_context(
        tc.tile_pool(name="spool", bufs=3, space=bass.MemorySpace.PSUM)
    )
    vpool = ctx.enter_context(
        tc.tile_pool(name="vpool", bufs=2, space=bass.MemorySpace.PSUM)
    )

    # ---- weight DMAs first, on the otherwise-idle GpSimd queue (tiny; if
    # they queued behind the 1MB x transfer the first matmul waits ~15us) --
    wq4T = consts.tile([C, 128], BF16)    # (w_q stacked 4x).T
    wk4T = consts.tile([C, 128], BF16)
    wvc = consts.tile([C, 128], BF16)     # [(wbn2 w_v).T | w_cam.T]
    wbn1T = consts.tile([C, C], F32)
    id64 = consts.tile([C, C], BF16)
    zb = consts.tile([128, 1], F32)
    nlog64 = consts.tile([128, 1], F32)   # exp bias: E'=E/64 fits fp8e4 max 240
    dummy = consts.tile([128, 1], F32)
    warm_w = consts.tile([128, BLK], BF16)

    nc.gpsimd.dma_start(wk4T[:], io["wk4T"][:])
    nc.gpsimd.dma_start(wq4T[:], io["wq4T"][:])
    nc.gpsimd.dma_start(wvc[:], io["wvc"][:])
    nc.gpsimd.dma_start(wbn1T[:], io["wbn1T"][:])
    nc.gpsimd.dma_start(id64[:], io["id64"][:])

    # ---- x DMA: 8 column chunks round-robin over 3 HWDGE queues (each
    # queue sustains only ~100 GB/s; the first 1024 columns -- all that
    # k01/q01 need -- land first) ----
    x_sb = bigs.tile([C, HW], F32)
    x_qs = [nc.sync, nc.scalar, nc.gpsimd]
    for xc_ in range(8):
        x_qs[xc_ % 3].dma_start(
            x_sb[:, xc_ * BLK : (xc_ + 1) * BLK],
            x_d[:, xc_ * BLK : (xc_ + 1) * BLK],
        )

    # warm_w memset is the FIRST DVE op so the PE warm-up burst below can
    # start as early as possible.
    nc.vector.memset(warm_w[:], 1.0)
    nc.vector.memset(zb[:], 0.0)
    # Trigger the exp ACT-table load right behind the x-DMA issue (overlaps
    # the transfer) instead of in front of the first real exp.
    nc.scalar.activation(dummy[:], zb[:], Exp, bias=zb[:])
    nc.vector.memset(nlog64[:], NLOG64)

    q4 = bigs.tile([128, HW], BF16)
    k4 = bigs.tile([128, HW], BF16)
    wt8 = bigs.tile([128, MT * WP], FP8)   # per m-tile [v'T | ones | pad]
    xct = bigs.tile([128, MT * C], BF16)   # xcT, m-tile-major
    x_bf = bigs.tile([C, HW], BF16)

    # ---- PE warm-up: ~6us of dense back-to-back matmuls on junk data
    # while the x DMA is in flight.  The HAM clock gate needs a full
    # free-running ~3.4us window of sustained PE busy to lift the PE from
    # 1.2 to 2.4 GHz; without this every preamble matmul runs at half
    # clock. ----
    for wp_ in range(5):
        warm_ps = spool.tile([128, BLK], F32, tag="s", name="warm_ps")
        nc.tensor.matmul(
            warm_ps[:], warm_w[:, 0:128], warm_w[:], start=True, stop=True
        )

    # ones column of wt8 (wvc copies below only write cols 0..63)
    nc.vector.memset(
        wt8[:].rearrange("p (t c) -> p t c", c=WP)[:, :, 64:65], 1.0
    )

    # x in bf16 feeds the q4/k4/wvc/bn matmuls at full PE rate; 8 chunks so
    # the first q/k matmuls start as soon as the first x columns land.
    for xc_ in range(8):
        nc.vector.tensor_copy(
            x_bf[:, xc_ * BLK : (xc_ + 1) * BLK], x_sb[:, xc_ * BLK : (xc_ + 1) * BLK]
        )

    # ---- q4 / k4: replicated q,k via stacked-weight 1x1 convs.  Each
    # 2-chunk group is ONE FD=1024 matmul.  Chunks 0-1 of k and q are
    # produced up front; the rest are fill-in groups inside the block
    # loop, each 2+ pairs ahead of its consumption deadline. ----
    def qk_group(which, cch, nch=1, on_scalar=False):
        wT, dst = (wk4T, k4) if which == "k" else (wq4T, q4)
        ps = spool.tile([128, nch * BLK], F32, tag="s", name="qkps")
        for i in range(nch):
            nc.tensor.matmul(
                ps[:, i * BLK : (i + 1) * BLK],
                wT[:],
                x_bf[:, (cch + i) * BLK : (cch + i + 1) * BLK],
                start=True,
                stop=True,
            )
        lo = cch * BLK
        if on_scalar:
            nc.scalar.copy(dst[:, lo : lo + nch * BLK], ps[:])
        else:
            nc.vector.tensor_copy(dst[:, lo : lo + nch * BLK], ps[:])

    qk_group("k", 0, nch=2)
    qk_group("q", 0, nch=2, on_scalar=True)

    # (block, pair) -> (which, chunk); deadlines: k chunk c is consumed
    # at block-0 pair c; q chunk c at block c.
    qk_fill = {
        (0, 0): ("k", 2), (0, 1): ("k", 3), (0, 2): ("k", 4),
        (0, 3): ("k", 5), (0, 4): ("k", 6), (0, 5): ("k", 7),
        (0, 6): ("q", 2), (0, 7): ("q", 3), (1, 1): ("q", 4),
        (1, 3): ("q", 5), (1, 5): ("q", 6), (1, 7): ("q", 7),
    }

    state = {}  # EC tile, allocated at block 1 start (vpool slot timing)

    def wvc_group(base, size):
        """xcT and WT (=[v'T|ones]) production for one m-tile group."""
        ps_w = spool.tile([128, BLK], F32, tag="s", name="wvcps")
        for j in range(size):
            m = base + j
            nc.tensor.matmul(
                ps_w[:, j * 128 : (j + 1) * 128],
                x_bf[:, m * 128 : (m + 1) * 128],
                wvc[:],
                start=True,
                stop=True,
            )
        src = ps_w[:, : size * 128].rearrange("p (j c) -> p j c", c=128)
        wt_dst = wt8[:, base * WP : (base + size) * WP].rearrange(
            "p (j c) -> p j c", c=WP
        )
        with nc.allow_low_precision(reason="v' in fp8 for DoubleRow acc"):
            nc.vector.tensor_copy(wt_dst[:, :, 0:C], src[:, :, 0:C])
        xct_dst = xct[:, base * C : (base + size) * C].rearrange(
            "p (j c) -> p j c", c=C
        )
        with nc.allow_low_precision(reason="xcT in bf16 for cheap ec matmuls"):
            nc.vector.tensor_copy(xct_dst, src[:, :, C : 2 * C])

    def ec_group(base, size):
        EC = state["EC"]
        for j in range(size):
            m = base + j
            nc.tensor.matmul(
                EC[0:C, 0:C],
                xct[:, m * C : (m + 1) * C],
                xct[:, m * C : (m + 1) * C],
                start=(m == 0),
                stop=(m == MT - 1),
            )

    # ---- per-block state for split epilogues ----
    vaccs = [None] * NB
    sam = [None] * NB   # sam65 [65, BLK] f32: rows 0..63 unnorm SAM out, 64 = Z
    rzs = [None] * NB   # rz [1, BLK] bf16 at partition 0
    M1T_sb = campool.tile([C, C], BF16)

    def epilogue_a(nb):
        """At block end: evacuate vacc (recip is emitted separately)."""
        aux = sampool.tile([C + 1, BLK], F32, tag="aux", name="aux")
        nc.vector.tensor_copy(aux[:], vaccs[nb][0 : C + 1, :])
        sam[nb] = aux

    def emit_recip(nb):
        """1/Z for block nb via the fast approx recip + bf16 cast.

        The custom DVE op only works at base partition 0 (and DVE lanes
        cannot move data across partitions), so the Z row is first moved
        from partition 64 to partition 0 by a tiny SBUF->SBUF DMA on the
        otherwise-idle sync queue.
        """
        z0 = sampool.tile([1, BLK], F32, tag="z0", name="z0")
        nc.sync.dma_start(z0[:], sam[nb][C : C + 1, :])
        rz32 = sampool.tile([1, BLK], F32, tag="rz32", name="rz32")
        nc.vector.reciprocal_approx_fast(rz32[:], z0[:])
        rzb = sampool.tile([1, BLK], BF16, tag="rz", name="rzb")
        with nc.allow_low_precision(reason="1/Z in bf16: 0.4% on the SAM term"):
            nc.vector.tensor_copy(rzb[:], rz32[:])
        rzs[nb] = rzb

    def epilogue_b1(nb):
        """Broadcast 1/Z to 64 partitions (GpSimd) and scale the SAM rows."""
        bcast = sampool.tile([C, BLK], BF16, tag="bc", name="bcast")
        nc.gpsimd.partition_broadcast(bcast[:], rzs[nb][:])
        sam_sc = sampool.tile([C, BLK], F32, tag="sc", name="sam_sc")
        nc.vector.tensor_mul(sam_sc[:], sam[nb][0:C, :], bcast[:])
        return sam_sc

    def epilogue_b2(nb, sam_sc):
        """CAM bottleneck (+residual via I) matmul, add SAM term, DMA out."""
        ncol = slice(nb * BLK, (nb + 1) * BLK)
        bn = spool.tile([128, BLK], F32, tag="s", name="bn")
        nc.tensor.matmul(
            bn[0:C, :], M1T_sb[:], x_bf[:, ncol], start=True, stop=True
        )
        o_t = sampool.tile([C, BLK], F32, tag="ot", name="o_t")
        nc.vector.tensor_add(o_t[:], bn[0:C, :], sam_sc[:])
        nc.sync.dma_start(out_d[:, ncol], o_t[:])

    def cam_chain():
        """CAM softmax -> attn_c -> M1T = (wbn1 @ attn_c).T + I"""
        EC = state["EC"]
        negmax = campool.tile([C, 1], F32)
        nc.vector.reduce_max(
            negmax[:], EC[0:C, 0:C], axis=mybir.AxisListType.X, negate=True
        )
        exp_c = campool.tile([C, C], F32)
        nc.scalar.activation(exp_c[:], EC[0:C, 0:C], Exp, bias=negmax[:])
        sum_c = campool.tile([C, 1], F32)
        nc.vector.reduce_sum(sum_c[:], exp_c[:], axis=mybir.AxisListType.X)
        rec_c = campool.tile([C, 1], F32)
        nc.vector.reciprocal(rec_c[:], sum_c[:])
        attn_c = campool.tile([C, C], F32)
        nc.vector.tensor_scalar_mul(attn_c[:], exp_c[:], rec_c[:])
        m1ps = spool.tile([128, BLK], F32, tag="s", name="m1ps")
        nc.tensor.matmul(
            m1ps[0:C, 0:C], attn_c[:], wbn1T[:], start=True, stop=True
        )
        with nc.allow_low_precision(reason="M1T in bf16 feeds a bf16 matmul"):
            nc.vector.tensor_add(M1T_sb[:], m1ps[0:C, 0:C], id64[:])

    # ---- main SAM loop over 8 n-blocks, groups emitted in PAIRS ----
    sc_pend = {}  # nb -> sam_sc awaiting epilogue_b2
    for nb in range(NB):
        if nb == 1:
            # EC takes a vpool rotation slot; its last readers (CAM softmax,
            # start of block 2) finish before vacc(2) re-claims the slot.
            state["EC"] = vpool.tile([128, BLK], F32, tag="v", name="EC")
        if nb == 2:
            # CAM chain first so vacc(2), which aliases EC's bank, only
            # waits on the (fast) softmax reads of EC.
            cam_chain()
        vacc = vpool.tile([128, BLK], F32, tag="v", name="vacc")
        vaccs[nb] = vacc
        ncol = slice(nb * BLK, (nb + 1) * BLK)
        for p in range(NG // 2):
            g0, g1 = 2 * p, 2 * p + 1
            s_ts = []
            for g in (g0, g1):
                s_t = spool.tile([128, 2 * BLK], F32, tag="s", name="s_t")
                s_ts.append(s_t)
                for j in range(2):
                    m = 2 * g + j
                    r = 2 * (g % 2) + j  # row quadrants 0,1 / 2,3
                    nc.tensor.matmul(
                        s_t[:, j * BLK : (j + 1) * BLK],
                        k4[32 * r : 32 * r + 32, m * 128 : (m + 1) * 128],
                        q4[32 * r : 32 * r + 32, ncol],
                        start=True,
                        stop=True,
                        tile_position=(32 * r, 0),
                    )
            if nb == 0:
                wvc_group(2 * g0, 2)
                wvc_group(2 * g1, 2)
            if nb == 1:
                ec_group(2 * g0, 2)
                ec_group(2 * g1, 2)
            if (nb, p) in qk_fill:
                qk_group(*qk_fill[(nb, p)])
            e_ts = []
            for g, s_t in zip((g0, g1), s_ts):
                e_t = epool.tile([128, 2 * BLK], FP8, tag="e", name="e_t")
                e_ts.append(e_t)
                with nc.allow_low_precision(reason="E in fp8: ~1e-4 on out"):
                    nc.scalar.activation(e_t[:], s_t[:], Exp, bias=nlog64[:])
            for g, e_t in zip((g0, g1), e_ts):
                lhsT = wt8[:, 2 * g * WP : (2 * g + 2) * WP].rearrange(
                    "p (two f) -> p two f", two=2
                )[:, :, 0:65]
                rhs = e_t[:].rearrange("p (two f) -> p two f", two=2)
                nc.tensor.matmul(
                    vacc[0 : C + 1, :],
                    lhsT,
                    rhs,
                    start=(g == 0),
                    stop=(g == NG - 1),
                    perf_mode=DR,
                )
            # deferred work, slotted into quiet spots mid-block:
            if nb >= 1 and p == 0:
                emit_recip(nb - 1)
            if nb >= 2 and p == 1:
                sc_pend[nb - 2] = epilogue_b1(nb - 2)
            if nb >= 2 and p == 3:
                epilogue_b2(nb - 2, sc_pend.pop(nb - 2))

        epilogue_a(nb)
    # ---- tail: last two blocks' epilogues ----
    sc6 = epilogue_b1(NB - 2)
    epilogue_b2(NB - 2, sc6)
    emit_recip(NB - 1)
    sc7 = epilogue_b1(NB - 1)
    epilogue_b2(NB - 1, sc7)


def build_nc():
    nc = bacc.Bacc(
        "TRN2",
        target_bir_lowering=False,
        debug=False,
        enable_asserts=False,
        num_devices=8,
    )
    io = {}
    io["x"] = nc.dram_tensor("x", [C, HW], F32, kind="ExternalInput").ap()
    io["wq4T"] = nc.dram_tensor("wq4T", [C, 128], BF16, kind="ExternalInput").ap()
    io["wk4T"] = nc.dram_tensor("wk4T", [C, 128], BF16, kind="ExternalInput").ap()
    io["wvc"] = nc.dram_tensor("wvc", [C, 128], BF16, kind="ExternalInput").ap()
    io["wbn1T"] = nc.dram_tensor("wbn1T", [C, C], F32, kind="ExternalInput").ap()
    io["id64"] = nc.dram_tensor("id64", [C, C], BF16, kind="ExternalInput").ap()
    io["out"] = nc.dram_tensor("out", [C, HW], F32, kind="ExternalOutput").ap()

    with tile.TileContext(nc) as tc:
        with ExitStack() as ctx:
            _build_kernel(ctx, tc, io)
    nc.compile()
    return nc


def make_in_maps(x, w_cam, w_q, w_k, w_v, w_bn):
    import ml_dtypes

    f = lambda a: np.ascontiguousarray(np.asarray(a, dtype=np.float32))
    fb = lambda a: np.ascontiguousarray(
        np.asarray(a, dtype=np.float32).astype(ml_dtypes.bfloat16)
    )
    w_bn = np.asarray(w_bn, dtype=np.float64)
    w_vp = w_bn[:, C:] @ np.asarray(w_v, dtype=np.float64)  # wbn2 folded into v
    base = {
        "wq4T": fb(np.concatenate([np.asarray(w_q).T] * 4, axis=1)),
        "wk4T": fb(np.concatenate([np.asarray(w_k).T] * 4, axis=1)),
        "wvc": fb(np.concatenate([w_vp.T, np.asarray(w_cam).T], axis=1)),
        "wbn1T": f(w_bn[:, :C].T),
        "id64": fb(np.eye(C)),
    }
    x = np.asarray(x)
    return [dict(base, x=f(x[b].reshape(C, HW))) for b in range(8)]


_NC_CACHE = None


def kernel(x, w_cam, w_q, w_k, w_v, w_bn):
    global _NC_CACHE
    if _NC_CACHE is None:
        _NC_CACHE = build_nc()
    nc = _NC_CACHE
    in_maps = make_in_maps(x, w_cam, w_q, w_k, w_v, w_bn)
    res = run_bass_kernel_spmd(nc, in_maps, list(range(8)))
    out = np.stack([res.results[b]["out"].reshape(C, 64, 64) for b in range(8)])
    return out.astype(np.float32)


# revision 17
# speedup vs baseline: 1.1977x; 1.0275x over previous
"""Trainium2 Bass kernel for dual-attention block (CAM + SAM + bottleneck).

Contract: kernel(**inputs) takes FULL unsharded inputs
  x     [8, 64, 64, 64] f32
  w_cam [64, 64], w_q [32, 64], w_k [32, 64], w_v [64, 64], w_bn [64, 128]
and returns the full [8, 64, 64, 64] f32 output.

Sharding: data-parallel over batch across 8 NeuronCores (1 image each);
weights replicated. Per-core math (c=64 channels, n=m=4096 spatial):

  CAM: xcT = x.T @ w_cam.T ; Ec = xcT.T @ xcT;
       attn_c = softmax_rows(Ec); bn = ((wbn1 @ attn_c) + I) @ x
       (the +I folds the residual x into the CAM bottleneck matmul)
  SAM: q4/k4 = (w stacked 4x) @ x  -> q,k replicated on 4 partition groups
       S[m,n] = sum_c k[c,m] q[c,n]  (row-tiled K=32 matmuls, 4-concurrent)
       E = exp(S - ln64) in fp8-e4m3  (max|S|=9.05 -> E'max 133 < 240;
           the 1/64 cancels between numerator and denominator)
       acc[c,n] = sum_m W[m,c] E[m,n]  with W = [v'.T | ones] in fp8 and
                  v' = (wbn2 @ w_v) x  (bottleneck conv folded into the
                  value weights on the host), one DoubleRow matmul per
                  m-tile PAIR (K=256) -> rows 0..63 = wbn2-projected SAM
                  contribution (unnormalized), row 64 = Z
  out = bn + acc[0:64] * (1/Z)
        (1/Z via custom-DVE fast reciprocal at partition 0, broadcast to
        64 partitions by GpSimd partition_broadcast)

v8 structure (v4 measured 221us, v7 206us):
  - spool: ONE 3-slot rotation (3 x 2 PSUM banks) for the S tiles.  With
    A/B ping-pong a pair's second S group had to wait for the previous
    exp, serializing the quadrant matmuls 2+2; with 3 slots a group's
    bank is free 3 exp-periods ahead, so all 4 K=32 quadrant matmuls of
    a pair issue back-to-back and run concurrently on disjoint row
    quadrants.  All other PSUM scratch (warm-up, q/k chunk production,
    wvc, bn, m1ps) rides the same rotation; vacc/EC keep 2 banks.
  - wbn2 folded into the DR weights host-side; the per-block bottleneck
    matmul on the SAM path is gone.  The residual +x is folded into the
    CAM bottleneck matmul as (M1+I) via a device identity add.
  - 1/Z: fast approx reciprocal (partition 0, via a 2KB DMA hop) and
    GpSimd partition_broadcast instead of a K=1 PE matmul.
  - Preamble: weight DMAs first on the GpSimd queue; x in 8 chunks over
    3 DMA queues; ~6us dense PE warm-up (HAM -> 2.4GHz) overlapping the
    x DMA; q/k chunks 0-1 as single FD=1024 matmuls right behind it.
PSUM: spool 3x2 + vacc/EC 2 = 8 banks.
"""

import sys
from contextlib import ExitStack

import numpy as np

if "/opt/trn_rl_repo" not in sys.path:
    sys.path.insert(0, "/opt/trn_rl_repo")

import concourse.bass as bass
import concourse.tile as tile
from concourse import bacc, mybir
from concourse.bass_utils import run_bass_kernel_spmd

F32 = mybir.dt.float32
BF16 = mybir.dt.bfloat16
FP8 = mybir.dt.float8e4

C = 64          # channels
HW = 4096       # 64*64 spatial
NB = 8          # number of 512-wide n blocks
BLK = 512
MT = 32         # m tiles of 128
NG = 16         # groups of 2 m-tiles per n-block
WP = 80         # wt8 per-m-tile stride (65 used; 80 for DoubleRow step%16==0)
NLOG64 = -4.1588830833596715

Exp = mybir.ActivationFunctionType.Exp
DR = mybir.MatmulPerfMode.DoubleRow


def _build_kernel(ctx: ExitStack, tc: tile.TileContext, io: dict):
    nc = tc.nc
    x_d = io["x"]
    out_d = io["out"]

    consts = ctx.enter_context(tc.tile_pool(name="consts", bufs=1))
    bigs = ctx.enter_context(tc.tile_pool(name="bigs", bufs=1))
    epool = ctx.enter_context(tc.tile_pool(name="epool", bufs=3))
    campool = ctx.enter_context(tc.tile_pool(name="campool", bufs=1))
    sampool = ctx.enter_context(tc.tile_pool(name="sampool", bufs=4))
    spool = ctx.enter_context(
        tc.tile_pool(name="spool", bufs=3, space=bass.MemorySpace.PSUM)
    )
    vpool = ctx.enter_context(
        tc.tile_pool(name="vpool", bufs=2, space=bass.MemorySpace.PSUM)
    )

    # ---- weight DMAs first, on the otherwise-idle GpSimd queue (tiny; if
    # they queued behind the 1MB x transfer the first matmul waits ~15us) --
    wq4T = consts.tile([C, 128], BF16)    # (w_q stacked 4x).T
    wk4T = consts.tile([C, 128], BF16)
    wvc = consts.tile([C, 128], BF16)     # [(wbn2 w_v).T | w_cam.T]
    wbn1T = consts.tile([C, C], F32)
    id64 = consts.tile([C, C], BF16)
    zb = consts.tile([128, 1], F32)
    nlog64 = consts.tile([128, 1], F32)   # exp bias: E'=E/64 fits fp8e4 max 240
    dummy = consts.tile([128, 1], F32)

    nc.gpsimd.dma_start(wk4T[:], io["wk4T"][:])
    nc.gpsimd.dma_start(wq4T[:], io["wq4T"][:])
    nc.gpsimd.dma_start(wvc[:], io["wvc"][:])
    nc.gpsimd.dma_start(wbn1T[:], io["wbn1T"][:])
    nc.gpsimd.dma_start(id64[:], io["id64"][:])

    # ---- x DMA: 8 column chunks round-robin over 3 HWDGE queues (each
    # queue sustains only ~100 GB/s; the first 1024 columns -- all that
    # k01/q01 need -- land first) ----
    x_sb = bigs.tile([C, HW], F32)
    x_qs = [nc.sync, nc.scalar, nc.gpsimd]
    for xc_ in range(8):
        x_qs[xc_ % 3].dma_start(
            x_sb[:, xc_ * BLK : (xc_ + 1) * BLK],
            x_d[:, xc_ * BLK : (xc_ + 1) * BLK],
        )

    nc.vector.memset(zb[:], 0.0)
    # Trigger the exp ACT-table load right behind the x-DMA issue (overlaps
    # the transfer) instead of in front of the first real exp.
    nc.scalar.activation(dummy[:], zb[:], Exp, bias=zb[:])
    nc.vector.memset(nlog64[:], NLOG64)

    q4 = bigs.tile([128, HW], BF16)
    k4 = bigs.tile([128, HW], BF16)
    wt8 = bigs.tile([128, MT * WP], FP8)   # per m-tile [v'T | ones | pad]
    xct = bigs.tile([128, MT * C], BF16)   # xcT, m-tile-major
    x_bf = bigs.tile([C, HW], BF16)
    e0 = bigs.tile([128, NG * 2 * BLK], FP8)  # block-0 E, consumed in block 1

    # ones column of wt8 (wvc copies below only write cols 0..63)
    nc.vector.memset(
        wt8[:].rearrange("p (t c) -> p t c", c=WP)[:, :, 64:65], 1.0
    )

    # x in bf16 feeds the q4/k4/wvc/bn matmuls at full PE rate; 8 chunks so
    # the first q/k matmuls start as soon as the first x columns land.
    for xc_ in range(8):
        nc.vector.tensor_copy(
            x_bf[:, xc_ * BLK : (xc_ + 1) * BLK], x_sb[:, xc_ * BLK : (xc_ + 1) * BLK]
        )

    # ---- q4 / k4: replicated q,k via stacked-weight 1x1 convs.  Each
    # 2-chunk group is ONE FD=1024 matmul.  Chunks 0-1 of k and q are
    # produced up front; the rest are fill-in groups inside the block
    # loop, each 2+ pairs ahead of its consumption deadline. ----
    def qk_group(which, cch, nch=1, on_scalar=False):
        wT, dst = (wk4T, k4) if which == "k" else (wq4T, q4)
        ps = spool.tile([128, nch * BLK], F32, tag="s", name="qkps")
        for i in range(nch):
            nc.tensor.matmul(
                ps[:, i * BLK : (i + 1) * BLK],
                wT[:],
                x_bf[:, (cch + i) * BLK : (cch + i + 1) * BLK],
                start=True,
                stop=True,
            )
        lo = cch * BLK
        if on_scalar:
            nc.scalar.copy(dst[:, lo : lo + nch * BLK], ps[:])
        else:
            nc.vector.tensor_copy(dst[:, lo : lo + nch * BLK], ps[:])

    qk_group("k", 0, nch=2)
    qk_group("q", 0, nch=2, on_scalar=True)

    # (block, pair) -> (which, chunk); deadlines: k chunk c is consumed
    # at block-0 pair c; q chunk c at block c.
    qk_fill = {
        (0, 0): ("k", 2), (0, 1): ("k", 3), (0, 2): ("k", 4),
        (0, 3): ("k", 5), (0, 4): ("k", 6), (0, 5): ("k", 7),
        (1, 1): ("q", 2), (1, 5): ("q", 3), (2, 1): ("q", 4),
        (2, 5): ("q", 5), (3, 1): ("q", 6), (3, 5): ("q", 7),
    }

    state = {}  # EC tile, allocated at block 1 start (vpool slot timing)

    def wvc_group(base, size):
        """xcT and WT (=[v'T|ones]) production for one m-tile group."""
        ps_w = spool.tile([128, BLK], F32, tag="s", name="wvcps")
        for j in range(size):
            m = base + j
            nc.tensor.matmul(
                ps_w[:, j * 128 : (j + 1) * 128],
                x_bf[:, m * 128 : (m + 1) * 128],
                wvc[:],
                start=True,
                stop=True,
            )
        src = ps_w[:, : size * 128].rearrange("p (j c) -> p j c", c=128)
        wt_dst = wt8[:, base * WP : (base + size) * WP].rearrange(
            "p (j c) -> p j c", c=WP
        )
        with nc.allow_low_precision(reason="v' in fp8 for DoubleRow acc"):
            nc.vector.tensor_copy(wt_dst[:, :, 0:C], src[:, :, 0:C])
        xct_dst = xct[:, base * C : (base + size) * C].rearrange(
            "p (j c) -> p j c", c=C
        )
        with nc.allow_low_precision(reason="xcT in bf16 for cheap ec matmuls"):
            nc.vector.tensor_copy(xct_dst, src[:, :, C : 2 * C])

    def ec_group(base, size):
        EC = state["EC"]
        for j in range(size):
            m = base + j
            nc.tensor.matmul(
                EC[0:C, 0:C],
                xct[:, m * C : (m + 1) * C],
                xct[:, m * C : (m + 1) * C],
                start=(m == 0),
                stop=(m == MT - 1),
            )

    # ---- per-block state for split epilogues ----
    vaccs = [None] * NB
    sam = [None] * NB   # sam65 [65, BLK] f32: rows 0..63 unnorm SAM out, 64 = Z
    rzs = [None] * NB   # rz [1, BLK] bf16 at partition 0
    M1T_sb = campool.tile([C, C], BF16)

    def epilogue_a(nb):
        """At block end: evacuate vacc (recip is emitted separately)."""
        aux = sampool.tile([C + 1, BLK], F32, tag="aux", name="aux")
        nc.vector.tensor_copy(aux[:], vaccs[nb][0 : C + 1, :])
        sam[nb] = aux

    def emit_recip(nb):
        """1/Z for block nb via the fast approx recip + bf16 cast.

        The custom DVE op only works at base partition 0 (and DVE lanes
        cannot move data across partitions), so the Z row is first moved
        from partition 64 to partition 0 by a tiny SBUF->SBUF DMA on the
        otherwise-idle sync queue.
        """
        z0 = sampool.tile([1, BLK], F32, tag="z0", name="z0")
        nc.sync.dma_start(z0[:], sam[nb][C : C + 1, :])
        rz32 = sampool.tile([1, BLK], F32, tag="rz32", name="rz32")
        nc.vector.reciprocal_approx_fast(rz32[:], z0[:])
        rzb = sampool.tile([1, BLK], BF16, tag="rz", name="rzb")
        with nc.allow_low_precision(reason="1/Z in bf16: 0.4% on the SAM term"):
            nc.vector.tensor_copy(rzb[:], rz32[:])
        rzs[nb] = rzb

    def epilogue_b1(nb):
        """Broadcast 1/Z to 64 partitions (GpSimd) and scale the SAM rows."""
        bcast = sampool.tile([C, BLK], BF16, tag="bc", name="bcast")
        nc.gpsimd.partition_broadcast(bcast[:], rzs[nb][:])
        sam_sc = sampool.tile([C, BLK], F32, tag="sc", name="sam_sc")
        nc.vector.tensor_mul(sam_sc[:], sam[nb][0:C, :], bcast[:])
        return sam_sc

    def epilogue_b2(nb, sam_sc):
        """CAM bottleneck (+residual via I) matmul, add SAM term, DMA out."""
        ncol = slice(nb * BLK, (nb + 1) * BLK)
        bn = spool.tile([128, BLK], F32, tag="s", name="bn")
        nc.tensor.matmul(
            bn[0:C, :], M1T_sb[:], x_bf[:, ncol], start=True, stop=True
        )
        o_t = sampool.tile([C, BLK], F32, tag="ot", name="o_t")
        nc.vector.tensor_add(o_t[:], bn[0:C, :], sam_sc[:])
        nc.sync.dma_start(out_d[:, ncol], o_t[:])

    def cam_chain():
        """CAM softmax -> attn_c -> M1T = (wbn1 @ attn_c).T + I"""
        EC = state["EC"]
        negmax = campool.tile([C, 1], F32)
        nc.vector.reduce_max(
            negmax[:], EC[0:C, 0:C], axis=mybir.AxisListType.X, negate=True
        )
        exp_c = campool.tile([C, C], F32)
        nc.scalar.activation(exp_c[:], EC[0:C, 0:C], Exp, bias=negmax[:])
        sum_c = campool.tile([C, 1], F32)
        nc.vector.reduce_sum(sum_c[:], exp_c[:], axis=mybir.AxisListType.X)
        rec_c = campool.tile([C, 1], F32)
        nc.vector.reciprocal(rec_c[:], sum_c[:])
        attn_c = campool.tile([C, C], F32)
        nc.vector.tensor_scalar_mul(attn_c[:], exp_c[:], rec_c[:])
        m1ps = spool.tile([128, BLK], F32, tag="s", name="m1ps")
        nc.tensor.matmul(
            m1ps[0:C, 0:C], attn_c[:], wbn1T[:], start=True, stop=True
        )
        with nc.allow_low_precision(reason="M1T in bf16 feeds a bf16 matmul"):
            nc.vector.tensor_add(M1T_sb[:], m1ps[0:C, 0:C], id64[:])

    # ---- main SAM loop over 8 n-blocks, groups emitted in PAIRS ----
    # Block 0 carries wvc + k-chunk fill-ins instead of its DR matmuls
    # (which would blow its PE budget); its E tiles persist in e0 and the
    # 16 deferred DRs ride block 1's slack.  ec runs in blocks 2-3 and the
    # CAM chain at block 4; the vacc/EC vpool rotation is:
    #   vacc0(A) vacc1(B) | vacc2(A) EC(B) | vacc3(A) | CAM, vacc4(B) |
    #   vacc5(A) | vacc6(B) | vacc7(A)
    # each claim one aux-evacuation behind its slot's previous tenant.
    sc_pend = {}
    recip_sched = {(2, 0): 0, (2, 4): 1, (3, 0): 2, (4, 0): 3,
                   (5, 0): 4, (6, 0): 5, (7, 0): 6}
    b1_sched = {(4, 1): 0, (4, 5): 1, (5, 1): 2, (5, 5): 3,
                (6, 1): 4, (6, 5): 5, (7, 2): 6}
    b2_sched = {(4, 3): 0, (4, 7): 1, (5, 3): 2, (5, 7): 3,
                (6, 3): 4, (6, 7): 5, (7, 4): 6}
    for nb in range(NB):
        if nb == 1:
            vaccs[0] = vpool.tile([128, BLK], F32, tag="v", name="vacc0")
        if nb == 4:
            cam_chain()
        if nb != 0:
            vacc = vpool.tile([128, BLK], F32, tag="v", name="vacc")
            vaccs[nb] = vacc
        if nb == 2:
            # EC right after vacc2: slot B, re-claimed by vacc4 after the
            # CAM chain at block 4 has consumed EC.
            state["EC"] = vpool.tile([128, BLK], F32, tag="v", name="EC")
        ncol = slice(nb * BLK, (nb + 1) * BLK)
        for p in range(NG // 2):
            g0, g1 = 2 * p, 2 * p + 1
            s_ts = []
            for g in (g0, g1):
                s_t = spool.tile([128, 2 * BLK], F32, tag="s", name="s_t")
                s_ts.append(s_t)
                for j in range(2):
                    m = 2 * g + j
                    r = 2 * (g % 2) + j  # row quadrants 0,1 / 2,3
                    nc.tensor.matmul(
                        s_t[:, j * BLK : (j + 1) * BLK],
                        k4[32 * r : 32 * r + 32, m * 128 : (m + 1) * 128],
                        q4[32 * r : 32 * r + 32, ncol],
                        start=True,
                        stop=True,
                        tile_position=(32 * r, 0),
                    )
            if nb == 0:
                wvc_group(2 * g0, 2)
                wvc_group(2 * g1, 2)
            if nb in (2, 3):
                ec_group((nb - 2) * NG + 2 * p, 2)
            if (nb, p) in qk_fill:
                qk_group(*qk_fill[(nb, p)])
            e_ts = []
            for g, s_t in zip((g0, g1), s_ts):
                if nb == 0:
                    e_t = e0[:, 2 * g * BLK : 2 * (g + 1) * BLK]
                else:
                    e_t = epool.tile([128, 2 * BLK], FP8, tag="e", name="e_t")
                e_ts.append(e_t)
                with nc.allow_low_precision(reason="E in fp8: ~1e-4 on out"):
                    nc.scalar.activation(e_t[:], s_t[:], Exp, bias=nlog64[:])
            if nb != 0:
                for g, e_t in zip((g0, g1), e_ts):
                    lhsT = wt8[:, 2 * g * WP : (2 * g + 2) * WP].rearrange(
                        "p (two f) -> p two f", two=2
                    )[:, :, 0:65]
                    rhs = e_t[:].rearrange("p (two f) -> p two f", two=2)
                    nc.tensor.matmul(
                        vaccs[nb][0 : C + 1, :],
                        lhsT,
                        rhs,
                        start=(g == 0),
                        stop=(g == NG - 1),
                        perf_mode=DR,
                    )
            if nb == 1:
                # block 0's deferred DR matmuls, 2 per pair
                for g in (g0, g1):
                    lhsT = wt8[:, 2 * g * WP : (2 * g + 2) * WP].rearrange(
                        "p (two f) -> p two f", two=2
                    )[:, :, 0:65]
                    rhs = e0[:, 2 * g * BLK : 2 * (g + 1) * BLK].rearrange(
                        "p (two f) -> p two f", two=2
                    )
                    nc.tensor.matmul(
                        vaccs[0][0 : C + 1, :],
                        lhsT,
                        rhs,
                        start=(g == 0),
                        stop=(g == NG - 1),
                        perf_mode=DR,
                    )
            if (nb, p) in recip_sched:
                emit_recip(recip_sched[(nb, p)])
            if (nb, p) in b1_sched:
                s = b1_sched[(nb, p)]
                sc_pend[s] = epilogue_b1(s)
            if (nb, p) in b2_sched:
                s = b2_sched[(nb, p)]
                epilogue_b2(s, sc_pend.pop(s))

        if nb == 1:
            epilogue_a(0)
            epilogue_a(1)
        elif nb != 0:
            epilogue_a(nb)
    # ---- tail: only the last block's epilogue chain remains ----
    emit_recip(NB - 1)
    sc7 = epilogue_b1(NB - 1)
    epilogue_b2(NB - 1, sc7)


def build_nc():
    nc = bacc.Bacc(
        "TRN2",
        target_bir_lowering=False,
        debug=False,
        enable_asserts=False,
        num_devices=8,
    )
    io = {}
    io["x"] = nc.dram_tensor("x", [C, HW], F32, kind="ExternalInput").ap()
    io["wq4T"] = nc.dram_tensor("wq4T", [C, 128], BF16, kind="ExternalInput").ap()
    io["wk4T"] = nc.dram_tensor("wk4T", [C, 128], BF16, kind="ExternalInput").ap()
    io["wvc"] = nc.dram_tensor("wvc", [C, 128], BF16, kind="ExternalInput").ap()
    io["wbn1T"] = nc.dram_tensor("wbn1T", [C, C], F32, kind="ExternalInput").ap()
    io["id64"] = nc.dram_tensor("id64", [C, C], BF16, kind="ExternalInput").ap()
    io["out"] = nc.dram_tensor("out", [C, HW], F32, kind="ExternalOutput").ap()

    with tile.TileContext(nc) as tc:
        with ExitStack() as ctx:
            _build_kernel(ctx, tc, io)
    nc.compile()
    return nc


def make_in_maps(x, w_cam, w_q, w_k, w_v, w_bn):
    import ml_dtypes

    f = lambda a: np.ascontiguousarray(np.asarray(a, dtype=np.float32))
    fb = lambda a: np.ascontiguousarray(
        np.asarray(a, dtype=np.float32).astype(ml_dtypes.bfloat16)
    )
    w_bn = np.asarray(w_bn, dtype=np.float64)
    w_vp = w_bn[:, C:] @ np.asarray(w_v, dtype=np.float64)  # wbn2 folded into v
    base = {
        "wq4T": fb(np.concatenate([np.asarray(w_q).T] * 4, axis=1)),
        "wk4T": fb(np.concatenate([np.asarray(w_k).T] * 4, axis=1)),
        "wvc": fb(np.concatenate([w_vp.T, np.asarray(w_cam).T], axis=1)),
        "wbn1T": f(w_bn[:, :C].T),
        "id64": fb(np.eye(C)),
    }
    x = np.asarray(x)
    return [dict(base, x=f(x[b].reshape(C, HW))) for b in range(8)]


_NC_CACHE = None


def kernel(x, w_cam, w_q, w_k, w_v, w_bn):
    global _NC_CACHE
    if _NC_CACHE is None:
        _NC_CACHE = build_nc()
    nc = _NC_CACHE
    in_maps = make_in_maps(x, w_cam, w_q, w_k, w_v, w_bn)
    res = run_bass_kernel_spmd(nc, in_maps, list(range(8)))
    out = np.stack([res.results[b]["out"].reshape(C, 64, 64) for b in range(8)])
    return out.astype(np.float32)


# revision 19
# speedup vs baseline: 1.2213x; 1.0197x over previous
"""Trainium2 Bass kernel for dual-attention block (CAM + SAM + bottleneck).

Contract: kernel(**inputs) takes FULL unsharded inputs
  x     [8, 64, 64, 64] f32
  w_cam [64, 64], w_q [32, 64], w_k [32, 64], w_v [64, 64], w_bn [64, 128]
and returns the full [8, 64, 64, 64] f32 output.

Sharding: data-parallel over batch across 8 NeuronCores (1 image each);
weights replicated. Per-core math (c=64 channels, n=m=4096 spatial):

  CAM: xcT = x.T @ w_cam.T ; Ec = xcT.T @ xcT;
       attn_c = softmax_rows(Ec); bn = ((wbn1 @ attn_c) + I) @ x
       (the +I folds the residual x into the CAM bottleneck matmul)
  SAM: q4/k4 = (w stacked 4x) @ x  -> q,k replicated on 4 partition groups
       S[m,n] = sum_c k[c,m] q[c,n]  (row-tiled K=32 matmuls, 4-concurrent)
       E = exp(S - ln64) in fp8-e4m3  (max|S|=9.05 -> E'max 133 < 240;
           the 1/64 cancels between numerator and denominator)
       acc[c,n] = sum_m W[m,c] E[m,n]  with W = [v'.T | ones] in fp8 and
                  v' = (wbn2 @ w_v) x  (bottleneck conv folded into the
                  value weights on the host), one DoubleRow matmul per
                  m-tile PAIR (K=256) -> rows 0..63 = wbn2-projected SAM
                  contribution (unnormalized), row 64 = Z
  out = bn + acc[0:64] * (1/Z)
        (1/Z via custom-DVE fast reciprocal at partition 0, broadcast to
        64 partitions by GpSimd partition_broadcast)

v8 structure (v4 measured 221us, v7 206us):
  - spool: ONE 3-slot rotation (3 x 2 PSUM banks) for the S tiles.  With
    A/B ping-pong a pair's second S group had to wait for the previous
    exp, serializing the quadrant matmuls 2+2; with 3 slots a group's
    bank is free 3 exp-periods ahead, so all 4 K=32 quadrant matmuls of
    a pair issue back-to-back and run concurrently on disjoint row
    quadrants.  All other PSUM scratch (warm-up, q/k chunk production,
    wvc, bn, m1ps) rides the same rotation; vacc/EC keep 2 banks.
  - wbn2 folded into the DR weights host-side; the per-block bottleneck
    matmul on the SAM path is gone.  The residual +x is folded into the
    CAM bottleneck matmul as (M1+I) via a device identity add.
  - 1/Z: fast approx reciprocal (partition 0, via a 2KB DMA hop) and
    GpSimd partition_broadcast instead of a K=1 PE matmul.
  - Preamble: weight DMAs first on the GpSimd queue; x in 8 chunks over
    3 DMA queues; ~6us dense PE warm-up (HAM -> 2.4GHz) overlapping the
    x DMA; q/k chunks 0-1 as single FD=1024 matmuls right behind it.
PSUM: spool 3x2 + vacc/EC 2 = 8 banks.
"""

import sys
from contextlib import ExitStack

import numpy as np

if "/opt/trn_rl_repo" not in sys.path:
    sys.path.insert(0, "/opt/trn_rl_repo")

import concourse.bass as bass
import concourse.tile as tile
from concourse import bacc, mybir
from concourse.bass_utils import run_bass_kernel_spmd

F32 = mybir.dt.float32
BF16 = mybir.dt.bfloat16
FP8 = mybir.dt.float8e4

C = 64          # channels
HW = 4096       # 64*64 spatial
NB = 8          # number of 512-wide n blocks
BLK = 512
MT = 32         # m tiles of 128
NG = 16         # groups of 2 m-tiles per n-block
WP = 80         # wt8 per-m-tile stride (65 used; 80 for DoubleRow step%16==0)
NLOG64 = -4.1588830833596715

Exp = mybir.ActivationFunctionType.Exp
DR = mybir.MatmulPerfMode.DoubleRow


def _build_kernel(ctx: ExitStack, tc: tile.TileContext, io: dict):
    nc = tc.nc
    x_d = io["x"]
    out_d = io["out"]

    consts = ctx.enter_context(tc.tile_pool(name="consts", bufs=1))
    bigs = ctx.enter_context(tc.tile_pool(name="bigs", bufs=1))
    epool = ctx.enter_context(tc.tile_pool(name="epool", bufs=3))
    campool = ctx.enter_context(tc.tile_pool(name="campool", bufs=1))
    sampool = ctx.enter_context(tc.tile_pool(name="sampool", bufs=4))
    spool = ctx.enter_context(
        tc.tile_pool(name="spool", bufs=3, space=bass.MemorySpace.PSUM)
    )
    vpool = ctx.enter_context(
        tc.tile_pool(name="vpool", bufs=2, space=bass.MemorySpace.PSUM)
    )

    # ---- weight DMAs first, on the otherwise-idle GpSimd queue (tiny; if
    # they queued behind the 1MB x transfer the first matmul waits ~15us) --
    wq4T = consts.tile([C, 128], BF16)    # (w_q stacked 4x).T
    wk4T = consts.tile([C, 128], BF16)
    wvc = consts.tile([C, 128], BF16)     # [(wbn2 w_v).T | w_cam.T]
    wbn1T = consts.tile([C, C], F32)
    id64 = consts.tile([C, C], BF16)
    zb = consts.tile([128, 1], F32)
    nlog64 = consts.tile([128, 1], F32)   # exp bias: E'=E/64 fits fp8e4 max 240
    dummy = consts.tile([128, 1], F32)

    nc.sync.dma_start(wk4T[:], io["wk4T"][:])
    nc.scalar.dma_start(wq4T[:], io["wq4T"][:])
    nc.gpsimd.dma_start(wvc[:], io["wvc"][:])
    nc.gpsimd.dma_start(wbn1T[:], io["wbn1T"][:])
    nc.gpsimd.dma_start(id64[:], io["id64"][:])

    # ---- x DMA: 8 column chunks round-robin over 3 HWDGE queues (each
    # queue sustains only ~100 GB/s; chunk 0 -- all that the first S
    # matmuls need -- lands first, right behind wk4T/wq4T) ----
    x_sb = bigs.tile([C, HW], F32)
    x_qs = [nc.sync, nc.scalar, nc.gpsimd]
    for xc_ in range(8):
        x_qs[xc_ % 3].dma_start(
            x_sb[:, xc_ * BLK : (xc_ + 1) * BLK],
            x_d[:, xc_ * BLK : (xc_ + 1) * BLK],
        )

    nc.vector.memset(zb[:], 0.0)
    # Trigger the exp ACT-table load right behind the x-DMA issue (overlaps
    # the transfer) instead of in front of the first real exp.
    nc.scalar.activation(dummy[:], zb[:], Exp, bias=zb[:])
    nc.vector.memset(nlog64[:], NLOG64)

    q4 = bigs.tile([128, HW], BF16)
    k4 = bigs.tile([128, HW], BF16)
    wt8 = bigs.tile([128, MT * WP], FP8)   # per m-tile [v'T | ones | pad]
    xct = bigs.tile([128, MT * C], BF16)   # xcT, m-tile-major
    x_bf = bigs.tile([C, HW], BF16)
    e0 = bigs.tile([128, NG * 2 * BLK], FP8)  # block-0 E, consumed in block 1

    # ones column of wt8 (wvc copies below only write cols 0..63)
    nc.vector.memset(
        wt8[:].rearrange("p (t c) -> p t c", c=WP)[:, :, 64:65], 1.0
    )

    # x in bf16 feeds the q4/k4/wvc/bn matmuls at full PE rate; 8 chunks so
    # the first q/k matmuls start as soon as the first x columns land.
    for xc_ in range(8):
        nc.vector.tensor_copy(
            x_bf[:, xc_ * BLK : (xc_ + 1) * BLK], x_sb[:, xc_ * BLK : (xc_ + 1) * BLK]
        )

    # ---- q4 / k4: replicated q,k via stacked-weight 1x1 convs.  Each
    # 2-chunk group is ONE FD=1024 matmul.  Chunks 0-1 of k and q are
    # produced up front; the rest are fill-in groups inside the block
    # loop, each 2+ pairs ahead of its consumption deadline. ----
    def qk_group(which, cch, nch=1, on_scalar=False, pool=None):
        wT, dst = (wk4T, k4) if which == "k" else (wq4T, q4)
        if pool is None:
            ps = spool.tile([128, nch * BLK], F32, tag="s", name="qkps")
        else:
            ps = pool.tile([128, nch * BLK], F32, tag="v", name="qkps")
        for i in range(nch):
            nc.tensor.matmul(
                ps[:, i * BLK : (i + 1) * BLK],
                wT[:],
                x_bf[:, (cch + i) * BLK : (cch + i + 1) * BLK],
                start=True,
                stop=True,
            )
        lo = cch * BLK
        if on_scalar:
            nc.scalar.copy(dst[:, lo : lo + nch * BLK], ps[:])
        else:
            nc.vector.tensor_copy(dst[:, lo : lo + nch * BLK], ps[:])

    qk_group("k", 0)
    qk_group("q", 0, on_scalar=True)

    # (block, pair) -> (which, chunk); deadlines: k chunk c is consumed
    # at block-0 pair c; q chunk c at block c.
    qk_fill = {
        (0, 0): [("k", 1), ("k", 2)], (0, 1): [("k", 3)],
        (0, 2): [("k", 4)], (0, 3): [("k", 5)], (0, 4): [("k", 6)],
        (0, 5): [("k", 7)], (0, 6): [("q", 1)],
        (1, 5): [("q", 2)], (2, 1): [("q", 3)], (2, 5): [("q", 4)],
        (3, 1): [("q", 5)], (3, 5): [("q", 6)], (4, 5): [("q", 7)],
    }

    state = {}  # EC tile, allocated at block 1 start (vpool slot timing)

    def wvc_group(base, size):
        """xcT and WT (=[v'T|ones]) production for one m-tile group."""
        ps_w = vpool.tile([128, BLK], F32, tag="v", name="wvcps")
        for j in range(size):
            m = base + j
            nc.tensor.matmul(
                ps_w[:, j * 128 : (j + 1) * 128],
                x_bf[:, m * 128 : (m + 1) * 128],
                wvc[:],
                start=True,
                stop=True,
            )
        src = ps_w[:, : size * 128].rearrange("p (j c) -> p j c", c=128)
        wt_dst = wt8[:, base * WP : (base + size) * WP].rearrange(
            "p (j c) -> p j c", c=WP
        )
        with nc.allow_low_precision(reason="v' in fp8 for DoubleRow acc"):
            nc.vector.tensor_copy(wt_dst[:, :, 0:C], src[:, :, 0:C])
        xct_dst = xct[:, base * C : (base + size) * C].rearrange(
            "p (j c) -> p j c", c=C
        )
        with nc.allow_low_precision(reason="xcT in bf16 for cheap ec matmuls"):
            nc.vector.tensor_copy(xct_dst, src[:, :, C : 2 * C])

    def ec_group(base, size):
        EC = state["EC"]
        for j in range(size):
            m = base + j
            nc.tensor.matmul(
                EC[0:C, 0:C],
                xct[:, m * C : (m + 1) * C],
                xct[:, m * C : (m + 1) * C],
                start=(m == 0),
                stop=(m == MT - 1),
            )

    # ---- per-block state for split epilogues ----
    vaccs = [None] * NB
    sam = [None] * NB   # sam65 [65, BLK] f32: rows 0..63 unnorm SAM out, 64 = Z
    rzs = [None] * NB   # rz [1, BLK] bf16 at partition 0
    M1T_sb = campool.tile([C, C], BF16)

    def epilogue_a(nb):
        """At block end: evacuate vacc (recip is emitted separately)."""
        aux = sampool.tile([C + 1, BLK], F32, tag="aux", name="aux")
        nc.vector.tensor_copy(aux[:], vaccs[nb][0 : C + 1, :])
        sam[nb] = aux

    def emit_recip(nb):
        """1/Z for block nb via the fast approx recip + bf16 cast.

        The custom DVE op only works at base partition 0 (and DVE lanes
        cannot move data across partitions), so the Z row is first moved
        from partition 64 to partition 0 by a tiny SBUF->SBUF DMA on the
        otherwise-idle sync queue.
        """
        z0 = sampool.tile([1, BLK], F32, tag="z0", name="z0")
        nc.sync.dma_start(z0[:], sam[nb][C : C + 1, :])
        rz32 = sampool.tile([1, BLK], F32, tag="rz32", name="rz32")
        nc.vector.reciprocal_approx_fast(rz32[:], z0[:])
        rzb = sampool.tile([1, BLK], BF16, tag="rz", name="rzb")
        with nc.allow_low_precision(reason="1/Z in bf16: 0.4% on the SAM term"):
            nc.vector.tensor_copy(rzb[:], rz32[:])
        rzs[nb] = rzb

    def epilogue_b1(nb):
        """Broadcast 1/Z to 64 partitions (GpSimd) and scale the SAM rows."""
        bcast = sampool.tile([C, BLK], BF16, tag="bc", name="bcast")
        nc.gpsimd.partition_broadcast(bcast[:], rzs[nb][:])
        sam_sc = sampool.tile([C, BLK], F32, tag="sc", name="sam_sc")
        nc.vector.tensor_mul(sam_sc[:], sam[nb][0:C, :], bcast[:])
        return sam_sc

    def epilogue_b2(nb, sam_sc):
        """CAM bottleneck (+residual via I) matmul, add SAM term, DMA out."""
        ncol = slice(nb * BLK, (nb + 1) * BLK)
        bn = spool.tile([128, BLK], F32, tag="s", name="bn")
        nc.tensor.matmul(
            bn[0:C, :], M1T_sb[:], x_bf[:, ncol], start=True, stop=True
        )
        o_t = sampool.tile([C, BLK], F32, tag="ot", name="o_t")
        nc.vector.tensor_add(o_t[:], bn[0:C, :], sam_sc[:])
        nc.sync.dma_start(out_d[:, ncol], o_t[:])

    def cam_chain():
        """CAM softmax -> attn_c -> M1T = (wbn1 @ attn_c).T + I"""
        EC = state["EC"]
        negmax = campool.tile([C, 1], F32)
        nc.vector.reduce_max(
            negmax[:], EC[0:C, 0:C], axis=mybir.AxisListType.X, negate=True
        )
        exp_c = campool.tile([C, C], F32)
        nc.scalar.activation(exp_c[:], EC[0:C, 0:C], Exp, bias=negmax[:])
        sum_c = campool.tile([C, 1], F32)
        nc.vector.reduce_sum(sum_c[:], exp_c[:], axis=mybir.AxisListType.X)
        rec_c = campool.tile([C, 1], F32)
        nc.vector.reciprocal(rec_c[:], sum_c[:])
        attn_c = campool.tile([C, C], F32)
        nc.vector.tensor_scalar_mul(attn_c[:], exp_c[:], rec_c[:])
        m1ps = spool.tile([128, BLK], F32, tag="s", name="m1ps")
        nc.tensor.matmul(
            m1ps[0:C, 0:C], attn_c[:], wbn1T[:], start=True, stop=True
        )
        with nc.allow_low_precision(reason="M1T in bf16 feeds a bf16 matmul"):
            nc.vector.tensor_add(M1T_sb[:], m1ps[0:C, 0:C], id64[:])

    # ---- main SAM loop over 8 n-blocks, groups emitted in PAIRS ----
    # Block 0 carries wvc + k-chunk fill-ins instead of its DR matmuls
    # (which would blow its PE budget); its E tiles persist in e0 and the
    # 16 deferred DRs ride block 1's slack.  ec runs in blocks 2-3 and the
    # CAM chain at block 4; the vacc/EC vpool rotation is:
    #   vacc0(A) vacc1(B) | vacc2(A) EC(B) | vacc3(A) | CAM, vacc4(B) |
    #   vacc5(A) | vacc6(B) | vacc7(A)
    # each claim one aux-evacuation behind its slot's previous tenant.
    sc_pend = {}
    recip_sched = {(2, 0): 0, (2, 4): 1, (3, 0): 2, (4, 0): 3,
                   (5, 0): 4, (6, 0): 5, (7, 0): 6}
    b1_sched = {(4, 1): 0, (4, 5): 1, (5, 1): 2, (5, 5): 3,
                (6, 1): 4, (6, 5): 5, (7, 2): 6}
    b2_sched = {(4, 3): 0, (4, 7): 1, (5, 3): 2, (5, 7): 3,
                (6, 3): 4, (6, 7): 5, (7, 4): 6}
    for nb in range(NB):
        if nb == 1:
            vaccs[0] = vpool.tile([128, BLK], F32, tag="v", name="vacc0")
        if nb == 4:
            cam_chain()
        if nb != 0:
            vacc = vpool.tile([128, BLK], F32, tag="v", name="vacc")
            vaccs[nb] = vacc
        if nb == 2:
            # EC right after vacc2: slot B, re-claimed by vacc4 after the
            # CAM chain at block 4 has consumed EC.
            state["EC"] = vpool.tile([128, BLK], F32, tag="v", name="EC")
        ncol = slice(nb * BLK, (nb + 1) * BLK)
        for p in range(NG // 2):
            g0, g1 = 2 * p, 2 * p + 1
            s_ts = []
            for g in (g0, g1):
                s_t = spool.tile([128, 2 * BLK], F32, tag="s", name="s_t")
                s_ts.append(s_t)
                for j in range(2):
                    m = 2 * g + j
                    r = 2 * (g % 2) + j  # row quadrants 0,1 / 2,3
                    nc.tensor.matmul(
                        s_t[:, j * BLK : (j + 1) * BLK],
                        k4[32 * r : 32 * r + 32, m * 128 : (m + 1) * 128],
                        q4[32 * r : 32 * r + 32, ncol],
                        start=True,
                        stop=True,
                        tile_position=(32 * r, 0),
                    )
            if nb == 0:
                wvc_group(2 * g0, 2)
                wvc_group(2 * g1, 2)
            if nb in (2, 3):
                ec_group((nb - 2) * NG + 2 * p, 2)
            for fill in qk_fill.get((nb, p), ()):
                qk_group(*fill, pool=vpool if nb == 0 else None)
            e_ts = []
            for g, s_t in zip((g0, g1), s_ts):
                if nb == 0:
                    e_t = e0[:, 2 * g * BLK : 2 * (g + 1) * BLK]
                else:
                    e_t = epool.tile([128, 2 * BLK], FP8, tag="e", name="e_t")
                e_ts.append(e_t)
                with nc.allow_low_precision(reason="E in fp8: ~1e-4 on out"):
                    nc.scalar.activation(e_t[:], s_t[:], Exp, bias=nlog64[:])
            if nb != 0:
                for g, e_t in zip((g0, g1), e_ts):
                    lhsT = wt8[:, 2 * g * WP : (2 * g + 2) * WP].rearrange(
                        "p (two f) -> p two f", two=2
                    )[:, :, 0:65]
                    rhs = e_t[:].rearrange("p (two f) -> p two f", two=2)
                    nc.tensor.matmul(
                        vaccs[nb][0 : C + 1, :],
                        lhsT,
                        rhs,
                        start=(g == 0),
                        stop=(g == NG - 1),
                        perf_mode=DR,
                    )
            if nb == 1:
                # block 0's deferred DR matmuls, 2 per pair
                for g in (g0, g1):
                    lhsT = wt8[:, 2 * g * WP : (2 * g + 2) * WP].rearrange(
                        "p (two f) -> p two f", two=2
                    )[:, :, 0:65]
                    rhs = e0[:, 2 * g * BLK : 2 * (g + 1) * BLK].rearrange(
                        "p (two f) -> p two f", two=2
                    )
                    nc.tensor.matmul(
                        vaccs[0][0 : C + 1, :],
                        lhsT,
                        rhs,
                        start=(g == 0),
                        stop=(g == NG - 1),
                        perf_mode=DR,
                    )
            if (nb, p) in recip_sched:
                emit_recip(recip_sched[(nb, p)])
            if (nb, p) in b1_sched:
                s = b1_sched[(nb, p)]
                sc_pend[s] = epilogue_b1(s)
            if (nb, p) in b2_sched:
                s = b2_sched[(nb, p)]
                epilogue_b2(s, sc_pend.pop(s))

        if nb == 1:
            epilogue_a(0)
            epilogue_a(1)
        elif nb != 0:
            epilogue_a(nb)
    # ---- tail: only the last block's epilogue chain remains ----
    emit_recip(NB - 1)
    sc7 = epilogue_b1(NB - 1)
    epilogue_b2(NB - 1, sc7)


def build_nc():
    nc = bacc.Bacc(
        "TRN2",
        target_bir_lowering=False,
        debug=False,
        enable_asserts=False,
        num_devices=8,
    )
    io = {}
    io["x"] = nc.dram_tensor("x", [C, HW], F32, kind="ExternalInput").ap()
    io["wq4T"] = nc.dram_tensor("wq4T", [C, 128], BF16, kind="ExternalInput").ap()
    io["wk4T"] = nc.dram_tensor("wk4T", [C, 128], BF16, kind="ExternalInput").ap()
    io["wvc"] = nc.dram_tensor("wvc", [C, 128], BF16, kind="ExternalInput").ap()
    io["wbn1T"] = nc.dram_tensor("wbn1T", [C, C], F32, kind="ExternalInput").ap()
    io["id64"] = nc.dram_tensor("id64", [C, C], BF16, kind="ExternalInput").ap()
    io["out"] = nc.dram_tensor("out", [C, HW], F32, kind="ExternalOutput").ap()

    with tile.TileContext(nc) as tc:
        with ExitStack() as ctx:
            _build_kernel(ctx, tc, io)
    nc.compile()
    return nc


def make_in_maps(x, w_cam, w_q, w_k, w_v, w_bn):
    import ml_dtypes

    f = lambda a: np.ascontiguousarray(np.asarray(a, dtype=np.float32))
    fb = lambda a: np.ascontiguousarray(
        np.asarray(a, dtype=np.float32).astype(ml_dtypes.bfloat16)
    )
    w_bn = np.asarray(w_bn, dtype=np.float64)
    w_vp = w_bn[:, C:] @ np.asarray(w_v, dtype=np.float64)  # wbn2 folded into v
    base = {
        "wq4T": fb(np.concatenate([np.asarray(w_q).T] * 4, axis=1)),
        "wk4T": fb(np.concatenate([np.asarray(w_k).T] * 4, axis=1)),
        "wvc": fb(np.concatenate([w_vp.T, np.asarray(w_cam).T], axis=1)),
        "wbn1T": f(w_bn[:, :C].T),
        "id64": fb(np.eye(C)),
    }
    x = np.asarray(x)
    return [dict(base, x=f(x[b].reshape(C, HW))) for b in range(8)]


_NC_CACHE = None


def kernel(x, w_cam, w_q, w_k, w_v, w_bn):
    global _NC_CACHE
    if _NC_CACHE is None:
        _NC_CACHE = build_nc()
    nc = _NC_CACHE
    in_maps = make_in_maps(x, w_cam, w_q, w_k, w_v, w_bn)
    res = run_bass_kernel_spmd(nc, in_maps, list(range(8)))
    out = np.stack([res.results[b]["out"].reshape(C, 64, 64) for b in range(8)])
    return out.astype(np.float32)


# revision 20
# speedup vs baseline: 1.2229x; 1.0013x over previous
"""Trainium2 Bass kernel for dual-attention block (CAM + SAM + bottleneck).

Contract: kernel(**inputs) takes FULL unsharded inputs
  x     [8, 64, 64, 64] f32
  w_cam [64, 64], w_q [32, 64], w_k [32, 64], w_v [64, 64], w_bn [64, 128]
and returns the full [8, 64, 64, 64] f32 output.

Sharding: data-parallel over batch across 8 NeuronCores (1 image each);
weights replicated. Per-core math (c=64 channels, n=m=4096 spatial):

  CAM: xcT = x.T @ w_cam.T ; Ec = xcT.T @ xcT;
       attn_c = softmax_rows(Ec); bn = ((wbn1 @ attn_c) + I) @ x
       (the +I folds the residual x into the CAM bottleneck matmul)
  SAM: q4/k4 = (w stacked 4x) @ x  -> q,k replicated on 4 partition groups
       S[m,n] = sum_c k[c,m] q[c,n]  (row-tiled K=32 matmuls, 4-concurrent)
       E = exp(S - ln64) in fp8-e4m3  (max|S|=9.05 -> E'max 133 < 240;
           the 1/64 cancels between numerator and denominator)
       acc[c,n] = sum_m W[m,c] E[m,n]  with W = [v'.T | ones] in fp8 and
                  v' = (wbn2 @ w_v) x  (bottleneck conv folded into the
                  value weights on the host), one DoubleRow matmul per
                  m-tile PAIR (K=256) -> rows 0..63 = wbn2-projected SAM
                  contribution (unnormalized), row 64 = Z
  out = bn + acc[0:64] * (1/Z)
        (1/Z via custom-DVE fast reciprocal at partition 0, broadcast to
        64 partitions by GpSimd partition_broadcast)

v8 structure (v4 measured 221us, v7 206us):
  - spool: ONE 3-slot rotation (3 x 2 PSUM banks) for the S tiles.  With
    A/B ping-pong a pair's second S group had to wait for the previous
    exp, serializing the quadrant matmuls 2+2; with 3 slots a group's
    bank is free 3 exp-periods ahead, so all 4 K=32 quadrant matmuls of
    a pair issue back-to-back and run concurrently on disjoint row
    quadrants.  All other PSUM scratch (warm-up, q/k chunk production,
    wvc, bn, m1ps) rides the same rotation; vacc/EC keep 2 banks.
  - wbn2 folded into the DR weights host-side; the per-block bottleneck
    matmul on the SAM path is gone.  The residual +x is folded into the
    CAM bottleneck matmul as (M1+I) via a device identity add.
  - 1/Z: fast approx reciprocal (partition 0, via a 2KB DMA hop) and
    GpSimd partition_broadcast instead of a K=1 PE matmul.
  - Preamble: weight DMAs first on the GpSimd queue; x in 8 chunks over
    3 DMA queues; ~6us dense PE warm-up (HAM -> 2.4GHz) overlapping the
    x DMA; q/k chunks 0-1 as single FD=1024 matmuls right behind it.
PSUM: spool 3x2 + vacc/EC 2 = 8 banks.
"""

import sys
from contextlib import ExitStack

import numpy as np

if "/opt/trn_rl_repo" not in sys.path:
    sys.path.insert(0, "/opt/trn_rl_repo")

import concourse.bass as bass
import concourse.tile as tile
from concourse import bacc, mybir
from concourse.bass_utils import run_bass_kernel_spmd

F32 = mybir.dt.float32
BF16 = mybir.dt.bfloat16
FP8 = mybir.dt.float8e4

C = 64          # channels
HW = 4096       # 64*64 spatial
NB = 8          # number of 512-wide n blocks
BLK = 512
MT = 32         # m tiles of 128
NG = 16         # groups of 2 m-tiles per n-block
WP = 80         # wt8 per-m-tile stride (65 used; 80 for DoubleRow step%16==0)
NLOG64 = -4.1588830833596715

Exp = mybir.ActivationFunctionType.Exp
DR = mybir.MatmulPerfMode.DoubleRow


def _build_kernel(ctx: ExitStack, tc: tile.TileContext, io: dict):
    nc = tc.nc
    x_d = io["x"]
    out_d = io["out"]

    consts = ctx.enter_context(tc.tile_pool(name="consts", bufs=1))
    bigs = ctx.enter_context(tc.tile_pool(name="bigs", bufs=1))
    epool = ctx.enter_context(tc.tile_pool(name="epool", bufs=3))
    campool = ctx.enter_context(tc.tile_pool(name="campool", bufs=1))
    sampool = ctx.enter_context(tc.tile_pool(name="sampool", bufs=4))
    spool = ctx.enter_context(
        tc.tile_pool(name="spool", bufs=3, space=bass.MemorySpace.PSUM)
    )
    vpool = ctx.enter_context(
        tc.tile_pool(name="vpool", bufs=2, space=bass.MemorySpace.PSUM)
    )

    # ---- weight DMAs first, on the otherwise-idle GpSimd queue (tiny; if
    # they queued behind the 1MB x transfer the first matmul waits ~15us) --
    wq4T = consts.tile([C, 128], BF16)    # (w_q stacked 4x).T
    wk4T = consts.tile([C, 128], BF16)
    wvc = consts.tile([C, 128], BF16)     # [(wbn2 w_v).T | w_cam.T]
    wbn1T = consts.tile([C, C], F32)
    id64 = consts.tile([C, C], BF16)
    zb = consts.tile([128, 1], F32)
    nlog64 = consts.tile([128, 1], F32)   # exp bias: E'=E/64 fits fp8e4 max 240
    dummy = consts.tile([128, 1], F32)

    nc.sync.dma_start(wk4T[:], io["wk4T"][:])
    nc.scalar.dma_start(wq4T[:], io["wq4T"][:])
    nc.gpsimd.dma_start(wvc[:], io["wvc"][:])
    nc.gpsimd.dma_start(wbn1T[:], io["wbn1T"][:])
    nc.gpsimd.dma_start(id64[:], io["id64"][:])

    # ---- x DMA: 8 column chunks round-robin over 3 HWDGE queues (each
    # queue sustains only ~100 GB/s; chunk 0 -- all that the first S
    # matmuls need -- lands first, right behind wk4T/wq4T) ----
    x_sb = bigs.tile([C, HW], F32)
    x_qs = [nc.sync, nc.scalar, nc.gpsimd]
    for xc_ in range(8):
        x_qs[xc_ % 3].dma_start(
            x_sb[:, xc_ * BLK : (xc_ + 1) * BLK],
            x_d[:, xc_ * BLK : (xc_ + 1) * BLK],
        )

    nc.vector.memset(zb[:], 0.0)
    # Trigger the exp ACT-table load right behind the x-DMA issue (overlaps
    # the transfer) instead of in front of the first real exp.
    nc.scalar.activation(dummy[:], zb[:], Exp, bias=zb[:])
    nc.vector.memset(nlog64[:], NLOG64)

    q4 = bigs.tile([128, HW], BF16)
    k4 = bigs.tile([128, HW], BF16)
    wt8 = bigs.tile([128, MT * WP], FP8)   # per m-tile [v'T | ones | pad]
    xct = bigs.tile([128, MT * C], BF16)   # xcT, m-tile-major
    x_bf = bigs.tile([C, HW], BF16)
    e0 = bigs.tile([128, NG * 2 * BLK], FP8)  # block-0 E, consumed in block 1

    # ones column of wt8 (wvc copies below only write cols 0..63)
    nc.vector.memset(
        wt8[:].rearrange("p (t c) -> p t c", c=WP)[:, :, 64:65], 1.0
    )

    # x in bf16 feeds the q4/k4/wvc/bn matmuls at full PE rate.  Only
    # chunks 0-1 are cast up front: casts for chunks that arrive later are
    # emitted inside the loop so they cannot head-of-line-block the DVE
    # queue in front of the k0/q0 evacuations.
    def x_cast(xc_):
        nc.vector.tensor_copy(
            x_bf[:, xc_ * BLK : (xc_ + 1) * BLK], x_sb[:, xc_ * BLK : (xc_ + 1) * BLK]
        )

    x_cast(0)
    x_cast(1)
    x_cast_sched = {(0, 0): 2, (0, 1): 3, (0, 2): 4, (0, 3): 5, (0, 4): 6,
                    (0, 5): 7}

    # ---- q4 / k4: replicated q,k via stacked-weight 1x1 convs.  Each
    # 2-chunk group is ONE FD=1024 matmul.  Chunks 0-1 of k and q are
    # produced up front; the rest are fill-in groups inside the block
    # loop, each 2+ pairs ahead of its consumption deadline. ----
    def qk_group(which, cch, nch=1, on_scalar=False, pool=None):
        wT, dst = (wk4T, k4) if which == "k" else (wq4T, q4)
        if pool is None:
            ps = spool.tile([128, nch * BLK], F32, tag="s", name="qkps")
        else:
            ps = pool.tile([128, nch * BLK], F32, tag="v", name="qkps")
        for i in range(nch):
            nc.tensor.matmul(
                ps[:, i * BLK : (i + 1) * BLK],
                wT[:],
                x_bf[:, (cch + i) * BLK : (cch + i + 1) * BLK],
                start=True,
                stop=True,
            )
        lo = cch * BLK
        if on_scalar:
            nc.scalar.copy(dst[:, lo : lo + nch * BLK], ps[:])
        else:
            nc.vector.tensor_copy(dst[:, lo : lo + nch * BLK], ps[:])

    qk_group("k", 0)
    qk_group("q", 0, on_scalar=True)

    # (block, pair) -> (which, chunk); deadlines: k chunk c is consumed
    # at block-0 pair c; q chunk c at block c.
    qk_fill = {
        (0, 0): [("k", 1), ("k", 2)], (0, 1): [("k", 3)],
        (0, 2): [("k", 4)], (0, 3): [("k", 5)], (0, 4): [("k", 6)],
        (0, 5): [("k", 7)], (0, 6): [("q", 1)],
        (1, 5): [("q", 2)], (2, 1): [("q", 3)], (2, 5): [("q", 4)],
        (3, 1): [("q", 5)], (3, 5): [("q", 6)], (4, 5): [("q", 7)],
    }

    state = {}  # EC tile, allocated at block 1 start (vpool slot timing)

    def wvc_group(base, size):
        """xcT and WT (=[v'T|ones]) production for one m-tile group."""
        ps_w = vpool.tile([128, BLK], F32, tag="v", name="wvcps")
        for j in range(size):
            m = base + j
            nc.tensor.matmul(
                ps_w[:, j * 128 : (j + 1) * 128],
                x_bf[:, m * 128 : (m + 1) * 128],
                wvc[:],
                start=True,
                stop=True,
            )
        src = ps_w[:, : size * 128].rearrange("p (j c) -> p j c", c=128)
        wt_dst = wt8[:, base * WP : (base + size) * WP].rearrange(
            "p (j c) -> p j c", c=WP
        )
        with nc.allow_low_precision(reason="v' in fp8 for DoubleRow acc"):
            nc.vector.tensor_copy(wt_dst[:, :, 0:C], src[:, :, 0:C])
        xct_dst = xct[:, base * C : (base + size) * C].rearrange(
            "p (j c) -> p j c", c=C
        )
        with nc.allow_low_precision(reason="xcT in bf16 for cheap ec matmuls"):
            nc.vector.tensor_copy(xct_dst, src[:, :, C : 2 * C])

    def ec_group(base, size):
        EC = state["EC"]
        for j in range(size):
            m = base + j
            nc.tensor.matmul(
                EC[0:C, 0:C],
                xct[:, m * C : (m + 1) * C],
                xct[:, m * C : (m + 1) * C],
                start=(m == 0),
                stop=(m == MT - 1),
            )

    # ---- per-block state for split epilogues ----
    vaccs = [None] * NB
    sam = [None] * NB   # sam65 [65, BLK] f32: rows 0..63 unnorm SAM out, 64 = Z
    rzs = [None] * NB   # rz [1, BLK] bf16 at partition 0
    M1T_sb = campool.tile([C, C], BF16)

    def epilogue_a(nb):
        """At block end: evacuate vacc (recip is emitted separately)."""
        aux = sampool.tile([C + 1, BLK], F32, tag="aux", name="aux")
        nc.vector.tensor_copy(aux[:], vaccs[nb][0 : C + 1, :])
        sam[nb] = aux

    def emit_recip(nb):
        """1/Z for block nb via the fast approx recip + bf16 cast.

        The custom DVE op only works at base partition 0 (and DVE lanes
        cannot move data across partitions), so the Z row is first moved
        from partition 64 to partition 0 by a tiny SBUF->SBUF DMA on the
        otherwise-idle sync queue.
        """
        z0 = sampool.tile([1, BLK], F32, tag="z0", name="z0")
        nc.sync.dma_start(z0[:], sam[nb][C : C + 1, :])
        rz32 = sampool.tile([1, BLK], F32, tag="rz32", name="rz32")
        nc.vector.reciprocal_approx_fast(rz32[:], z0[:])
        rzb = sampool.tile([1, BLK], BF16, tag="rz", name="rzb")
        with nc.allow_low_precision(reason="1/Z in bf16: 0.4% on the SAM term"):
            nc.vector.tensor_copy(rzb[:], rz32[:])
        rzs[nb] = rzb

    def epilogue_b1(nb):
        """Broadcast 1/Z to 64 partitions (GpSimd) and scale the SAM rows."""
        bcast = sampool.tile([C, BLK], BF16, tag="bc", name="bcast")
        nc.gpsimd.partition_broadcast(bcast[:], rzs[nb][:])
        sam_sc = sampool.tile([C, BLK], F32, tag="sc", name="sam_sc")
        nc.vector.tensor_mul(sam_sc[:], sam[nb][0:C, :], bcast[:])
        return sam_sc

    def epilogue_b2(nb, sam_sc):
        """CAM bottleneck (+residual via I) matmul, add SAM term, DMA out."""
        ncol = slice(nb * BLK, (nb + 1) * BLK)
        bn = spool.tile([128, BLK], F32, tag="s", name="bn")
        nc.tensor.matmul(
            bn[0:C, :], M1T_sb[:], x_bf[:, ncol], start=True, stop=True
        )
        o_t = sampool.tile([C, BLK], F32, tag="ot", name="o_t")
        nc.vector.tensor_add(o_t[:], bn[0:C, :], sam_sc[:])
        nc.sync.dma_start(out_d[:, ncol], o_t[:])

    def cam_chain():
        """CAM softmax -> attn_c -> M1T = (wbn1 @ attn_c).T + I"""
        EC = state["EC"]
        negmax = campool.tile([C, 1], F32)
        nc.vector.reduce_max(
            negmax[:], EC[0:C, 0:C], axis=mybir.AxisListType.X, negate=True
        )
        exp_c = campool.tile([C, C], F32)
        nc.scalar.activation(exp_c[:], EC[0:C, 0:C], Exp, bias=negmax[:])
        sum_c = campool.tile([C, 1], F32)
        nc.vector.reduce_sum(sum_c[:], exp_c[:], axis=mybir.AxisListType.X)
        rec_c = campool.tile([C, 1], F32)
        nc.vector.reciprocal(rec_c[:], sum_c[:])
        attn_c = campool.tile([C, C], F32)
        nc.vector.tensor_scalar_mul(attn_c[:], exp_c[:], rec_c[:])
        m1ps = spool.tile([128, BLK], F32, tag="s", name="m1ps")
        nc.tensor.matmul(
            m1ps[0:C, 0:C], attn_c[:], wbn1T[:], start=True, stop=True
        )
        with nc.allow_low_precision(reason="M1T in bf16 feeds a bf16 matmul"):
            nc.vector.tensor_add(M1T_sb[:], m1ps[0:C, 0:C], id64[:])

    # ---- main SAM loop over 8 n-blocks, groups emitted in PAIRS ----
    # Block 0 carries wvc + k-chunk fill-ins instead of its DR matmuls
    # (which would blow its PE budget); its E tiles persist in e0 and the
    # 16 deferred DRs ride block 1's slack.  ec runs in blocks 2-3 and the
    # CAM chain at block 4; the vacc/EC vpool rotation is:
    #   vacc0(A) vacc1(B) | vacc2(A) EC(B) | vacc3(A) | CAM, vacc4(B) |
    #   vacc5(A) | vacc6(B) | vacc7(A)
    # each claim one aux-evacuation behind its slot's previous tenant.
    sc_pend = {}
    recip_sched = {(2, 0): 0, (2, 4): 1, (3, 0): 2, (4, 0): 3,
                   (5, 0): 4, (6, 0): 5, (7, 0): 6}
    b1_sched = {(4, 4): 0, (5, 1): 1, (5, 5): 2, (6, 1): 3,
                (6, 5): 4, (7, 1): 5, (7, 4): 6}
    b2_sched = {(4, 6): 0, (5, 3): 1, (5, 7): 2, (6, 3): 3,
                (6, 7): 4, (7, 3): 5, (7, 6): 6}
    for nb in range(NB):
        if nb == 1:
            vaccs[0] = vpool.tile([128, BLK], F32, tag="v", name="vacc0")
        if nb == 4:
            cam_chain()
        if nb != 0:
            vacc = vpool.tile([128, BLK], F32, tag="v", name="vacc")
            vaccs[nb] = vacc
        if nb == 2:
            # EC right after vacc2: slot B, re-claimed by vacc4 after the
            # CAM chain at block 4 has consumed EC.
            state["EC"] = vpool.tile([128, BLK], F32, tag="v", name="EC")
        ncol = slice(nb * BLK, (nb + 1) * BLK)
        for p in range(NG // 2):
            g0, g1 = 2 * p, 2 * p + 1
            s_ts = []
            for g in (g0, g1):
                s_t = spool.tile([128, 2 * BLK], F32, tag="s", name="s_t")
                s_ts.append(s_t)
                for j in range(2):
                    m = 2 * g + j
                    r = 2 * (g % 2) + j  # row quadrants 0,1 / 2,3
                    nc.tensor.matmul(
                        s_t[:, j * BLK : (j + 1) * BLK],
                        k4[32 * r : 32 * r + 32, m * 128 : (m + 1) * 128],
                        q4[32 * r : 32 * r + 32, ncol],
                        start=True,
                        stop=True,
                        tile_position=(32 * r, 0),
                    )
            if nb == 0:
                wvc_group(2 * g0, 2)
                wvc_group(2 * g1, 2)
            if nb in (2, 3):
                ec_group((nb - 2) * NG + 2 * p, 2)
            if (nb, p) in x_cast_sched:
                x_cast(x_cast_sched[(nb, p)])
            for fill in qk_fill.get((nb, p), ()):
                qk_group(*fill, pool=vpool if nb == 0 else None)
            e_ts = []
            for g, s_t in zip((g0, g1), s_ts):
                if nb == 0:
                    e_t = e0[:, 2 * g * BLK : 2 * (g + 1) * BLK]
                else:
                    e_t = epool.tile([128, 2 * BLK], FP8, tag="e", name="e_t")
                e_ts.append(e_t)
                with nc.allow_low_precision(reason="E in fp8: ~1e-4 on out"):
                    nc.scalar.activation(e_t[:], s_t[:], Exp, bias=nlog64[:])
            if nb != 0:
                for g, e_t in zip((g0, g1), e_ts):
                    lhsT = wt8[:, 2 * g * WP : (2 * g + 2) * WP].rearrange(
                        "p (two f) -> p two f", two=2
                    )[:, :, 0:65]
                    rhs = e_t[:].rearrange("p (two f) -> p two f", two=2)
                    nc.tensor.matmul(
                        vaccs[nb][0 : C + 1, :],
                        lhsT,
                        rhs,
                        start=(g == 0),
                        stop=(g == NG - 1),
                        perf_mode=DR,
                    )
            if nb == 1:
                # block 0's deferred DR matmuls, 2 per pair
                for g in (g0, g1):
                    lhsT = wt8[:, 2 * g * WP : (2 * g + 2) * WP].rearrange(
                        "p (two f) -> p two f", two=2
                    )[:, :, 0:65]
                    rhs = e0[:, 2 * g * BLK : 2 * (g + 1) * BLK].rearrange(
                        "p (two f) -> p two f", two=2
                    )
                    nc.tensor.matmul(
                        vaccs[0][0 : C + 1, :],
                        lhsT,
                        rhs,
                        start=(g == 0),
                        stop=(g == NG - 1),
                        perf_mode=DR,
                    )
            if (nb, p) in recip_sched:
                emit_recip(recip_sched[(nb, p)])
            if (nb, p) in b1_sched:
                s = b1_sched[(nb, p)]
                sc_pend[s] = epilogue_b1(s)
            if (nb, p) in b2_sched:
                s = b2_sched[(nb, p)]
                epilogue_b2(s, sc_pend.pop(s))

        if nb == 1:
            epilogue_a(0)
            epilogue_a(1)
        elif nb != 0:
            epilogue_a(nb)
    # ---- tail: only the last block's epilogue chain remains ----
    emit_recip(NB - 1)
    sc7 = epilogue_b1(NB - 1)
    epilogue_b2(NB - 1, sc7)


def build_nc():
    nc = bacc.Bacc(
        "TRN2",
        target_bir_lowering=False,
        debug=False,
        enable_asserts=False,
        num_devices=8,
    )
    io = {}
    io["x"] = nc.dram_tensor("x", [C, HW], F32, kind="ExternalInput").ap()
    io["wq4T"] = nc.dram_tensor("wq4T", [C, 128], BF16, kind="ExternalInput").ap()
    io["wk4T"] = nc.dram_tensor("wk4T", [C, 128], BF16, kind="ExternalInput").ap()
    io["wvc"] = nc.dram_tensor("wvc", [C, 128], BF16, kind="ExternalInput").ap()
    io["wbn1T"] = nc.dram_tensor("wbn1T", [C, C], F32, kind="ExternalInput").ap()
    io["id64"] = nc.dram_tensor("id64", [C, C], BF16, kind="ExternalInput").ap()
    io["out"] = nc.dram_tensor("out", [C, HW], F32, kind="ExternalOutput").ap()

    with tile.TileContext(nc) as tc:
        with ExitStack() as ctx:
            _build_kernel(ctx, tc, io)
    nc.compile()
    return nc


def make_in_maps(x, w_cam, w_q, w_k, w_v, w_bn):
    import ml_dtypes

    f = lambda a: np.ascontiguousarray(np.asarray(a, dtype=np.float32))
    fb = lambda a: np.ascontiguousarray(
        np.asarray(a, dtype=np.float32).astype(ml_dtypes.bfloat16)
    )
    w_bn = np.asarray(w_bn, dtype=np.float64)
    w_vp = w_bn[:, C:] @ np.asarray(w_v, dtype=np.float64)  # wbn2 folded into v
    base = {
        "wq4T": fb(np.concatenate([np.asarray(w_q).T] * 4, axis=1)),
        "wk4T": fb(np.concatenate([np.asarray(w_k).T] * 4, axis=1)),
        "wvc": fb(np.concatenate([w_vp.T, np.asarray(w_cam).T], axis=1)),
        "wbn1T": f(w_bn[:, :C].T),
        "id64": fb(np.eye(C)),
    }
    x = np.asarray(x)
    return [dict(base, x=f(x[b].reshape(C, HW))) for b in range(8)]


_NC_CACHE = None


def kernel(x, w_cam, w_q, w_k, w_v, w_bn):
    global _NC_CACHE
    if _NC_CACHE is None:
        _NC_CACHE = build_nc()
    nc = _NC_CACHE
    in_maps = make_in_maps(x, w_cam, w_q, w_k, w_v, w_bn)
    res = run_bass_kernel_spmd(nc, in_maps, list(range(8)))
    out = np.stack([res.results[b]["out"].reshape(C, 64, 64) for b in range(8)])
    return out.astype(np.float32)


# revision 21
# speedup vs baseline: 1.2433x; 1.0167x over previous
"""Trainium2 Bass kernel for dual-attention block (CAM + SAM + bottleneck).

Contract: kernel(**inputs) takes FULL unsharded inputs
  x     [8, 64, 64, 64] f32
  w_cam [64, 64], w_q [32, 64], w_k [32, 64], w_v [64, 64], w_bn [64, 128]
and returns the full [8, 64, 64, 64] f32 output.

Sharding: data-parallel over batch across 8 NeuronCores (1 image each);
weights replicated. Per-core math (c=64 channels, n=m=4096 spatial):

  CAM: xcT = x.T @ w_cam.T ; Ec = xcT.T @ xcT;
       attn_c = softmax_rows(Ec); bn = ((wbn1 @ attn_c) + I) @ x
       (the +I folds the residual x into the CAM bottleneck matmul)
  SAM: q4/k4 = (w stacked 4x) @ x  -> q,k replicated on 4 partition groups
       S[m,n] = sum_c k[c,m] q[c,n]  (row-tiled K=32 matmuls, 4-concurrent)
       E = exp(S - ln64) in fp8-e4m3  (max|S|=9.05 -> E'max 133 < 240;
           the 1/64 cancels between numerator and denominator)
       acc[c,n] = sum_m W[m,c] E[m,n]  with W = [v'.T | ones] in fp8 and
                  v' = (wbn2 @ w_v) x  (bottleneck conv folded into the
                  value weights on the host), one DoubleRow matmul per
                  m-tile PAIR (K=256) -> rows 0..63 = wbn2-projected SAM
                  contribution (unnormalized), row 64 = Z
  out = bn + acc[0:64] * (1/Z)
        (1/Z via custom-DVE fast reciprocal at partition 0, broadcast to
        64 partitions by GpSimd partition_broadcast)

v8 structure (v4 measured 221us, v7 206us):
  - spool: ONE 3-slot rotation (3 x 2 PSUM banks) for the S tiles.  With
    A/B ping-pong a pair's second S group had to wait for the previous
    exp, serializing the quadrant matmuls 2+2; with 3 slots a group's
    bank is free 3 exp-periods ahead, so all 4 K=32 quadrant matmuls of
    a pair issue back-to-back and run concurrently on disjoint row
    quadrants.  All other PSUM scratch (warm-up, q/k chunk production,
    wvc, bn, m1ps) rides the same rotation; vacc/EC keep 2 banks.
  - wbn2 folded into the DR weights host-side; the per-block bottleneck
    matmul on the SAM path is gone.  The residual +x is folded into the
    CAM bottleneck matmul as (M1+I) via a device identity add.
  - 1/Z: fast approx reciprocal (partition 0, via a 2KB DMA hop) and
    GpSimd partition_broadcast instead of a K=1 PE matmul.
  - Preamble: weight DMAs first on the GpSimd queue; x in 8 chunks over
    3 DMA queues; ~6us dense PE warm-up (HAM -> 2.4GHz) overlapping the
    x DMA; q/k chunks 0-1 as single FD=1024 matmuls right behind it.
PSUM: spool 3x2 + vacc/EC 2 = 8 banks.
"""

import sys
from contextlib import ExitStack

import numpy as np

if "/opt/trn_rl_repo" not in sys.path:
    sys.path.insert(0, "/opt/trn_rl_repo")

import concourse.bass as bass
import concourse.tile as tile
from concourse import bacc, mybir
from concourse.bass_utils import run_bass_kernel_spmd

F32 = mybir.dt.float32
BF16 = mybir.dt.bfloat16
FP8 = mybir.dt.float8e4

C = 64          # channels
HW = 4096       # 64*64 spatial
NB = 8          # number of 512-wide n blocks
BLK = 512
MT = 32         # m tiles of 128
NG = 16         # groups of 2 m-tiles per n-block
WP = 80         # wt8 per-m-tile stride (65 used; 80 for DoubleRow step%16==0)
NLOG64 = -4.1588830833596715

Exp = mybir.ActivationFunctionType.Exp
DR = mybir.MatmulPerfMode.DoubleRow


def _build_kernel(ctx: ExitStack, tc: tile.TileContext, io: dict):
    nc = tc.nc
    x_d = io["x"]
    out_d = io["out"]

    consts = ctx.enter_context(tc.tile_pool(name="consts", bufs=1))
    bigs = ctx.enter_context(tc.tile_pool(name="bigs", bufs=1))
    epool = ctx.enter_context(tc.tile_pool(name="epool", bufs=3))
    campool = ctx.enter_context(tc.tile_pool(name="campool", bufs=1))
    sampool = ctx.enter_context(tc.tile_pool(name="sampool", bufs=4))
    spool = ctx.enter_context(
        tc.tile_pool(name="spool", bufs=3, space=bass.MemorySpace.PSUM)
    )
    vpool = ctx.enter_context(
        tc.tile_pool(name="vpool", bufs=2, space=bass.MemorySpace.PSUM)
    )

    # ---- weight DMAs first, on the otherwise-idle GpSimd queue (tiny; if
    # they queued behind the 1MB x transfer the first matmul waits ~15us) --
    wq4T = consts.tile([C, 128], BF16)    # (w_q stacked 4x).T
    wk4T = consts.tile([C, 128], BF16)
    wvc = consts.tile([C, 128], BF16)     # [(wbn2 w_v).T | w_cam.T]
    wbn1T = consts.tile([C, C], F32)
    id64 = consts.tile([C, C], BF16)
    zb = consts.tile([128, 1], F32)
    nlog64 = consts.tile([128, 1], F32)   # exp bias: E'=E/64 fits fp8e4 max 240
    dummy = consts.tile([128, 1], F32)

    nc.sync.dma_start(wk4T[:], io["wk4T"][:])
    nc.scalar.dma_start(wq4T[:], io["wq4T"][:])
    nc.gpsimd.dma_start(wvc[:], io["wvc"][:])
    nc.gpsimd.dma_start(wbn1T[:], io["wbn1T"][:])
    nc.gpsimd.dma_start(id64[:], io["id64"][:])

    # ---- x DMA: 8 column chunks round-robin over 3 HWDGE queues (each
    # queue sustains only ~100 GB/s; chunk 0 -- all that the first S
    # matmuls need -- lands first, right behind wk4T/wq4T) ----
    x_sb = bigs.tile([C, HW], F32)
    x_qs = [nc.sync, nc.scalar, nc.gpsimd]
    for xc_ in range(8):
        x_qs[xc_ % 3].dma_start(
            x_sb[:, xc_ * BLK : (xc_ + 1) * BLK],
            x_d[:, xc_ * BLK : (xc_ + 1) * BLK],
        )

    nc.vector.memset(zb[:], 0.0)
    # Trigger the exp ACT-table load right behind the x-DMA issue (overlaps
    # the transfer) instead of in front of the first real exp.
    nc.scalar.activation(dummy[:], zb[:], Exp, bias=zb[:])
    nc.vector.memset(nlog64[:], NLOG64)

    q4 = bigs.tile([128, HW], BF16)
    k4 = bigs.tile([128, HW], BF16)
    wt8 = bigs.tile([128, MT * WP], FP8)   # per m-tile [v'T | ones | pad]
    xct = bigs.tile([128, MT * C], BF16)   # xcT, m-tile-major
    x_bf = bigs.tile([C, HW], BF16)
    e0 = bigs.tile([128, NG * 2 * BLK], FP8)  # block-0 E, consumed in block 1

    # ones column of wt8 (wvc copies below only write cols 0..63)
    nc.vector.memset(
        wt8[:].rearrange("p (t c) -> p t c", c=WP)[:, :, 64:65], 1.0
    )

    # x in bf16 feeds the q4/k4/wvc/bn matmuls at full PE rate.  Only
    # chunks 0-1 are cast up front: casts for chunks that arrive later are
    # emitted inside the loop so they cannot head-of-line-block the DVE
    # queue in front of the k0/q0 evacuations.
    def x_cast(xc_):
        nc.vector.tensor_copy(
            x_bf[:, xc_ * BLK : (xc_ + 1) * BLK], x_sb[:, xc_ * BLK : (xc_ + 1) * BLK]
        )

    x_cast(0)
    x_cast(1)
    x_cast_sched = {(0, 0): 2, (0, 1): 3, (0, 2): 4, (0, 3): 5, (0, 4): 6,
                    (0, 5): 7}

    # ---- q4 / k4: replicated q,k via stacked-weight 1x1 convs.  Each
    # 2-chunk group is ONE FD=1024 matmul.  Chunks 0-1 of k and q are
    # produced up front; the rest are fill-in groups inside the block
    # loop, each 2+ pairs ahead of its consumption deadline. ----
    def qk_group(which, cch, nch=1, on_scalar=False, pool=None):
        wT, dst = (wk4T, k4) if which == "k" else (wq4T, q4)
        if pool is None:
            ps = spool.tile([128, nch * BLK], F32, tag="s", name="qkps")
        else:
            ps = pool.tile([128, nch * BLK], F32, tag="v", name="qkps")
        for i in range(nch):
            nc.tensor.matmul(
                ps[:, i * BLK : (i + 1) * BLK],
                wT[:],
                x_bf[:, (cch + i) * BLK : (cch + i + 1) * BLK],
                start=True,
                stop=True,
            )
        lo = cch * BLK
        if on_scalar:
            nc.scalar.copy(dst[:, lo : lo + nch * BLK], ps[:])
        else:
            nc.vector.tensor_copy(dst[:, lo : lo + nch * BLK], ps[:])

    qk_group("k", 0)
    qk_group("q", 0, on_scalar=True)

    # (block, pair) -> (which, chunk); deadlines: k chunk c is consumed
    # at block-0 pair c; q chunk c at block c.
    qk_fill = {
        (0, 0): [("k", 1), ("k", 2)], (0, 1): [("k", 3)],
        (0, 2): [("k", 4)], (0, 3): [("k", 5)], (0, 4): [("k", 6)],
        (0, 5): [("k", 7)], (0, 6): [("q", 1)],
        (1, 5): [("q", 2)], (2, 2): [("q", 3, 2)],
        (3, 2): [("q", 5, 2)], (4, 2): [("q", 7)],
    }

    state = {}  # EC tile, allocated at block 1 start (vpool slot timing)

    def wvc_group(base, size):
        """xcT and WT (=[v'T|ones]) production for one m-tile group."""
        ps_w = vpool.tile([128, BLK], F32, tag="v", name="wvcps")
        for j in range(size):
            m = base + j
            nc.tensor.matmul(
                ps_w[:, j * 128 : (j + 1) * 128],
                x_bf[:, m * 128 : (m + 1) * 128],
                wvc[:],
                start=True,
                stop=True,
            )
        src = ps_w[:, : size * 128].rearrange("p (j c) -> p j c", c=128)
        wt_dst = wt8[:, base * WP : (base + size) * WP].rearrange(
            "p (j c) -> p j c", c=WP
        )
        with nc.allow_low_precision(reason="v' in fp8 for DoubleRow acc"):
            nc.vector.tensor_copy(wt_dst[:, :, 0:C], src[:, :, 0:C])
        xct_dst = xct[:, base * C : (base + size) * C].rearrange(
            "p (j c) -> p j c", c=C
        )
        with nc.allow_low_precision(reason="xcT in bf16 for cheap ec matmuls"):
            nc.vector.tensor_copy(xct_dst, src[:, :, C : 2 * C])

    def ec_group(base, size):
        EC = state["EC"]
        for j in range(size):
            m = base + j
            nc.tensor.matmul(
                EC[0:C, 0:C],
                xct[:, m * C : (m + 1) * C],
                xct[:, m * C : (m + 1) * C],
                start=(m == 0),
                stop=(m == MT - 1),
            )

    # ---- per-block state for split epilogues ----
    vaccs = [None] * NB
    sam = [None] * NB   # sam65 [65, BLK] f32: rows 0..63 unnorm SAM out, 64 = Z
    rzs = [None] * NB   # rz [1, BLK] bf16 at partition 0
    M1T_sb = campool.tile([C, C], BF16)

    def epilogue_a(nb):
        """At block end: evacuate vacc (recip is emitted separately)."""
        aux = sampool.tile([C + 1, BLK], F32, tag="aux", name="aux")
        nc.vector.tensor_copy(aux[:], vaccs[nb][0 : C + 1, :])
        sam[nb] = aux

    def emit_recip(nb):
        """1/Z for block nb via the fast approx recip + bf16 cast.

        The custom DVE op only works at base partition 0 (and DVE lanes
        cannot move data across partitions), so the Z row is first moved
        from partition 64 to partition 0 by a tiny SBUF->SBUF DMA on the
        otherwise-idle sync queue.
        """
        z0 = sampool.tile([1, BLK], F32, tag="z0", name="z0")
        nc.sync.dma_start(z0[:], sam[nb][C : C + 1, :])
        rz32 = sampool.tile([1, BLK], F32, tag="rz32", name="rz32")
        nc.vector.reciprocal_approx_fast(rz32[:], z0[:])
        rzb = sampool.tile([1, BLK], BF16, tag="rz", name="rzb")
        with nc.allow_low_precision(reason="1/Z in bf16: 0.4% on the SAM term"):
            nc.vector.tensor_copy(rzb[:], rz32[:])
        rzs[nb] = rzb

    def epilogue_b1(nb):
        """Broadcast 1/Z to 64 partitions (GpSimd) and scale the SAM rows."""
        bcast = sampool.tile([C, BLK], BF16, tag="bc", name="bcast")
        nc.gpsimd.partition_broadcast(bcast[:], rzs[nb][:])
        sam_sc = sampool.tile([C, BLK], F32, tag="sc", name="sam_sc")
        nc.vector.tensor_mul(sam_sc[:], sam[nb][0:C, :], bcast[:])
        return sam_sc

    def epilogue_b2(nb, sam_sc):
        """CAM bottleneck (+residual via I) matmul, add SAM term, DMA out."""
        ncol = slice(nb * BLK, (nb + 1) * BLK)
        bn = spool.tile([128, BLK], F32, tag="s", name="bn")
        nc.tensor.matmul(
            bn[0:C, :], M1T_sb[:], x_bf[:, ncol], start=True, stop=True
        )
        o_t = sampool.tile([C, BLK], F32, tag="ot", name="o_t")
        nc.vector.tensor_add(o_t[:], bn[0:C, :], sam_sc[:])
        nc.sync.dma_start(out_d[:, ncol], o_t[:])

    def cam_chain():
        """CAM softmax -> attn_c -> M1T = (wbn1 @ attn_c).T + I"""
        EC = state["EC"]
        negmax = campool.tile([C, 1], F32)
        nc.vector.reduce_max(
            negmax[:], EC[0:C, 0:C], axis=mybir.AxisListType.X, negate=True
        )
        exp_c = campool.tile([C, C], F32)
        nc.scalar.activation(exp_c[:], EC[0:C, 0:C], Exp, bias=negmax[:])
        sum_c = campool.tile([C, 1], F32)
        nc.vector.reduce_sum(sum_c[:], exp_c[:], axis=mybir.AxisListType.X)
        rec_c = campool.tile([C, 1], F32)
        nc.vector.reciprocal(rec_c[:], sum_c[:])
        attn_c = campool.tile([C, C], F32)
        nc.vector.tensor_scalar_mul(attn_c[:], exp_c[:], rec_c[:])
        m1ps = spool.tile([128, BLK], F32, tag="s", name="m1ps")
        nc.tensor.matmul(
            m1ps[0:C, 0:C], attn_c[:], wbn1T[:], start=True, stop=True
        )
        with nc.allow_low_precision(reason="M1T in bf16 feeds a bf16 matmul"):
            nc.vector.tensor_add(M1T_sb[:], m1ps[0:C, 0:C], id64[:])

    # ---- main SAM loop over 8 n-blocks, groups emitted in PAIRS ----
    # Block 0 carries wvc + k-chunk fill-ins instead of its DR matmuls
    # (which would blow its PE budget); its E tiles persist in e0 and the
    # 16 deferred DRs ride block 1's slack.  ec runs in blocks 2-3 and the
    # CAM chain at block 4; the vacc/EC vpool rotation is:
    #   vacc0(A) vacc1(B) | vacc2(A) EC(B) | vacc3(A) | CAM, vacc4(B) |
    #   vacc5(A) | vacc6(B) | vacc7(A)
    # each claim one aux-evacuation behind its slot's previous tenant.
    sc_pend = {}
    # ec m-tile coverage: 3/pair on block-2 pairs 0-3, then 2/pair; all 32
    # done by block-3 pair 5 so the CAM chain can run at (3,6).
    ec_sched = {(2, 0): (0, 3), (2, 1): (3, 3), (2, 2): (6, 3),
                (2, 3): (9, 3), (2, 4): (12, 2), (2, 5): (14, 2),
                (2, 6): (16, 2), (2, 7): (18, 2), (3, 0): (20, 2),
                (3, 1): (22, 2), (3, 2): (24, 2), (3, 3): (26, 2),
                (3, 4): (28, 2), (3, 5): (30, 2)}
    recip_sched = {(2, 0): 0, (2, 4): 1, (3, 0): 2, (4, 0): 3,
                   (5, 0): 4, (6, 0): 5, (7, 0): 6}
    b1_sched = {(4, 4): 0, (5, 1): 1, (5, 5): 2, (6, 1): 3,
                (6, 5): 4, (7, 1): 5, (7, 4): 6}
    b2_sched = {(4, 6): 0, (5, 3): 1, (5, 7): 2, (6, 3): 3,
                (6, 7): 4, (7, 3): 5, (7, 6): 6}
    for nb in range(NB):
        if nb == 1:
            vaccs[0] = vpool.tile([128, BLK], F32, tag="v", name="vacc0")
        if nb != 0:
            vacc = vpool.tile([128, BLK], F32, tag="v", name="vacc")
            vaccs[nb] = vacc
        if nb == 2:
            # EC right after vacc2: slot B, re-claimed by vacc4 after the
            # CAM chain at block 4 has consumed EC.
            state["EC"] = vpool.tile([128, BLK], F32, tag="v", name="EC")
        ncol = slice(nb * BLK, (nb + 1) * BLK)
        for p in range(NG // 2):
            g0, g1 = 2 * p, 2 * p + 1
            s_ts = []
            for g in (g0, g1):
                s_t = spool.tile([128, 2 * BLK], F32, tag="s", name="s_t")
                s_ts.append(s_t)
                for j in range(2):
                    m = 2 * g + j
                    r = 2 * (g % 2) + j  # row quadrants 0,1 / 2,3
                    nc.tensor.matmul(
                        s_t[:, j * BLK : (j + 1) * BLK],
                        k4[32 * r : 32 * r + 32, m * 128 : (m + 1) * 128],
                        q4[32 * r : 32 * r + 32, ncol],
                        start=True,
                        stop=True,
                        tile_position=(32 * r, 0),
                    )
            if nb == 0:
                wvc_group(2 * g0, 2)
                wvc_group(2 * g1, 2)
            if (nb, p) in ec_sched:
                ec_group(*ec_sched[(nb, p)])
            if (nb, p) == (3, 6):
                cam_chain()
            if (nb, p) in x_cast_sched:
                x_cast(x_cast_sched[(nb, p)])
            for fill in qk_fill.get((nb, p), ()):
                qk_group(*fill, pool=vpool if nb == 0 else None)
            e_ts = []
            for g, s_t in zip((g0, g1), s_ts):
                if nb == 0:
                    e_t = e0[:, 2 * g * BLK : 2 * (g + 1) * BLK]
                else:
                    e_t = epool.tile([128, 2 * BLK], FP8, tag="e", name="e_t")
                e_ts.append(e_t)
                with nc.allow_low_precision(reason="E in fp8: ~1e-4 on out"):
                    nc.scalar.activation(e_t[:], s_t[:], Exp, bias=nlog64[:])
            if nb != 0:
                for g, e_t in zip((g0, g1), e_ts):
                    lhsT = wt8[:, 2 * g * WP : (2 * g + 2) * WP].rearrange(
                        "p (two f) -> p two f", two=2
                    )[:, :, 0:65]
                    rhs = e_t[:].rearrange("p (two f) -> p two f", two=2)
                    nc.tensor.matmul(
                        vaccs[nb][0 : C + 1, :],
                        lhsT,
                        rhs,
                        start=(g == 0),
                        stop=(g == NG - 1),
                        perf_mode=DR,
                    )
            if nb == 1:
                # block 0's deferred DR matmuls, 2 per pair
                for g in (g0, g1):
                    lhsT = wt8[:, 2 * g * WP : (2 * g + 2) * WP].rearrange(
                        "p (two f) -> p two f", two=2
                    )[:, :, 0:65]
                    rhs = e0[:, 2 * g * BLK : 2 * (g + 1) * BLK].rearrange(
                        "p (two f) -> p two f", two=2
                    )
                    nc.tensor.matmul(
                        vaccs[0][0 : C + 1, :],
                        lhsT,
                        rhs,
                        start=(g == 0),
                        stop=(g == NG - 1),
                        perf_mode=DR,
                    )
            if (nb, p) in recip_sched:
                emit_recip(recip_sched[(nb, p)])
            if (nb, p) in b1_sched:
                s = b1_sched[(nb, p)]
                sc_pend[s] = epilogue_b1(s)
            if (nb, p) in b2_sched:
                s = b2_sched[(nb, p)]
                epilogue_b2(s, sc_pend.pop(s))

        if nb == 1:
            epilogue_a(0)
            epilogue_a(1)
        elif nb != 0:
            epilogue_a(nb)
    # ---- tail: only the last block's epilogue chain remains ----
    emit_recip(NB - 1)
    sc7 = epilogue_b1(NB - 1)
    epilogue_b2(NB - 1, sc7)


def build_nc():
    nc = bacc.Bacc(
        "TRN2",
        target_bir_lowering=False,
        debug=False,
        enable_asserts=False,
        num_devices=8,
    )
    io = {}
    io["x"] = nc.dram_tensor("x", [C, HW], F32, kind="ExternalInput").ap()
    io["wq4T"] = nc.dram_tensor("wq4T", [C, 128], BF16, kind="ExternalInput").ap()
    io["wk4T"] = nc.dram_tensor("wk4T", [C, 128], BF16, kind="ExternalInput").ap()
    io["wvc"] = nc.dram_tensor("wvc", [C, 128], BF16, kind="ExternalInput").ap()
    io["wbn1T"] = nc.dram_tensor("wbn1T", [C, C], F32, kind="ExternalInput").ap()
    io["id64"] = nc.dram_tensor("id64", [C, C], BF16, kind="ExternalInput").ap()
    io["out"] = nc.dram_tensor("out", [C, HW], F32, kind="ExternalOutput").ap()

    with tile.TileContext(nc) as tc:
        with ExitStack() as ctx:
            _build_kernel(ctx, tc, io)
    nc.compile()
    return nc


def make_in_maps(x, w_cam, w_q, w_k, w_v, w_bn):
    import ml_dtypes

    f = lambda a: np.ascontiguousarray(np.asarray(a, dtype=np.float32))
    fb = lambda a: np.ascontiguousarray(
        np.asarray(a, dtype=np.float32).astype(ml_dtypes.bfloat16)
    )
    w_bn = np.asarray(w_bn, dtype=np.float64)
    w_vp = w_bn[:, C:] @ np.asarray(w_v, dtype=np.float64)  # wbn2 folded into v
    base = {
        "wq4T": fb(np.concatenate([np.asarray(w_q).T] * 4, axis=1)),
        "wk4T": fb(np.concatenate([np.asarray(w_k).T] * 4, axis=1)),
        "wvc": fb(np.concatenate([w_vp.T, np.asarray(w_cam).T], axis=1)),
        "wbn1T": f(w_bn[:, :C].T),
        "id64": fb(np.eye(C)),
    }
    x = np.asarray(x)
    return [dict(base, x=f(x[b].reshape(C, HW))) for b in range(8)]


_NC_CACHE = None


def kernel(x, w_cam, w_q, w_k, w_v, w_bn):
    global _NC_CACHE
    if _NC_CACHE is None:
        _NC_CACHE = build_nc()
    nc = _NC_CACHE
    in_maps = make_in_maps(x, w_cam, w_q, w_k, w_v, w_bn)
    res = run_bass_kernel_spmd(nc, in_maps, list(range(8)))
    out = np.stack([res.results[b]["out"].reshape(C, 64, 64) for b in range(8)])
    return out.astype(np.float32)
